# revision 43
# baseline (speedup 1.0000x reference)
"""Transformer-XL multi-head self-attention on 8 Trainium2 NeuronCores.

Sharding: core c handles batch b = c//4 and heads {2*(c%4), 2*(c%4)+1}
(data-parallel over B x tensor-parallel over heads). Each core produces a
partial [N, E] output (its heads' w_o contributions); the host sums the 4
partials per batch element.

The XL relative-position term BD[i,j] = (q_i+v)·BDk[j-i+N-1] is computed
without the rel_shift gather via per-query rotation (angle-difference
identities): BD^T = Psi @ UW with Psi a shape-derived constant basis
(128 exact sin rows + 128 exact cos rows + 64 Chebyshev rows for the slow
frequencies) and UW per-query rotated coefficients.

Scores run on the PE in fp8e4 DoubleRow mode (0.5 cycles/row in the cost
model) with hi/lo error compensation: a bf16-accurate operand x is split
as x = hi + lo with hi = fp8(x), lo = fp8(x - hi), keeping selected cross
terms. Per 128-key tile the contraction is 6 chunks of 128 rows consumed
by 3 DoubleRow calls:
  [sin|cos]x[Uhi|Whi],
  [khi|Thi]x[qhi|chi], [klo|Thi]x[qhi|clo], [khi|Tlo]x[qlo|chi], pad
where T/c are the Chebyshev basis/coefficients and k/q carry the content
term (q+u)·k. The U/W (fast psi coefficient) lo-compensation is dropped
(one-sided both psi and U/W): host-side simulation puts the end-to-end
max-rel error at ~1.4% vs the 2% gate (vs ~1.2% with the compensation).
The value path (exp, V, attn@V, output projection) stays in bf16: fp8
noise there does not average out. exp is spread over Act/DVE/Pool
(Schraudolph on DVE/Pool; the extra Schraudolph noise is ~free: ~1.47%
even if every tile uses it).
"""

import sys

sys.path.insert(0, "/opt/trn_rl_repo")

import ml_dtypes
import numpy as np

import concourse.bass as bass
import concourse.mybir as mybir
from concourse import bacc
from concourse.masks import make_identity
from concourse.tile import TileContext

F32 = mybir.dt.float32
BF16 = mybir.dt.bfloat16
FP8 = mybir.dt.float8e4
I16 = mybir.dt.int16
AF = mybir.ActivationFunctionType
ALU = mybir.AluOpType
DR = mybir.MatmulPerfMode.DoubleRow

B, N, H, E, NH, D = 2, 2048, 2048, 512, 8, 64
HpN = H + N  # 4096
P = 128
NKT = HpN // P  # 32 key tiles
NPAIR = NKT // 2  # 16 key-tile pairs
NQC = N // 512  # 4 query chunks of 512
NEC = E // P  # 4 contraction chunks over E
NS = N // P  # 16 output row tiles
NT = 64  # chebyshev terms
HEADS_PER_CORE = 2
N_CORES = 8

LOG2E = 1.4426950408889634
SCORE_SHIFT = 1.5  # exp(s - c): cancels in softmax, bounds exp values
# exp tile engine rotation: (ctr % MOD) -> r < EXP_ACT on Act (exact),
# rest on DVE (Schraudolph). GPSIMD cannot read PSUM so Pool is out.
# Strict alternation: consecutive units' exps overlap across the two
# engines (each engine sees one ~1.1us exp per two 858ns PE units).
EXP_MOD, EXP_ACT = 2, 1


def build_program():
    nc = bacc.Bacc("TRN2", target_bir_lowering=False, debug=False)

    axT_d = nc.declare_dram_parameter("axT", [E, HpN], BF16, isOutput=False)
    rot_d = nc.declare_dram_parameter("rot", [E, N], BF16, isOutput=False)
    # SgF: shared fast-psi chunks, partition-major [p][t][c][j] so the DMA is
    # an identity layout with 8KB per-partition runs
    psiF_d = nc.declare_dram_parameter("psiF", [P, NKT * 2 * P], FP8, isOutput=False)
    # shared cheb T basis rows [p(64)][hi/lo][t][j]; identical for both heads
    # (placed at opposite partition halves on device)
    psiT_d = nc.declare_dram_parameter("psiT", [NT, 2 * NKT * P], FP8, isOutput=False)
    # fast-psi half-compensation stationary [cos_hi(f0:64)|sin_hi(f64:128)],
    # shared by both heads: [p][t][j]
    psiC_d = nc.declare_dram_parameter("psiC", [P, NKT * P], FP8, isOutput=False)
    sc_d = nc.declare_dram_parameter("sc", [2 * P, NT], BF16, isOutput=False)
    wq2_d = nc.declare_dram_parameter("wq2", [E, P], BF16, isOutput=False)
    wk2_d = nc.declare_dram_parameter("wk2", [E, P], BF16, isOutput=False)
    wv2_d = nc.declare_dram_parameter("wv2", [E, P], BF16, isOutput=False)
    wkrT_d = nc.declare_dram_parameter("wkrT", [P, E], BF16, isOutput=False)
    wo2_d = nc.declare_dram_parameter("wo2", [D, 2 * E], BF16, isOutput=False)
    ub2_d = nc.declare_dram_parameter("ub2", [P, 1], F32, isOutput=False)
    vb2_d = nc.declare_dram_parameter("vb2", [P, 1], F32, isOutput=False)
    out_d = nc.declare_dram_parameter("out", [N, E], BF16, isOutput=True)

    with TileContext(nc) as tc:
        with (
            tc.tile_pool(name="persist", bufs=1) as persist,
            tc.tile_pool(name="gst", bufs=2) as gst,       # G copies stream
            tc.tile_pool(name="mst", bufs=2) as mst,       # rotation temps
            tc.tile_pool(name="est", bufs=6) as est,       # exp tiles
            tc.tile_pool(name="dram", bufs=1, space="DRAM") as dram_pool,
            tc.tile_pool(name="pr", bufs=2, space="PSUM") as pr,   # 2x [P,1024]
            tc.tile_pool(name="ph", bufs=1, space="PSUM") as ph,   # 4x [P,512]
        ):
            _sm = [0]

            def small_psum(shape, name, dtype=F32, tag=None):
                if tag is None:
                    i = _sm[0] % 4
                    _sm[0] += 1
                    tag = f"bank{i}"
                return ph.tile(shape, dtype, tag=tag, name=name)

            # ---------------- DMAs ----------------
            # One prioritized stream on the sync queue: the DMA engines are a
            # serialized resource, so emission order here IS the priority.
            # q proj needs {wq2, x-half}; the uw chain adds {wkr, rot, sc};
            # emit_k(4..7)/emit_v(x) add {wk2, wv2}; history keys come next,
            # then the attention-only psi tables and wo.
            wq2_s = persist.tile([P, NEC, P], BF16, tag="wq2")
            nc.sync.dma_start(wq2_s[:], wq2_d[:].rearrange("(c p) d -> p c d", p=P))
            # wkr stacked on partitions: rows 0:64 = head0 d, 64:128 = head1 d
            wkr_s = persist.tile([P, NEC, P], BF16, tag="wkr")
            nc.sync.dma_start(
                wkr_s[:], wkrT_d[:].rearrange("p (c e) -> p c e", c=NEC)
            )
            ub_s = persist.tile([P, 1], F32, tag="ub")
            nc.sync.dma_start(ub_s[:], ub2_d[:])
            vb_s = persist.tile([P, 1], F32, tag="vb")
            nc.sync.dma_start(vb_s[:], vb2_d[:])
            axT_s = persist.tile([P, NEC, HpN], BF16, tag="axT", name="axT")
            axT = [axT_s[:, c, :] for c in range(NEC)]
            nc.sync.dma_start(
                axT_s[:, :, H:], axT_d[:, H:].rearrange("(c p) k -> p c k", p=P)
            )
            rot_s = persist.tile([P, 4, N], BF16, tag="rot")
            nc.sync.dma_start(rot_s[:, 0, :], rot_d[0:P, :])
            nc.sync.dma_start(rot_s[:, 2, :], rot_d[2 * P : 3 * P, :])
            nc.sync.dma_start(rot_s[:, 1, :], rot_d[P : 2 * P, :])
            nc.sync.dma_start(rot_s[:, 3, :], rot_d[3 * P : 4 * P, :])
            sc_s = persist.tile([P, 2, NT], BF16, tag="sc")
            nc.sync.dma_start(sc_s[:], sc_d[:].rearrange("(k p) r -> p k r", p=P))
            wk2_s = persist.tile([P, NEC, P], BF16, tag="wk2")
            nc.sync.dma_start(wk2_s[:], wk2_d[:].rearrange("(c p) d -> p c d", p=P))
            wv2_s = persist.tile([P, NEC, P], BF16, tag="wv2")
            nc.sync.dma_start(wv2_s[:], wv2_d[:].rearrange("(c p) d -> p c d", p=P))
            nc.sync.dma_start(
                axT_s[:, :, 0:H], axT_d[:, 0:H].rearrange("(c p) k -> p c k", p=P)
            )
            SgF = persist.tile([P, NKT, 2, P], FP8, tag="SgF")
            nc.sync.dma_start(
                SgF[:], psiF_d[:].rearrange("p (t c j) -> p t c j", c=2, j=P)
            )
            # SgA free layout is chunk-major [c][t][j] so partition-sliced
            # chunk DMAs have 4KB contiguous runs.
            SgA = []
            for h in range(HEADS_PER_CORE):
                t = persist.tile([P, 4, NKT, P], FP8, tag=f"SgA{h}", name=f"SgA{h}")
                SgA.append(t)
            for h in range(HEADS_PER_CORE):
                tp = (1 - h) * D
                tps = slice(tp, tp + NT)
                nc.sync.dma_start(
                    SgA[h][tps, 0, :, :],
                    psiT_d[:, 0 : NKT * P].rearrange("p (t j) -> p t j", j=P),
                )
                nc.sync.dma_start(
                    SgA[h][tps, 2, :, :],
                    psiT_d[:, NKT * P :].rearrange("p (t j) -> p t j", j=P),
                )
            for h in range(HEADS_PER_CORE):
                nc.sync.dma_start(
                    SgA[h][:, 3, :, :],
                    psiC_d[:].rearrange("p (t j) -> p t j", j=P),
                )
                # chunk1's T-half duplicates chunk0's (device-side dup)
                tps = slice((1 - h) * D, (1 - h) * D + NT)
                nc.sync.dma_start(SgA[h][tps, 1, :, :], SgA[h][tps, 0, :, :])
            wo_s = persist.tile([D, 2, E], BF16, tag="wo")
            nc.sync.dma_start(wo_s[:], wo2_d[:].rearrange("p (h e) -> p h e", h=2))

            identb = persist.tile([P, P], BF16, tag="identb")
            make_identity(nc, identb[:])

            # ---------------- persistent compute tiles ----------------
            # M chunks per head: 0=Uhi 1=Whi 2=[qhi|chi]
            # 3=[qhi-dup|clo] 4=[qlo|chi-dup] 5=[Wlo(f 0:64)|Ulo(f 64:128)]
            # (chunk 5 pairs with the psiA half-compensation stationary
            # [cos_hi(0:64)|sin_hi(64:128)] in the otherwise-wasted pad slot)
            M = []
            for h in range(HEADS_PER_CORE):
                m = persist.tile([P, 6, NQC, 512], FP8, tag=f"M{h}", name=f"M{h}")
                M.append(m)
            qv_s = persist.tile([P, N], BF16, tag="qv_s")
            vo = []
            for h in range(HEADS_PER_CORE):
                v = persist.tile([P, NKT, 66], BF16, tag=f"vo{h}", name=f"vo{h}")
                nc.gpsimd.memset(v[:, :, 64:66], 0.0)
                nc.gpsimd.memset(v[:, :, 64:65], 1.0)
                vo.append(v)
            numT = []
            numTT = []
            for h in range(HEADS_PER_CORE):
                t = persist.tile([D, N], BF16, tag=f"numT{h}", name=f"numT{h}")
                numT.append(t)
                tt = persist.tile(
                    [P, NS, 65], BF16, tag=f"numTT{h}", name=f"numTT{h}"
                )
                numTT.append(tt)
            out_acc = persist.tile([P, NS, E], BF16, tag="out_acc")
            nbias = persist.tile([P, 1], F32, tag="nbias")
            nc.vector.memset(nbias[:], -SCORE_SHIFT)

            # ---------------- phase A: projections ----------------
            # q projection, both heads packed, emitted chunk-outer so the PE
            # starts as soon as each axT chunk lands
            pqs = [small_psum([P, 512], f"pq{qc}") for qc in range(NQC)]
            for c in range(NEC):
                for qc in range(NQC):
                    nc.tensor.matmul(
                        pqs[qc][:],
                        wq2_s[:, c, :],
                        axT[c][:, H + qc * 512 : H + (qc + 1) * 512],
                        start=(c == 0),
                        stop=(c == NEC - 1),
                    )
            for qc in range(NQC):
                pq = pqs[qc]
                qs = slice(qc * 512, (qc + 1) * 512)
                nc.vector.tensor_scalar_add(qv_s[:, qs], pq[:], vb_s[:])
                for h in range(HEADS_PER_CORE):
                    hp = slice(h * D, (h + 1) * D)
                    nc.vector.tensor_scalar_add(
                        M[h][hp, 2, qc, :], pq[hp, :], ub_s[hp]
                    )
                    nc.vector.scalar_tensor_tensor(
                        M[h][hp, 4, qc, :], pq[hp, :], ub_s[hp],
                        M[h][hp, 2, qc, :], ALU.add, ALU.subtract,
                    )

            def emit_uw_g(h, qc, sfd, ssd):
                hp = slice(h * D, (h + 1) * D)
                qs = slice(qc * 512, (qc + 1) * 512)
                # G: e 0:128 sin-fast + 256:384 cos-fast (sf);
                #    e 128:256 sin-slow + 384:512 cos-slow (ss)
                gf = pr.tile([P, 1024], F32, tag="sp", name="gf")
                nc.tensor.matmul(
                    gf[:, 0:512], wkr_s[hp, 0, :], qv_s[hp, qs],
                    start=True, stop=True,
                )
                nc.tensor.matmul(
                    gf[:, 512:1024], wkr_s[hp, 2, :], qv_s[hp, qs],
                    start=True, stop=True,
                )
                gs = pr.tile([P, 1024], F32, tag="sp", name="gs")
                nc.tensor.matmul(
                    gs[:, 0:512], wkr_s[hp, 1, :], qv_s[hp, qs],
                    start=True, stop=True,
                )
                nc.tensor.matmul(
                    gs[:, 512:1024], wkr_s[hp, 3, :], qv_s[hp, qs],
                    start=True, stop=True,
                )
                nc.scalar.copy(sfd, gf[:])
                nc.scalar.copy(ssd, gs[:])

            def emit_uw_rot(h, qc, sf, ss, usw):
                qs = slice(qc * 512, (qc + 1) * 512)
                # fast half: U = G*cos + Gc*sin ; W = Gc*cos - G*sin
                # (one-sided fp8: no lo chunks, so write M directly)
                m1 = mst.tile([P, 512], BF16, tag="m1")
                m2 = mst.tile([P, 512], BF16, tag="m2")
                m3 = mst.tile([P, 512], BF16, tag="m3")
                m4 = mst.tile([P, 512], BF16, tag="m4")
                nc.vector.tensor_mul(m1[:], sf[:, 0:512], rot_s[:, 0, qs])
                nc.vector.tensor_mul(m2[:], sf[:, 512:1024], rot_s[:, 2, qs])
                nc.vector.tensor_mul(m3[:], sf[:, 512:1024], rot_s[:, 0, qs])
                nc.vector.tensor_mul(m4[:], sf[:, 0:512], rot_s[:, 2, qs])
                ubf = mst.tile([P, 512], BF16, tag="ubf")
                wbf = mst.tile([P, 512], BF16, tag="wbf")
                nc.gpsimd.tensor_add(ubf[:], m1[:], m2[:])
                nc.gpsimd.tensor_sub(wbf[:], m3[:], m4[:])
                nc.vector.tensor_copy(M[h][:, 0, qc, :], ubf[:])
                nc.gpsimd.tensor_copy(M[h][:, 1, qc, :], wbf[:])
                # half lo-comp into the pad slot (partition-aligned halves)
                nc.vector.tensor_sub(
                    M[h][0:D, 5, qc, :], wbf[0:D, :], M[h][0:D, 1, qc, :]
                )
                nc.vector.tensor_sub(
                    M[h][D:P, 5, qc, :], ubf[D:P, :], M[h][D:P, 0, qc, :]
                )
                # slow half: rotate; compression happens in emit_uw_cheb
                m5 = mst.tile([P, 512], BF16, tag="m1", name="m5")
                m6 = mst.tile([P, 512], BF16, tag="m2", name="m6")
                m7 = mst.tile([P, 512], BF16, tag="m3", name="m7")
                m8 = mst.tile([P, 512], BF16, tag="m4", name="m8")
                nc.vector.tensor_mul(m5[:], ss[:, 0:512], rot_s[:, 1, qs])
                nc.vector.tensor_mul(m6[:], ss[:, 512:1024], rot_s[:, 3, qs])
                nc.vector.tensor_mul(m7[:], ss[:, 512:1024], rot_s[:, 1, qs])
                nc.vector.tensor_mul(m8[:], ss[:, 0:512], rot_s[:, 3, qs])
                nc.gpsimd.tensor_add(usw[:, 0, :], m5[:], m6[:])
                nc.gpsimd.tensor_sub(usw[:, 1, :], m7[:], m8[:])

            def emit_uw_cheb(h, qc, usw, tag=None):
                # cheb coefs land on the head's opposite partition half
                po = (1 - h) * D
                cs = slice(po, po + NT)
                pc = small_psum([P, 512], "pc", tag=tag)
                for k in range(2):
                    nc.tensor.matmul(
                        pc[cs, :], sc_s[:, k, :], usw[:, k, :],
                        start=(k == 0), stop=(k == 1),
                    )
                nc.scalar.copy(M[h][cs, 2, qc, :], pc[cs, :])
                nc.vector.tensor_sub(
                    M[h][cs, 3, qc, :], pc[cs, :], M[h][cs, 2, qc, :]
                )

            def emit_k(kc):
                pk = small_psum([P, 512], "pk")
                for c in range(NEC):
                    nc.tensor.matmul(
                        pk[:],
                        wk2_s[:, c, :],
                        axT[c][:, kc * 512 : (kc + 1) * 512],
                        start=(c == 0),
                        stop=(c == NEC - 1),
                    )
                ks = slice(4 * kc, 4 * kc + 4)
                for h in range(HEADS_PER_CORE):
                    hp = slice(h * D, (h + 1) * D)
                    pkv = pk[hp, :].rearrange("p (t j) -> p t j", j=P)
                    nc.scalar.copy(SgA[h][hp, 0, ks, :], pkv)
                    nc.vector.tensor_sub(
                        SgA[h][hp, 1, ks, :], pkv, SgA[h][hp, 0, ks, :]
                    )

            def emit_v(h, g, tag=None):
                hs = slice(h * D, (h + 1) * D)
                pv = small_psum([P, 512], "pv", tag=tag)
                for k8 in range(8):
                    kt = g * 8 + k8
                    for c in range(NEC):
                        nc.tensor.matmul(
                            pv[:, k8 * D : (k8 + 1) * D],
                            axT[c][:, kt * P : (kt + 1) * P],
                            wv2_s[:, c, hs],
                            start=(c == 0),
                            stop=(c == NEC - 1),
                        )
                nc.vector.tensor_copy(
                    vo[h][:, g * 8 : (g + 1) * 8, 0:D],
                    pv[:].rearrange("p (t d) -> p t d", d=D),
                )

            # h0 UW fully in phase A (streaming); h1's G copies land in a
            # persistent tile recycled from axT's tag so h1's rotation
            # (engine-only) can run during h0's attention.
            h1b = persist.tile(
                [P, NQC, 6, 512], BF16, tag="axT", name="h1buf"
            )
            h1buf = [h1b[:, u, :, :] for u in range(NQC)]

            # x keys (kc 4..7, v groups 2..3) first: their axT DMA lands well
            # before the history half.
            for u in range(NQC):
                sf = gst.tile([P, 1024], BF16, tag="sf")
                ss = gst.tile([P, 1024], BF16, tag="ss")
                usw = mst.tile([P, 2, 512], BF16, tag="usw")
                emit_uw_g(0, u, sf[:], ss[:])
                emit_uw_rot(0, u, sf, ss, usw)
                emit_uw_cheb(0, u, usw)
                emit_k(4 + u)
            for g in (2, 3):
                emit_v(0, g)
                emit_v(1, g)
            for u in range(NQC):
                emit_k(u)
            for g in (0, 1):
                emit_v(0, g)
                emit_v(1, g)

            # dups via DMA (off-engine): M chunk 3 q-half <- chunk 2 q-half;
            # M chunk 4 cheb-half <- chunk 2 cheb-half (h0 now, h1 after its
            # cheb block); SgA chunk 2 <- chunk 0
            for h in range(HEADS_PER_CORE):
                hp = slice(h * D, (h + 1) * D)
                nc.sync.dma_start(M[h][hp, 3, :, :], M[h][hp, 2, :, :])
                nc.sync.dma_start(SgA[h][hp, 2, :, :], SgA[h][hp, 0, :, :])
            cs0 = slice(D, D + NT)
            nc.sync.dma_start(M[0][cs0, 4, :, :], M[0][cs0, 2, :, :])

            # ---------------- phase B: attention ----------------
            _expctr = [0]

            def emit_pair(h, pi, avv, pend, qcs=None):
                kt0 = 2 * pi
                for qc in (range(NQC) if qcs is None else qcs):
                    ps = pr.tile([P, 1024], F32, tag="sp", name="ps")
                    for half in range(2):
                        kt = kt0 + half
                        os = slice(half * 512, (half + 1) * 512)
                        nc.tensor.matmul(
                            ps[:, os], SgF[:, kt, :, :], M[h][:, 0:2, qc, :],
                            start=True, stop=False, perf_mode=DR,
                        )
                        nc.tensor.matmul(
                            ps[:, os], SgA[h][:, 0:2, kt, :], M[h][:, 2:4, qc, :],
                            start=False, stop=False, perf_mode=DR,
                        )
                        nc.tensor.matmul(
                            ps[:, os], SgA[h][:, 2:4, kt, :], M[h][:, 4:6, qc, :],
                            start=False, stop=True, perf_mode=DR,
                        )
                    if qc in pend:
                        ppi, pE = pend.pop(qc)
                        for j in range(2):
                            for qt in range(4):
                                qg = qc * 4 + qt
                                bk, sl = divmod(qg, 6)
                                first = ppi == 0 and j == 0 and sl == 0
                                last = (
                                    ppi == NPAIR - 1 and j == 1
                                    and (qg in (5, 11, 15))
                                )
                                nc.tensor.matmul(
                                    avv[bk][:, sl, :],
                                    pE[:, j, qt * P : (qt + 1) * P],
                                    vo[h][:, 2 * ppi + j, 0:65],
                                    start=first, stop=last,
                                    skip_group_check=True,
                                )
                    et = est.tile([P, 2, 512], BF16, tag="E")
                    r = _expctr[0] % EXP_MOD
                    if r < EXP_ACT:
                        nc.scalar.activation(
                            et[:], ps[:], AF.Exp, scale=0.125, bias=nbias[:]
                        )
                    else:
                        # Schraudolph: int16 bits = 128*(log2e*(s/8 - c) + 127)
                        nc.vector.tensor_scalar(
                            et[:].bitcast(I16), ps[:],
                            0.125 * P * LOG2E,
                            P * 127.0 - SCORE_SHIFT * P * LOG2E - 8.5,
                            ALU.mult, ALU.add,
                        )
                    _expctr[0] += 1
                    pend[qc] = (pi, et)

            def emit_av_flush(h, avv, pend):
                for qc, (ppi, pE) in sorted(pend.items()):
                    for j in range(2):
                        for qt in range(4):
                            qg = qc * 4 + qt
                            bk, sl = divmod(qg, 6)
                            nc.tensor.matmul(
                                avv[bk][:, sl, :],
                                pE[:, j, qt * P : (qt + 1) * P],
                                vo[h][:, 2 * ppi + j, 0:65],
                                start=False,
                                stop=(
                                    ppi == NPAIR - 1 and j == 1
                                    and (qg in (5, 11, 15))
                                ),
                                skip_group_check=True,
                            )
                pend.clear()

            def emit_z(h, avv):
                # av is query-major with the ones-column z in slot 64; copy to
                # sbuf and take the per-partition reciprocal directly
                ntt = numTT[h]
                nc.vector.tensor_copy(ntt[:, 0:6, :], avv[0][:])
                nc.vector.tensor_copy(ntt[:, 6:12, :], avv[1][:])
                nc.vector.tensor_copy(ntt[:, 12:16, :], avv[2][:])
                zrec = persist.tile([P, NS], F32, tag=f"zrec{h}", name=f"zrec{h}")
                nc.vector.reciprocal(zrec[:], ntt[:, :, 64])
                return zrec

            def emit_z_tr(h, s, tag=None):
                # transpose one numerator tile back to d-major for the out proj
                pz = small_psum([D, P], "pz", BF16, tag=tag)
                nc.tensor.transpose(pz[:], numTT[h][:, s, 0:D], identb[:])
                nc.scalar.copy(numT[h][:, s * P : (s + 1) * P], pz[:])

            def emit_out_pair(h, sp, zrec):
                pp = pr.tile([P, 1024], F32, tag="sp", name="pp")
                for j in range(2):
                    s = sp + j
                    nc.tensor.matmul(
                        pp[:, j * 512 : (j + 1) * 512],
                        numT[h][0:D, s * P : (s + 1) * P], wo_s[:, h, :],
                        start=True, stop=True,
                    )
                for j in range(2):
                    s = sp + j
                    pj = pp[:, j * 512 : (j + 1) * 512]
                    nc.scalar.activation(
                        out_acc[:, s, :], pj, AF.Copy, scale=zrec[:, s : s + 1]
                    )

            # h0 attention with h1's rotation (engine-only) interleaved
            # av[j]: value accumulators for qtiles 8j..8j+7; avz: denominators
            av0 = [
                ph.tile([P, 6 if j < 2 else 4, 65], F32, tag=f"bank{j}",
                        name=f"av0{j}")
                for j in range(3)
            ]
            pend0 = {}
            for pi in range(NPAIR):
                if pi >= 1:
                    emit_pair(0, pi - 1, av0, pend0, qcs=(3,))
                emit_pair(0, pi, av0, pend0, qcs=(0, 1, 2))
                if pi in (1, 3, 5, 7):
                    u = (pi - 1) // 2
                    emit_uw_g(
                        1, u,
                        h1buf[u][:, 0:2, :].rearrange("p a b -> p (a b)"),
                        h1buf[u][:, 2:4, :].rearrange("p a b -> p (a b)"),
                    )
                if pi in (3, 5, 7, 9):
                    u = (pi - 3) // 2
                    emit_uw_rot(
                        1, u, h1buf[u][:, 0:2, :].rearrange("p a b -> p (a b)"),
                        h1buf[u][:, 2:4, :].rearrange("p a b -> p (a b)"),
                        h1buf[u][:, 4:6, :],
                    )
                if pi in (11, 12, 13, 14):
                    emit_uw_cheb(1, pi - 11, h1buf[pi - 11][:, 4:6, :], tag="bank3")

            emit_pair(0, NPAIR - 1, av0, pend0, qcs=(3,))
            emit_av_flush(0, av0, pend0)
            zrec0 = emit_z(0, av0)
            cs1 = slice(0, NT)
            nc.sync.dma_start(M[1][cs1, 4, :, :], M[1][cs1, 2, :, :])

            # h1 attention with h0's output projection interleaved
            av1 = [
                ph.tile([P, 6 if j < 2 else 4, 65], F32, tag=f"bank{j}",
                        name=f"av1{j}")
                for j in range(3)
            ]
            pend1 = {}
            for pi in range(NPAIR):
                if pi >= 1:
                    emit_pair(1, pi - 1, av1, pend1, qcs=(3,))
                emit_pair(1, pi, av1, pend1, qcs=(0, 1, 2))
                if 1 <= pi <= 8:
                    emit_z_tr(0, 2 * (pi - 1), tag="bank3")
                    emit_z_tr(0, 2 * (pi - 1) + 1, tag="bank3")
                if pi >= 9:
                    emit_out_pair(0, (pi - 9) * 2, zrec0)
            emit_out_pair(0, 14, zrec0)
            emit_pair(1, NPAIR - 1, av1, pend1, qcs=(3,))
            emit_av_flush(1, av1, pend1)
            zrec1 = emit_z(1, av1)
            for s in range(NS):
                emit_z_tr(1, s)
            # h1 out: all matmuls first (buffered over pr + ph banks), then
            # the zrec-gated stores drain as buffers free
            h1bufs = []
            for i, sp in enumerate(range(0, NS, 2)):
                if i % 2 == 0:
                    pp = pr.tile([P, 1024], F32, tag="sp", name="pp")
                    slots = (pp[:, 0:512], pp[:, 512:1024])
                else:
                    sa = small_psum([P, 512], "poa")
                    sb = small_psum([P, 512], "pob")
                    slots = (sa[:], sb[:])
                for j in range(2):
                    s = sp + j
                    nc.tensor.matmul(
                        slots[j], numT[1][0:D, s * P : (s + 1) * P], wo_s[:, 1, :],
                        start=True, stop=True,
                    )
                h1bufs.append((sp, slots))
            for sp, slots in h1bufs:
                for j in range(2):
                    s = sp + j
                    nc.vector.scalar_tensor_tensor(
                        out_acc[:, s, :], slots[j], zrec1[:, s : s + 1],
                        out_acc[:, s, :], ALU.mult, ALU.add,
                    )
                    nc.sync.dma_start(
                        out_d[:].rearrange("(s p) e -> p s e", p=P)[:, s, :],
                        out_acc[:, s, :],
                    )

    nc.compile()
    return nc


_NC_CACHE = None


def _get_program():
    global _NC_CACHE
    if _NC_CACHE is None:
        _NC_CACHE = build_program()
    return _NC_CACHE


def _fp8_hl(x):
    hi = np.clip(np.asarray(x, np.float32), -240, 240).astype(ml_dtypes.float8_e4m3)
    lo = np.clip(
        np.asarray(x, np.float32) - hi.astype(np.float32), -240, 240
    ).astype(ml_dtypes.float8_e4m3)
    return hi, lo


def make_in_maps(x, history, w_q, w_k, w_v, w_kr, w_o, u_bias, v_bias):
    bf = ml_dtypes.bfloat16
    all_x = np.concatenate([history, x], axis=1)  # [B, HpN, E]

    inv_freq = 1.0 / (10000.0 ** (np.arange(0, E, 2, dtype=np.float64) / E))  # [256]
    ang_f = np.outer(inv_freq[:128], np.arange(HpN, dtype=np.float64) - H)
    xn = (np.arange(HpN, dtype=np.float64) - H) / 2048.0
    T = np.polynomial.chebyshev.chebvander(xn, NT - 1)  # [HpN, NT]
    ang_s = np.outer(xn * 2048.0, inv_freq[128:256])  # [HpN, 128]
    tgt = np.concatenate([np.sin(ang_s), np.cos(ang_s)], axis=1)  # [HpN, 256]
    coef, *_ = np.linalg.lstsq(T, tgt, rcond=None)  # [NT, 256]
    sc = np.ascontiguousarray(coef.T)  # [256, NT]: rows 0-127 sin, 128-255 cos

    sin_hi, _ = _fp8_hl(np.sin(ang_f))
    cos_hi, _ = _fp8_hl(np.cos(ang_f))
    T_hi, T_lo = _fp8_hl(T.T)  # [NT, HpN]
    sin_f = sin_hi.astype(np.float32)
    cos_f = cos_hi.astype(np.float32)
    # SgF partition-major: [p][t][c][j], chunks c = [sin_hi, cos_hi]
    psiF = np.ascontiguousarray(
        np.stack(
            [sin_f.reshape(P, NKT, P), cos_f.reshape(P, NKT, P)], axis=2
        ).reshape(P, NKT * 2 * P)
    )
    # shared cheb T basis [p(64)][hi/lo][t][j] (device places it per head)
    psiT = np.ascontiguousarray(
        np.stack(
            [
                T_hi.astype(np.float32).reshape(NT, NKT, P),
                T_lo.astype(np.float32).reshape(NT, NKT, P),
            ],
            axis=1,
        ).reshape(NT, 2 * NKT * P)
    )
    # fast-psi half-compensation stationary [cos_hi(f0:64)|sin_hi(f64:128)]:
    # pairs with M chunk 5 = [Wlo(f0:64)|Ulo(f64:128)]
    psiC = np.ascontiguousarray(
        np.concatenate([cos_f[0:D], sin_f[D:P]], axis=0).reshape(P, NKT * P)
    )

    ang_b = np.outer(inv_freq, np.arange(N, dtype=np.float64))  # [256, N]
    rot = np.ascontiguousarray(
        np.concatenate([np.cos(ang_b), np.sin(ang_b)]).astype(bf)
    )  # [512, N]: rows 0:128 cos-fast, 128:256 cos-slow, 256:384 sin-fast, ...

    clip8 = lambda a: np.clip(a, -240, 240).astype(ml_dtypes.float8_e4m3)

    in_maps = []
    for c in range(N_CORES):
        b = c // 4
        h0 = HEADS_PER_CORE * (c % 4)
        axT = np.ascontiguousarray(all_x[b].T).astype(bf)
        wq2 = np.concatenate([w_q[h0], w_q[h0 + 1]], axis=1).astype(bf)  # [E, 128]
        wk2 = np.concatenate([w_k[h0], w_k[h0 + 1]], axis=1).astype(bf)
        wv2 = np.concatenate([w_v[h0], w_v[h0 + 1]], axis=1).astype(bf)
        wkrT = np.concatenate(
            [w_kr[h0].T, w_kr[h0 + 1].T], axis=0
        ).astype(bf)  # [128, E]: rows 0:64 = head0 (d), 64:128 = head1
        wo2 = np.stack([w_o[h0], w_o[h0 + 1]], axis=1).reshape(D, 2 * E).astype(bf)
        in_maps.append(
            {
                "axT": axT,
                "rot": rot,
                "psiF": clip8(psiF),
                "psiT": clip8(psiT),
                "psiC": clip8(psiC),
                "sc": np.ascontiguousarray(sc).astype(bf),
                "wq2": np.ascontiguousarray(wq2),
                "wk2": np.ascontiguousarray(wk2),
                "wv2": np.ascontiguousarray(wv2),
                "wkrT": np.ascontiguousarray(wkrT),
                "wo2": np.ascontiguousarray(wo2),
                "ub2": np.ascontiguousarray(
                    np.concatenate([u_bias[h0], u_bias[h0 + 1]]).reshape(P, 1)
                ).astype(np.float32),
                "vb2": np.ascontiguousarray(
                    np.concatenate([v_bias[h0], v_bias[h0 + 1]]).reshape(P, 1)
                ).astype(np.float32),
            }
        )
    return in_maps


def run(inputs, trace=False, **kw):
    from concourse.bass_utils import run_bass_kernel_spmd

    nc = _get_program()
    in_maps = make_in_maps(
        np.asarray(inputs["x"], np.float32),
        np.asarray(inputs["history"], np.float32),
        np.asarray(inputs["w_q"], np.float32),
        np.asarray(inputs["w_k"], np.float32),
        np.asarray(inputs["w_v"], np.float32),
        np.asarray(inputs["w_kr"], np.float32),
        np.asarray(inputs["w_o"], np.float32),
        np.asarray(inputs["u_bias"], np.float32),
        np.asarray(inputs["v_bias"], np.float32),
    )
    res = run_bass_kernel_spmd(nc, in_maps, list(range(N_CORES)), trace=trace, **kw)
    out = np.zeros((B, N, E), np.float32)
    for c in range(N_CORES):
        out[c // 4] += res.results[c]["out"].astype(np.float32).reshape(N, E)
    return out, res


def kernel(**inputs):
    # mask is all ones (per the problem spec), so score masking is a no-op
    # and the tensor is ignored.
    out, _ = run(inputs, trace=False)
    return out



# revision 71
# speedup vs baseline: 1.0293x; 1.0293x over previous
"""Transformer-XL multi-head self-attention on 8 Trainium2 NeuronCores.

Sharding: core c handles batch b = c//4 and heads {2*(c%4), 2*(c%4)+1}
(data-parallel over B x tensor-parallel over heads). Each core produces a
partial [N, E] output (its heads' w_o contributions); the host sums the 4
partials per batch element.

The XL relative-position term BD[i,j] = (q_i+v)·BDk[j-i+N-1] is computed
without the rel_shift gather via per-query rotation (angle-difference
identities): BD^T = Psi @ UW with Psi a shape-derived constant basis
(128 exact sin rows + 128 exact cos rows + 64 Chebyshev rows for the slow
frequencies) and UW per-query rotated coefficients.

Scores run on the PE in fp8e4 DoubleRow mode (0.5 cycles/row in the cost
model) with hi/lo error compensation: a bf16-accurate operand x is split
as x = hi + lo with hi = fp8(x), lo = fp8(x - hi), keeping selected cross
terms. Per 128-key tile the contraction is 6 chunks of 128 rows consumed
by 3 DoubleRow calls:
  [sin|cos]x[Uhi|Whi],
  [khi|Thi]x[qhi|chi], [klo|Thi]x[qhi|clo], [khi|Tlo]x[qlo|chi], pad
where T/c are the Chebyshev basis/coefficients and k/q carry the content
term (q+u)·k. The U/W (fast psi coefficient) lo-compensation is dropped
(one-sided both psi and U/W): host-side simulation puts the end-to-end
max-rel error at ~1.4% vs the 2% gate (vs ~1.2% with the compensation).
The value path (exp, V, attn@V, output projection) stays in bf16: fp8
noise there does not average out. exp is spread over Act/DVE/Pool
(Schraudolph on DVE/Pool; the extra Schraudolph noise is ~free: ~1.47%
even if every tile uses it).
"""

import sys

sys.path.insert(0, "/opt/trn_rl_repo")

import ml_dtypes
import numpy as np

import concourse.bass as bass
import concourse.mybir as mybir
from concourse import bacc
from concourse.masks import make_identity
from concourse.tile import TileContext

F32 = mybir.dt.float32
BF16 = mybir.dt.bfloat16
FP8 = mybir.dt.float8e4
I16 = mybir.dt.int16
AF = mybir.ActivationFunctionType
ALU = mybir.AluOpType
DR = mybir.MatmulPerfMode.DoubleRow

B, N, H, E, NH, D = 2, 2048, 2048, 512, 8, 64
HpN = H + N  # 4096
P = 128
NKT = HpN // P  # 32 key tiles
NPAIR = NKT // 2  # 16 key-tile pairs
NQC = N // 512  # 4 query chunks of 512
NEC = E // P  # 4 contraction chunks over E
NS = N // P  # 16 output row tiles
NT = 64  # chebyshev terms
HEADS_PER_CORE = 2
N_CORES = 8

LOG2E = 1.4426950408889634
SCORE_SHIFT = 1.5  # exp(s - c): cancels in softmax, bounds exp values
# exp tile engine rotation: (ctr % MOD) -> r < EXP_ACT on Act (exact),
# rest on DVE (Schraudolph). GPSIMD cannot read PSUM so Pool is out.
# Strict alternation: consecutive units' exps overlap across the two
# engines (each engine sees one ~1.1us exp per two 858ns PE units).
EXP_MOD, EXP_ACT = 2, 1


def build_program():
    nc = bacc.Bacc("TRN2", target_bir_lowering=False, debug=False)

    axT_d = nc.declare_dram_parameter("axT", [E, HpN], BF16, isOutput=False)
    rot_d = nc.declare_dram_parameter("rot", [E, N], BF16, isOutput=False)
    # SgF: shared fast-psi chunks, partition-major [p][t][c][j] so the DMA is
    # an identity layout with 8KB per-partition runs
    psiF_d = nc.declare_dram_parameter("psiF", [P, NKT * 2 * P], FP8, isOutput=False)
    # shared cheb T basis rows [p(64)][hi/lo][t][j]; identical for both heads
    # (placed at opposite partition halves on device)
    psiT_d = nc.declare_dram_parameter("psiT", [NT, 2 * NKT * P], FP8, isOutput=False)
    # fast-psi half-compensation stationary [cos_hi(f0:64)|sin_hi(f64:128)],
    # shared by both heads: [p][t][j]
    psiC_d = nc.declare_dram_parameter("psiC", [P, NKT * P], FP8, isOutput=False)
    sc_d = nc.declare_dram_parameter("sc", [2 * P, NT], BF16, isOutput=False)
    wq2_d = nc.declare_dram_parameter("wq2", [E, P], BF16, isOutput=False)
    wk2_d = nc.declare_dram_parameter("wk2", [E, P], BF16, isOutput=False)
    wv2_d = nc.declare_dram_parameter("wv2", [E, P], BF16, isOutput=False)
    wkrT_d = nc.declare_dram_parameter("wkrT", [P, E], BF16, isOutput=False)
    # wo duplicated on both partition halves (odd numT s-tiles live at 64:128)
    wo2_d = nc.declare_dram_parameter("wo2", [P, 2 * E], BF16, isOutput=False)
    ub2_d = nc.declare_dram_parameter("ub2", [P, 1], F32, isOutput=False)
    vb2_d = nc.declare_dram_parameter("vb2", [P, 1], F32, isOutput=False)
    # two per-head partial outputs (host sums): h0 streams during h1's
    # attention; h1 drains at the tail
    oA_d = nc.declare_dram_parameter("oA", [N, E], BF16, isOutput=True)
    oB_d = nc.declare_dram_parameter("oB", [N, E], BF16, isOutput=True)

    with TileContext(nc) as tc:
        with (
            tc.tile_pool(name="persist", bufs=1) as persist,
            tc.tile_pool(name="gst", bufs=2) as gst,       # G copies stream
            tc.tile_pool(name="mst", bufs=2) as mst,       # rotation temps
            tc.tile_pool(name="est", bufs=6) as est,       # exp tiles
            tc.tile_pool(name="dram", bufs=1, space="DRAM") as dram_pool,
            tc.tile_pool(name="pr", bufs=5, space="PSUM") as pr,   # 5x [P,512]
            tc.tile_pool(name="ph", bufs=1, space="PSUM") as ph,   # 3x [P,512]
        ):
            _sm = [0]

            def small_psum(shape, name, dtype=F32, tag=None):
                if tag is None:
                    i = _sm[0] % 3
                    _sm[0] += 1
                    tag = f"bank{i}"
                return ph.tile(shape, dtype, tag=tag, name=name)

            # ---------------- DMAs ----------------
            # One prioritized stream on the sync queue: the DMA engines are a
            # serialized resource, so emission order here IS the priority.
            # q proj needs {wq2, x-half}; the uw chain adds {wkr, rot, sc};
            # emit_k(4..7)/emit_v(x) add {wk2, wv2}; history keys come next,
            # then the attention-only psi tables and wo.
            wq2_s = persist.tile([P, NEC, P], BF16, tag="wq2")
            nc.sync.dma_start(wq2_s[:], wq2_d[:].rearrange("(c p) d -> p c d", p=P))
            # wkr stacked on partitions: rows 0:64 = head0 d, 64:128 = head1 d
            wkr_s = persist.tile([P, NEC, P], BF16, tag="wkr")
            nc.sync.dma_start(
                wkr_s[:], wkrT_d[:].rearrange("p (c e) -> p c e", c=NEC)
            )
            ub_s = persist.tile([P, 1], F32, tag="ub")
            nc.sync.dma_start(ub_s[:], ub2_d[:])
            vb_s = persist.tile([P, 1], F32, tag="vb")
            nc.sync.dma_start(vb_s[:], vb2_d[:])
            axT_s = persist.tile([P, NEC, HpN], BF16, tag="axT", name="axT")
            axT = [axT_s[:, c, :] for c in range(NEC)]
            nc.sync.dma_start(
                axT_s[:, :, H:], axT_d[:, H:].rearrange("(c p) k -> p c k", p=P)
            )
            rot_s = persist.tile([P, 4, N], BF16, tag="rot")
            nc.sync.dma_start(rot_s[:, 0, :], rot_d[0:P, :])
            nc.sync.dma_start(rot_s[:, 2, :], rot_d[2 * P : 3 * P, :])
            nc.sync.dma_start(rot_s[:, 1, :], rot_d[P : 2 * P, :])
            nc.sync.dma_start(rot_s[:, 3, :], rot_d[3 * P : 4 * P, :])
            sc_s = persist.tile([P, 2, NT], BF16, tag="sc")
            nc.sync.dma_start(sc_s[:], sc_d[:].rearrange("(k p) r -> p k r", p=P))
            wk2_s = persist.tile([P, NEC, P], BF16, tag="wk2")
            nc.sync.dma_start(wk2_s[:], wk2_d[:].rearrange("(c p) d -> p c d", p=P))
            wv2_s = persist.tile([P, NEC, P], BF16, tag="wv2")
            nc.sync.dma_start(wv2_s[:], wv2_d[:].rearrange("(c p) d -> p c d", p=P))
            nc.sync.dma_start(
                axT_s[:, :, 0:H], axT_d[:, 0:H].rearrange("(c p) k -> p c k", p=P)
            )
            SgF = persist.tile([P, NKT, 2, P], FP8, tag="SgF")
            nc.sync.dma_start(
                SgF[:], psiF_d[:].rearrange("p (t c j) -> p t c j", c=2, j=P)
            )
            # SgA free layout is chunk-major [c][t][j] so partition-sliced
            # chunk DMAs have 4KB contiguous runs.
            SgA = []
            for h in range(HEADS_PER_CORE):
                t = persist.tile([P, 4, NKT, P], FP8, tag=f"SgA{h}", name=f"SgA{h}")
                SgA.append(t)
            for h in range(HEADS_PER_CORE):
                tp = (1 - h) * D
                tps = slice(tp, tp + NT)
                nc.sync.dma_start(
                    SgA[h][tps, 0, :, :],
                    psiT_d[:, 0 : NKT * P].rearrange("p (t j) -> p t j", j=P),
                )
                nc.sync.dma_start(
                    SgA[h][tps, 2, :, :],
                    psiT_d[:, NKT * P :].rearrange("p (t j) -> p t j", j=P),
                )
            for h in range(HEADS_PER_CORE):
                nc.sync.dma_start(
                    SgA[h][:, 3, :, :],
                    psiC_d[:].rearrange("p (t j) -> p t j", j=P),
                )
                # chunk1's T-half duplicates chunk0's (device-side dup)
                tps = slice((1 - h) * D, (1 - h) * D + NT)
                nc.sync.dma_start(SgA[h][tps, 1, :, :], SgA[h][tps, 0, :, :])
            wo_s = persist.tile([P, 2, E], BF16, tag="wo")
            nc.sync.dma_start(wo_s[:], wo2_d[:].rearrange("p (h e) -> p h e", h=2))

            identb = persist.tile([P, P], BF16, tag="identb")
            make_identity(nc, identb[:])

            # ---------------- persistent compute tiles ----------------
            # M chunks per head: 0=Uhi 1=Whi 2=[qhi|chi]
            # 3=[qhi-dup|clo] 4=[qlo|chi-dup] 5=[Wlo(f 0:64)|Ulo(f 64:128)]
            # (chunk 5 pairs with the psiA half-compensation stationary
            # [cos_hi(0:64)|sin_hi(64:128)] in the otherwise-wasted pad slot)
            M = []
            for h in range(HEADS_PER_CORE):
                m = persist.tile([P, 6, NQC, 512], FP8, tag=f"M{h}", name=f"M{h}")
                M.append(m)
            qv_s = persist.tile([P, N], BF16, tag="qv_s")
            vo = []
            for h in range(HEADS_PER_CORE):
                v = persist.tile([P, NKT, 66], BF16, tag=f"vo{h}", name=f"vo{h}")
                nc.gpsimd.memset(v[:, :, 64:66], 0.0)
                nc.gpsimd.memset(v[:, :, 64:65], 1.0)
                vo.append(v)
            # numTT: query-major pre-scaled numerators [q, s, d] (z separate);
            # numT: d-major via 128x128 transposes of s-tile PAIRS -- even
            # s-tile's d on partitions 0:64, odd on 64:128
            numT = []
            numTT = []
            zcs = []
            for h in range(HEADS_PER_CORE):
                t = persist.tile(
                    [P, NS // 2, P], BF16, tag=f"numT{h}", name=f"numT{h}"
                )
                numT.append(t)
                tt = persist.tile(
                    [P, NS, D], BF16, tag=f"numTT{h}", name=f"numTT{h}"
                )
                numTT.append(tt)
                zcs.append(
                    persist.tile([P, NS], F32, tag=f"zc{h}", name=f"zc{h}")
                )
            out_acc = persist.tile([P, NS, E], BF16, tag="out_acc")
            nbias = persist.tile([P, 1], F32, tag="nbias")
            nc.vector.memset(nbias[:], -SCORE_SHIFT)

            # ---------------- phase A: projections ----------------
            # q projection, both heads packed, emitted chunk-outer so the PE
            # starts as soon as each axT chunk lands. pq psums use the
            # 1-bank pr slots (score stream is idle in phase A).
            pqs = [
                pr.tile([P, 512], F32, tag="sp", name=f"pq{qc}")
                for qc in range(NQC)
            ]
            for c in range(NEC):
                for qc in range(NQC):
                    nc.tensor.matmul(
                        pqs[qc][:],
                        wq2_s[:, c, :],
                        axT[c][:, H + qc * 512 : H + (qc + 1) * 512],
                        start=(c == 0),
                        stop=(c == NEC - 1),
                    )
            for qc in range(NQC):
                pq = pqs[qc]
                qs = slice(qc * 512, (qc + 1) * 512)
                nc.vector.tensor_scalar_add(qv_s[:, qs], pq[:], vb_s[:])
                for h in range(HEADS_PER_CORE):
                    hp = slice(h * D, (h + 1) * D)
                    nc.vector.tensor_scalar_add(
                        M[h][hp, 2, qc, :], pq[hp, :], ub_s[hp]
                    )
                    nc.vector.scalar_tensor_tensor(
                        M[h][hp, 4, qc, :], pq[hp, :], ub_s[hp],
                        M[h][hp, 2, qc, :], ALU.add, ALU.subtract,
                    )

            def emit_uw_g(h, qc, sfd, ssd):
                hp = slice(h * D, (h + 1) * D)
                qs = slice(qc * 512, (qc + 1) * 512)
                # G: e 0:128 sin-fast + 256:384 cos-fast (sf);
                #    e 128:256 sin-slow + 384:512 cos-slow (ss)
                # four 1-bank psums; sfd/ssd halves get separate copies
                for half, dst in ((0, sfd), (1, ssd)):
                    for j in range(2):
                        g = pr.tile([P, 512], F32, tag="sp", name="g")
                        nc.tensor.matmul(
                            g[:], wkr_s[hp, 2 * j + half, :], qv_s[hp, qs],
                            start=True, stop=True,
                        )
                        nc.scalar.copy(dst[:, j * 512 : (j + 1) * 512], g[:])

            def emit_uw_rot(h, qc, sf, ss, usw):
                qs = slice(qc * 512, (qc + 1) * 512)
                # h0's rotation runs in phase A (DVE has slack); h1's runs
                # during h0's attention, where DVE carries exp -> muls on Pool
                mul = nc.vector if h == 0 else nc.gpsimd
                # fast half: U = G*cos + Gc*sin ; W = Gc*cos - G*sin
                m1 = mst.tile([P, 512], BF16, tag="m1")
                m2 = mst.tile([P, 512], BF16, tag="m2")
                m3 = mst.tile([P, 512], BF16, tag="m3")
                m4 = mst.tile([P, 512], BF16, tag="m4")
                mul.tensor_mul(m1[:], sf[:, 0:512], rot_s[:, 0, qs])
                mul.tensor_mul(m2[:], sf[:, 512:1024], rot_s[:, 2, qs])
                mul.tensor_mul(m3[:], sf[:, 512:1024], rot_s[:, 0, qs])
                mul.tensor_mul(m4[:], sf[:, 0:512], rot_s[:, 2, qs])
                ubf = mst.tile([P, 512], BF16, tag="ubf")
                wbf = mst.tile([P, 512], BF16, tag="wbf")
                nc.gpsimd.tensor_add(ubf[:], m1[:], m2[:])
                nc.gpsimd.tensor_sub(wbf[:], m3[:], m4[:])
                nc.vector.tensor_copy(M[h][:, 0, qc, :], ubf[:])
                nc.gpsimd.tensor_copy(M[h][:, 1, qc, :], wbf[:])
                # half lo-comp into the pad slot (partition-aligned halves)
                nc.vector.tensor_sub(
                    M[h][0:D, 5, qc, :], wbf[0:D, :], M[h][0:D, 1, qc, :]
                )
                nc.vector.tensor_sub(
                    M[h][D:P, 5, qc, :], ubf[D:P, :], M[h][D:P, 0, qc, :]
                )
                # slow half: rotate; compression happens in emit_uw_cheb
                m5 = mst.tile([P, 512], BF16, tag="m1", name="m5")
                m6 = mst.tile([P, 512], BF16, tag="m2", name="m6")
                m7 = mst.tile([P, 512], BF16, tag="m3", name="m7")
                m8 = mst.tile([P, 512], BF16, tag="m4", name="m8")
                mul.tensor_mul(m5[:], ss[:, 0:512], rot_s[:, 1, qs])
                mul.tensor_mul(m6[:], ss[:, 512:1024], rot_s[:, 3, qs])
                mul.tensor_mul(m7[:], ss[:, 512:1024], rot_s[:, 1, qs])
                mul.tensor_mul(m8[:], ss[:, 0:512], rot_s[:, 3, qs])
                nc.gpsimd.tensor_add(usw[:, 0, :], m5[:], m6[:])
                nc.gpsimd.tensor_sub(usw[:, 1, :], m7[:], m8[:])

            def emit_uw_cheb(h, qc, usw, pc=None):
                # cheb coefs land on the head's opposite partition half
                po = (1 - h) * D
                cs = slice(po, po + NT)
                if pc is None:
                    pc = small_psum([P, 512], "pc")
                for k in range(2):
                    nc.tensor.matmul(
                        pc[cs, :], sc_s[:, k, :], usw[:, k, :],
                        start=(k == 0), stop=(k == 1),
                    )
                nc.scalar.copy(M[h][cs, 2, qc, :], pc[cs, :])
                nc.vector.tensor_sub(
                    M[h][cs, 3, qc, :], pc[cs, :], M[h][cs, 2, qc, :]
                )

            def emit_k(kc):
                pk = small_psum([P, 512], "pk")
                for c in range(NEC):
                    nc.tensor.matmul(
                        pk[:],
                        wk2_s[:, c, :],
                        axT[c][:, kc * 512 : (kc + 1) * 512],
                        start=(c == 0),
                        stop=(c == NEC - 1),
                    )
                ks = slice(4 * kc, 4 * kc + 4)
                for h in range(HEADS_PER_CORE):
                    hp = slice(h * D, (h + 1) * D)
                    pkv = pk[hp, :].rearrange("p (t j) -> p t j", j=P)
                    nc.scalar.copy(SgA[h][hp, 0, ks, :], pkv)
                    nc.vector.tensor_sub(
                        SgA[h][hp, 1, ks, :], pkv, SgA[h][hp, 0, ks, :]
                    )

            def emit_v(h, g, tag=None):
                hs = slice(h * D, (h + 1) * D)
                pv = small_psum([P, 512], "pv", tag=tag)
                for k8 in range(8):
                    kt = g * 8 + k8
                    for c in range(NEC):
                        nc.tensor.matmul(
                            pv[:, k8 * D : (k8 + 1) * D],
                            axT[c][:, kt * P : (kt + 1) * P],
                            wv2_s[:, c, hs],
                            start=(c == 0),
                            stop=(c == NEC - 1),
                        )
                nc.vector.tensor_copy(
                    vo[h][:, g * 8 : (g + 1) * 8, 0:D],
                    pv[:].rearrange("p (t d) -> p t d", d=D),
                )

            # h0 UW fully in phase A (streaming); h1's G copies land in a
            # persistent tile recycled from axT's tag so h1's rotation
            # (engine-only) can run during h0's attention.
            h1b = persist.tile(
                [P, NQC, 6, 512], BF16, tag="axT", name="h1buf"
            )
            h1buf = [h1b[:, u, :, :] for u in range(NQC)]

            # x keys (kc 4..7, v groups 2..3) first: their axT DMA lands well
            # before the history half.
            for u in range(NQC):
                sf = gst.tile([P, 1024], BF16, tag="sf")
                ss = gst.tile([P, 1024], BF16, tag="ss")
                usw = mst.tile([P, 2, 512], BF16, tag="usw")
                emit_uw_g(0, u, sf[:], ss[:])
                emit_uw_rot(0, u, sf, ss, usw)
                emit_uw_cheb(0, u, usw)
                emit_k(4 + u)
            for g in (2, 3):
                emit_v(0, g)
                emit_v(1, g)
            for u in range(NQC):
                emit_k(u)
            for g in (0, 1):
                emit_v(0, g)
                emit_v(1, g)

            # dups via DMA (off-engine): M chunk 3 q-half <- chunk 2 q-half;
            # M chunk 4 cheb-half <- chunk 2 cheb-half (h0 now, h1 after its
            # cheb block); SgA chunk 2 <- chunk 0
            for h in range(HEADS_PER_CORE):
                hp = slice(h * D, (h + 1) * D)
                nc.sync.dma_start(M[h][hp, 3, :, :], M[h][hp, 2, :, :])
                nc.sync.dma_start(SgA[h][hp, 2, :, :], SgA[h][hp, 0, :, :])
            cs0 = slice(D, D + NT)
            nc.sync.dma_start(M[0][cs0, 4, :, :], M[0][cs0, 2, :, :])

            # ---------------- phase B: attention ----------------
            # Unit = one (key tile, query chunk): score psum is a 1-bank
            # [P, 512] tile from the 5-deep pr pool, so the
            # ps -> exp -> frees-slot chain never stalls the PE. exp
            # alternates Act (exact) / DVE (Schraudolph) per unit.
            _expctr = [0]

            def emit_av(h, kt, qc, pE, avv):
                for qt in range(4):
                    qg = qc * 4 + qt
                    bk, sl = divmod(qg, 6)
                    nc.tensor.matmul(
                        avv[bk][:, sl, :],
                        pE[:, qt * P : (qt + 1) * P],
                        vo[h][:, kt, 0:65],
                        start=(kt == 0 and qg in (0, 6, 12)),
                        stop=(kt == NKT - 1 and qg in (5, 11, 15)),
                        skip_group_check=True,
                    )

            def emit_unit(h, kt, qc, avv, pend):
                ps = pr.tile([P, 512], F32, tag="sp", name="ps")
                nc.tensor.matmul(
                    ps[:], SgF[:, kt, :, :], M[h][:, 0:2, qc, :],
                    start=True, stop=False, perf_mode=DR,
                )
                nc.tensor.matmul(
                    ps[:], SgA[h][:, 0:2, kt, :], M[h][:, 2:4, qc, :],
                    start=False, stop=False, perf_mode=DR,
                )
                nc.tensor.matmul(
                    ps[:], SgA[h][:, 2:4, kt, :], M[h][:, 4:6, qc, :],
                    start=False, stop=True, perf_mode=DR,
                )
                if qc in pend:
                    pkt, pE = pend.pop(qc)
                    emit_av(h, pkt, qc, pE, avv)
                et = est.tile([P, 512], BF16, tag="E")
                if _expctr[0] % 2 == 0:
                    nc.scalar.activation(
                        et[:], ps[:], AF.Exp, scale=0.125, bias=nbias[:]
                    )
                else:
                    # Schraudolph: int16 bits = 128*(log2e*(s/8 - c) + 127)
                    nc.vector.tensor_scalar(
                        et[:].bitcast(I16), ps[:],
                        0.125 * P * LOG2E,
                        P * 127.0 - SCORE_SHIFT * P * LOG2E - 8.5,
                        ALU.mult, ALU.add,
                    )
                _expctr[0] += 1
                pend[qc] = (kt, et)

            def emit_av_flush(h, avv, pend):
                for qc, (pkt, pE) in sorted(pend.items()):
                    emit_av(h, pkt, qc, pE, avv)
                pend.clear()

            def emit_z(h, avv):
                # av is query-major with the ones-column z in slot 64: copy
                # the z columns, take the reciprocal, then write numTT
                # PRE-SCALED by 1/z (per-partition scalar per s-tile) so the
                # out-projection result needs no further scaling.
                zc = zcs[h]
                nc.vector.tensor_copy(zc[:, 0:6], avv[0][:, :, 64])
                nc.vector.tensor_copy(zc[:, 6:12], avv[1][:, :, 64])
                nc.vector.tensor_copy(zc[:, 12:16], avv[2][:, :, 64])
                zrec = persist.tile([P, NS], F32, tag=f"zrec{h}", name=f"zrec{h}")
                nc.vector.reciprocal(zrec[:], zc[:])
                ntt = numTT[h]
                for s in range(NS):
                    bk, sl = divmod(s, 6)
                    nc.vector.tensor_scalar_mul(
                        ntt[:, s, :], avv[bk][:, sl, 0:D], zrec[:, s : s + 1]
                    )

            def emit_z_tr(h, s2):
                # transpose one PAIR of numerator s-tiles ([128,128] block)
                # back to d-major via the DMA xbar (off-engine)
                nc.sync.dma_start_transpose(
                    numT[h][:, s2, :],
                    numTT[h][:, 2 * s2 : 2 * s2 + 2, :],
                )

            def emit_z_tr_pe(h, s2, copy_eng):
                pz = pr.tile([P, P], BF16, tag="sp", name="pz")
                nc.tensor.transpose(
                    pz[:], numTT[h][:, 2 * s2 : 2 * s2 + 2, :], identb[:]
                )
                copy_eng(numT[h][:, s2, :], pz[:])

            def emit_out_s(h, s):
                # numT is pre-scaled by 1/z, so the psum->sbuf conversion is
                # a plain copy (alternating Act/DVE to spread the load)
                po = pr.tile([P, 512], F32, tag="sp", name="po")
                hp = (s % 2) * D
                nc.tensor.matmul(
                    po[:], numT[h][hp : hp + D, s // 2, :],
                    wo_s[hp : hp + D, h, :],
                    start=True, stop=True,
                )
                if s % 2 == 0:
                    nc.scalar.copy(out_acc[:, s, :], po[:])
                else:
                    nc.vector.tensor_copy(out_acc[:, s, :], po[:])
                od = oA_d if h == 0 else oB_d
                nc.sync.dma_start(
                    od[:].rearrange("(s p) e -> p s e", p=P)[:, s, :],
                    out_acc[:, s, :],
                )

            # h0 attention with h1's G/rotation/cheb interleaved (their
            # elementwise runs on Pool/Act; DVE carries the exp stream)
            av0 = [
                ph.tile([P, 6 if j < 2 else 4, 65], F32, tag=f"bank{j}",
                        name=f"av0{j}")
                for j in range(3)
            ]
            pend0 = {}
            for kt in range(NKT):
                for qc in range(NQC):
                    emit_unit(0, kt, qc, av0, pend0)
                if kt in (2, 5, 8, 11):
                    u = (kt - 2) // 3
                    emit_uw_g(
                        1, u,
                        h1buf[u][:, 0:2, :].rearrange("p a b -> p (a b)"),
                        h1buf[u][:, 2:4, :].rearrange("p a b -> p (a b)"),
                    )
                if kt in (4, 7, 10, 13):
                    u = (kt - 4) // 3
                    emit_uw_rot(
                        1, u, h1buf[u][:, 0:2, :].rearrange("p a b -> p (a b)"),
                        h1buf[u][:, 2:4, :].rearrange("p a b -> p (a b)"),
                        h1buf[u][:, 4:6, :],
                    )
                if kt in (20, 22, 24, 26):
                    # pr slot: the ph banks are held by av0 here (a ph
                    # allocation would deadlock the in-order PE queue)
                    emit_uw_cheb(
                        1, (kt - 20) // 2, h1buf[(kt - 20) // 2][:, 4:6, :],
                        pc=pr.tile([P, 512], F32, tag="sp", name="pc1"),
                    )

            emit_av_flush(0, av0, pend0)
            emit_z(0, av0)
            cs1 = slice(0, NT)
            nc.sync.dma_start(M[1][cs1, 4, :, :], M[1][cs1, 2, :, :])

            # h1 attention with h0's transpose + output projection streamed
            # (out tile s at kt = 6 + 3s//2, i.e. 2 tiles per 3 key tiles)
            _out_sched = {6 + (3 * s) // 2: s for s in range(NS)}
            av1 = [
                ph.tile([P, 6 if j < 2 else 4, 65], F32, tag=f"bank{j}",
                        name=f"av1{j}")
                for j in range(3)
            ]
            pend1 = {}
            for kt in range(NKT):
                for qc in range(NQC):
                    emit_unit(1, kt, qc, av1, pend1)
                if 1 <= kt <= 8:
                    emit_z_tr(0, kt - 1)
                if kt in _out_sched:
                    emit_out_s(0, _out_sched[kt])
            emit_av_flush(1, av1, pend1)
            emit_z(1, av1)
            for s2 in range(NS // 2):
                emit_z_tr_pe(
                    1, s2, nc.vector.tensor_copy if s2 % 2 else nc.scalar.copy
                )
                emit_out_s(1, 2 * s2)
                emit_out_s(1, 2 * s2 + 1)

    nc.compile()
    return nc


_NC_CACHE = None


def _get_program():
    global _NC_CACHE
    if _NC_CACHE is None:
        _NC_CACHE = build_program()
    return _NC_CACHE


def _fp8_hl(x):
    hi = np.clip(np.asarray(x, np.float32), -240, 240).astype(ml_dtypes.float8_e4m3)
    lo = np.clip(
        np.asarray(x, np.float32) - hi.astype(np.float32), -240, 240
    ).astype(ml_dtypes.float8_e4m3)
    return hi, lo


def make_in_maps(x, history, w_q, w_k, w_v, w_kr, w_o, u_bias, v_bias):
    bf = ml_dtypes.bfloat16
    all_x = np.concatenate([history, x], axis=1)  # [B, HpN, E]

    inv_freq = 1.0 / (10000.0 ** (np.arange(0, E, 2, dtype=np.float64) / E))  # [256]
    ang_f = np.outer(inv_freq[:128], np.arange(HpN, dtype=np.float64) - H)
    xn = (np.arange(HpN, dtype=np.float64) - H) / 2048.0
    T = np.polynomial.chebyshev.chebvander(xn, NT - 1)  # [HpN, NT]
    ang_s = np.outer(xn * 2048.0, inv_freq[128:256])  # [HpN, 128]
    tgt = np.concatenate([np.sin(ang_s), np.cos(ang_s)], axis=1)  # [HpN, 256]
    coef, *_ = np.linalg.lstsq(T, tgt, rcond=None)  # [NT, 256]
    sc = np.ascontiguousarray(coef.T)  # [256, NT]: rows 0-127 sin, 128-255 cos

    sin_hi, _ = _fp8_hl(np.sin(ang_f))
    cos_hi, _ = _fp8_hl(np.cos(ang_f))
    T_hi, T_lo = _fp8_hl(T.T)  # [NT, HpN]
    sin_f = sin_hi.astype(np.float32)
    cos_f = cos_hi.astype(np.float32)
    # SgF partition-major: [p][t][c][j], chunks c = [sin_hi, cos_hi]
    psiF = np.ascontiguousarray(
        np.stack(
            [sin_f.reshape(P, NKT, P), cos_f.reshape(P, NKT, P)], axis=2
        ).reshape(P, NKT * 2 * P)
    )
    # shared cheb T basis [p(64)][hi/lo][t][j] (device places it per head)
    psiT = np.ascontiguousarray(
        np.stack(
            [
                T_hi.astype(np.float32).reshape(NT, NKT, P),
                T_lo.astype(np.float32).reshape(NT, NKT, P),
            ],
            axis=1,
        ).reshape(NT, 2 * NKT * P)
    )
    # fast-psi half-compensation stationary [cos_hi(f0:64)|sin_hi(f64:128)]:
    # pairs with M chunk 5 = [Wlo(f0:64)|Ulo(f64:128)]
    psiC = np.ascontiguousarray(
        np.concatenate([cos_f[0:D], sin_f[D:P]], axis=0).reshape(P, NKT * P)
    )

    ang_b = np.outer(inv_freq, np.arange(N, dtype=np.float64))  # [256, N]
    rot = np.ascontiguousarray(
        np.concatenate([np.cos(ang_b), np.sin(ang_b)]).astype(bf)
    )  # [512, N]: rows 0:128 cos-fast, 128:256 cos-slow, 256:384 sin-fast, ...

    clip8 = lambda a: np.clip(a, -240, 240).astype(ml_dtypes.float8_e4m3)

    in_maps = []
    for c in range(N_CORES):
        b = c // 4
        h0 = HEADS_PER_CORE * (c % 4)
        axT = np.ascontiguousarray(all_x[b].T).astype(bf)
        wq2 = np.concatenate([w_q[h0], w_q[h0 + 1]], axis=1).astype(bf)  # [E, 128]
        wk2 = np.concatenate([w_k[h0], w_k[h0 + 1]], axis=1).astype(bf)
        wv2 = np.concatenate([w_v[h0], w_v[h0 + 1]], axis=1).astype(bf)
        wkrT = np.concatenate(
            [w_kr[h0].T, w_kr[h0 + 1].T], axis=0
        ).astype(bf)  # [128, E]: rows 0:64 = head0 (d), 64:128 = head1
        wo1h = np.stack([w_o[h0], w_o[h0 + 1]], axis=1).reshape(D, 2 * E)
        wo2 = np.concatenate([wo1h, wo1h], axis=0).astype(bf)  # [P, 2E]
        in_maps.append(
            {
                "axT": axT,
                "rot": rot,
                "psiF": clip8(psiF),
                "psiT": clip8(psiT),
                "psiC": clip8(psiC),
                "sc": np.ascontiguousarray(sc).astype(bf),
                "wq2": np.ascontiguousarray(wq2),
                "wk2": np.ascontiguousarray(wk2),
                "wv2": np.ascontiguousarray(wv2),
                "wkrT": np.ascontiguousarray(wkrT),
                "wo2": np.ascontiguousarray(wo2),
                "ub2": np.ascontiguousarray(
                    np.concatenate([u_bias[h0], u_bias[h0 + 1]]).reshape(P, 1)
                ).astype(np.float32),
                "vb2": np.ascontiguousarray(
                    np.concatenate([v_bias[h0], v_bias[h0 + 1]]).reshape(P, 1)
                ).astype(np.float32),
            }
        )
    return in_maps


def run(inputs, trace=False, **kw):
    from concourse.bass_utils import run_bass_kernel_spmd

    nc = _get_program()
    in_maps = make_in_maps(
        np.asarray(inputs["x"], np.float32),
        np.asarray(inputs["history"], np.float32),
        np.asarray(inputs["w_q"], np.float32),
        np.asarray(inputs["w_k"], np.float32),
        np.asarray(inputs["w_v"], np.float32),
        np.asarray(inputs["w_kr"], np.float32),
        np.asarray(inputs["w_o"], np.float32),
        np.asarray(inputs["u_bias"], np.float32),
        np.asarray(inputs["v_bias"], np.float32),
    )
    res = run_bass_kernel_spmd(nc, in_maps, list(range(N_CORES)), trace=trace, **kw)
    out = np.zeros((B, N, E), np.float32)
    for c in range(N_CORES):
        out[c // 4] += res.results[c]["oA"].astype(np.float32).reshape(N, E)
        out[c // 4] += res.results[c]["oB"].astype(np.float32).reshape(N, E)
    return out, res


def kernel(**inputs):
    # mask is all ones (per the problem spec), so score masking is a no-op
    # and the tensor is ignored.
    out, _ = run(inputs, trace=False)
    return out



# revision 75
# speedup vs baseline: 1.0792x; 1.0484x over previous
"""Transformer-XL multi-head self-attention on 8 Trainium2 NeuronCores.

Sharding: core c handles batch b = c//4 and heads {2*(c%4), 2*(c%4)+1}
(data-parallel over B x tensor-parallel over heads). Each core produces a
partial [N, E] output (its heads' w_o contributions); the host sums the 4
partials per batch element.

The XL relative-position term BD[i,j] = (q_i+v)·BDk[j-i+N-1] is computed
without the rel_shift gather via per-query rotation (angle-difference
identities): BD^T = Psi @ UW with Psi a shape-derived constant basis
(128 exact sin rows + 128 exact cos rows + 64 Chebyshev rows for the slow
frequencies) and UW per-query rotated coefficients.

Scores run on the PE in fp8e4 DoubleRow mode (0.5 cycles/row in the cost
model) with hi/lo error compensation: a bf16-accurate operand x is split
as x = hi + lo with hi = fp8(x), lo = fp8(x - hi), keeping selected cross
terms. Per 128-key tile the contraction is 6 chunks of 128 rows consumed
by 3 DoubleRow calls:
  [sin|cos]x[Uhi|Whi],
  [khi|Thi]x[qhi|chi], [klo|Thi]x[qhi|clo], [khi|Tlo]x[qlo|chi], pad
where T/c are the Chebyshev basis/coefficients and k/q carry the content
term (q+u)·k. The U/W (fast psi coefficient) lo-compensation is dropped
(one-sided both psi and U/W): host-side simulation puts the end-to-end
max-rel error at ~1.4% vs the 2% gate (vs ~1.2% with the compensation).
The value path (exp, V, attn@V, output projection) stays in bf16: fp8
noise there does not average out. exp is spread over Act/DVE/Pool
(Schraudolph on DVE/Pool; the extra Schraudolph noise is ~free: ~1.47%
even if every tile uses it).
"""

import sys

sys.path.insert(0, "/opt/trn_rl_repo")

import ml_dtypes
import numpy as np

import concourse.bass as bass
import concourse.mybir as mybir
from concourse import bacc
from concourse.masks import make_identity
from concourse.tile import TileContext

F32 = mybir.dt.float32
BF16 = mybir.dt.bfloat16
FP8 = mybir.dt.float8e4
I16 = mybir.dt.int16
AF = mybir.ActivationFunctionType
ALU = mybir.AluOpType
DR = mybir.MatmulPerfMode.DoubleRow

B, N, H, E, NH, D = 2, 2048, 2048, 512, 8, 64
HpN = H + N  # 4096
P = 128
NKT = HpN // P  # 32 key tiles
NPAIR = NKT // 2  # 16 key-tile pairs
NQC = N // 512  # 4 query chunks of 512
NEC = E // P  # 4 contraction chunks over E
NS = N // P  # 16 output row tiles
NT = 64  # chebyshev terms
HEADS_PER_CORE = 2
N_CORES = 8

LOG2E = 1.4426950408889634
SCORE_SHIFT = 1.5  # exp(s - c): cancels in softmax, bounds exp values
# exp tile engine rotation: (ctr % MOD) -> r < EXP_ACT on Act (exact),
# rest on DVE (Schraudolph). GPSIMD cannot read PSUM so Pool is out.
# Strict alternation: consecutive units' exps overlap across the two
# engines (each engine sees one ~1.1us exp per two 858ns PE units).
EXP_MOD, EXP_ACT = 2, 1


def build_program():
    nc = bacc.Bacc("TRN2", target_bir_lowering=False, debug=False)

    axT_d = nc.declare_dram_parameter("axT", [E, HpN], BF16, isOutput=False)
    rot_d = nc.declare_dram_parameter("rot", [E, N], BF16, isOutput=False)
    # SgF: shared fast-psi chunks, partition-major [p][t][c][j] so the DMA is
    # an identity layout with 8KB per-partition runs
    psiF_d = nc.declare_dram_parameter("psiF", [P, NKT * 2 * P], FP8, isOutput=False)
    # shared cheb T basis rows [p(64)][hi/lo][t][j]; identical for both heads
    # (placed at opposite partition halves on device)
    psiT_d = nc.declare_dram_parameter("psiT", [NT, 2 * NKT * P], FP8, isOutput=False)
    # fast-psi half-compensation stationary [cos_hi(f0:64)|sin_hi(f64:128)],
    # shared by both heads: [p][t][j]
    psiC_d = nc.declare_dram_parameter("psiC", [P, NKT * P], FP8, isOutput=False)
    sc_d = nc.declare_dram_parameter("sc", [2 * P, NT], BF16, isOutput=False)
    wq2_d = nc.declare_dram_parameter("wq2", [E, P], BF16, isOutput=False)
    wk2_d = nc.declare_dram_parameter("wk2", [E, P], BF16, isOutput=False)
    wv2_d = nc.declare_dram_parameter("wv2", [E, P], BF16, isOutput=False)
    wkrT_d = nc.declare_dram_parameter("wkrT", [P, E], BF16, isOutput=False)
    # wo duplicated on both partition halves (odd numT s-tiles live at 64:128)
    wo2_d = nc.declare_dram_parameter("wo2", [P, 2 * E], BF16, isOutput=False)
    ub2_d = nc.declare_dram_parameter("ub2", [P, 1], F32, isOutput=False)
    vb2_d = nc.declare_dram_parameter("vb2", [P, 1], F32, isOutput=False)
    # two per-head partial outputs (host sums): h0 streams during h1's
    # attention; h1 drains at the tail
    oA_d = nc.declare_dram_parameter("oA", [N, E], BF16, isOutput=True)
    oB_d = nc.declare_dram_parameter("oB", [N, E], BF16, isOutput=True)

    with TileContext(nc) as tc:
        with (
            tc.tile_pool(name="persist", bufs=1) as persist,
            tc.tile_pool(name="gst", bufs=2) as gst,       # G copies stream
            tc.tile_pool(name="mst", bufs=2) as mst,       # rotation temps
            tc.tile_pool(name="est", bufs=6) as est,       # exp tiles
            tc.tile_pool(name="dram", bufs=1, space="DRAM") as dram_pool,
            tc.tile_pool(name="pr", bufs=5, space="PSUM") as pr,   # 5x [P,512]
            tc.tile_pool(name="ph", bufs=1, space="PSUM") as ph,   # 3x [P,512]
        ):
            _sm = [0]

            def small_psum(shape, name, dtype=F32, tag=None):
                if tag is None:
                    i = _sm[0] % 3
                    _sm[0] += 1
                    tag = f"bank{i}"
                return ph.tile(shape, dtype, tag=tag, name=name)

            # ---------------- DMAs ----------------
            # One prioritized stream on the sync queue: the DMA engines are a
            # serialized resource, so emission order here IS the priority.
            # q proj needs {wq2, x-half}; the uw chain adds {wkr, rot, sc};
            # emit_k(4..7)/emit_v(x) add {wk2, wv2}; history keys come next,
            # then the attention-only psi tables and wo.
            wq2_s = persist.tile([P, NEC, P], BF16, tag="wq2")
            nc.sync.dma_start(wq2_s[:], wq2_d[:].rearrange("(c p) d -> p c d", p=P))
            # wkr stacked on partitions: rows 0:64 = head0 d, 64:128 = head1 d
            wkr_s = persist.tile([P, NEC, P], BF16, tag="wkr")
            nc.sync.dma_start(
                wkr_s[:], wkrT_d[:].rearrange("p (c e) -> p c e", c=NEC)
            )
            ub_s = persist.tile([P, 1], F32, tag="ub")
            nc.sync.dma_start(ub_s[:], ub2_d[:])
            vb_s = persist.tile([P, 1], F32, tag="vb")
            nc.sync.dma_start(vb_s[:], vb2_d[:])
            axT_s = persist.tile([P, NEC, HpN], BF16, tag="axT", name="axT")
            axT = [axT_s[:, c, :] for c in range(NEC)]
            nc.sync.dma_start(
                axT_s[:, :, H:], axT_d[:, H:].rearrange("(c p) k -> p c k", p=P)
            )
            rot_s = persist.tile([P, 4, N], BF16, tag="rot")
            nc.sync.dma_start(rot_s[:, 0, :], rot_d[0:P, :])
            nc.sync.dma_start(rot_s[:, 2, :], rot_d[2 * P : 3 * P, :])
            nc.sync.dma_start(rot_s[:, 1, :], rot_d[P : 2 * P, :])
            nc.sync.dma_start(rot_s[:, 3, :], rot_d[3 * P : 4 * P, :])
            sc_s = persist.tile([P, 2, NT], BF16, tag="sc")
            nc.sync.dma_start(sc_s[:], sc_d[:].rearrange("(k p) r -> p k r", p=P))
            wk2_s = persist.tile([P, NEC, P], BF16, tag="wk2")
            nc.sync.dma_start(wk2_s[:], wk2_d[:].rearrange("(c p) d -> p c d", p=P))
            wv2_s = persist.tile([P, NEC, P], BF16, tag="wv2")
            nc.sync.dma_start(wv2_s[:], wv2_d[:].rearrange("(c p) d -> p c d", p=P))
            nc.sync.dma_start(
                axT_s[:, :, 0:H], axT_d[:, 0:H].rearrange("(c p) k -> p c k", p=P)
            )
            SgF = persist.tile([P, NKT, 2, P], FP8, tag="SgF")
            nc.sync.dma_start(
                SgF[:], psiF_d[:].rearrange("p (t c j) -> p t c j", c=2, j=P)
            )
            # SgA free layout is chunk-major [c][t][j] so partition-sliced
            # chunk DMAs have 4KB contiguous runs.
            SgA = []
            for h in range(HEADS_PER_CORE):
                t = persist.tile([P, 4, NKT, P], FP8, tag=f"SgA{h}", name=f"SgA{h}")
                SgA.append(t)
            for h in range(HEADS_PER_CORE):
                tp = (1 - h) * D
                tps = slice(tp, tp + NT)
                nc.sync.dma_start(
                    SgA[h][tps, 0, :, :],
                    psiT_d[:, 0 : NKT * P].rearrange("p (t j) -> p t j", j=P),
                )
                nc.sync.dma_start(
                    SgA[h][tps, 2, :, :],
                    psiT_d[:, NKT * P :].rearrange("p (t j) -> p t j", j=P),
                )
            for h in range(HEADS_PER_CORE):
                nc.sync.dma_start(
                    SgA[h][:, 3, :, :],
                    psiC_d[:].rearrange("p (t j) -> p t j", j=P),
                )
                # chunk1's T-half duplicates chunk0's (device-side dup)
                tps = slice((1 - h) * D, (1 - h) * D + NT)
                nc.sync.dma_start(SgA[h][tps, 1, :, :], SgA[h][tps, 0, :, :])
            wo_s = persist.tile([P, 2, E], BF16, tag="wo")
            nc.sync.dma_start(wo_s[:], wo2_d[:].rearrange("p (h e) -> p h e", h=2))

            identb = persist.tile([P, P], BF16, tag="identb")
            make_identity(nc, identb[:])

            # ---------------- persistent compute tiles ----------------
            # M chunks per head: 0=Uhi 1=Whi 2=[qhi|chi]
            # 3=[qhi-dup|clo] 4=[qlo|chi-dup] 5=[Wlo(f 0:64)|Ulo(f 64:128)]
            # (chunk 5 pairs with the psiA half-compensation stationary
            # [cos_hi(0:64)|sin_hi(64:128)] in the otherwise-wasted pad slot)
            M = []
            for h in range(HEADS_PER_CORE):
                m = persist.tile([P, 6, NQC, 512], FP8, tag=f"M{h}", name=f"M{h}")
                M.append(m)
            qv_s = persist.tile([P, N], BF16, tag="qv_s")
            vo = []
            for h in range(HEADS_PER_CORE):
                v = persist.tile([P, NKT, 66], BF16, tag=f"vo{h}", name=f"vo{h}")
                nc.gpsimd.memset(v[:, :, 64:66], 0.0)
                nc.gpsimd.memset(v[:, :, 64:65], 1.0)
                vo.append(v)
            # numTT: query-major pre-scaled numerators [q, s, d] (z separate);
            # numT: d-major via 128x128 transposes of s-tile PAIRS -- even
            # s-tile's d on partitions 0:64, odd on 64:128
            numT = []
            numTT = []
            zcs = []
            for h in range(HEADS_PER_CORE):
                t = persist.tile(
                    [P, NS // 2, P], BF16, tag=f"numT{h}", name=f"numT{h}"
                )
                numT.append(t)
                tt = persist.tile(
                    [P, NS, D], BF16, tag=f"numTT{h}", name=f"numTT{h}"
                )
                numTT.append(tt)
                zcs.append(
                    persist.tile([P, NS], F32, tag=f"zc{h}", name=f"zc{h}")
                )
            out_acc = persist.tile([P, NS, E], BF16, tag="out_acc")
            nbias = persist.tile([P, 1], F32, tag="nbias")
            nc.vector.memset(nbias[:], -SCORE_SHIFT)

            # ---------------- phase A: projections ----------------
            # q projection, both heads packed, emitted chunk-outer so the PE
            # starts as soon as each axT chunk lands. pq psums use the
            # 1-bank pr slots (score stream is idle in phase A).
            pqs = [
                pr.tile([P, 512], F32, tag="sp", name=f"pq{qc}")
                for qc in range(NQC)
            ]
            for c in range(NEC):
                for qc in range(NQC):
                    nc.tensor.matmul(
                        pqs[qc][:],
                        wq2_s[:, c, :],
                        axT[c][:, H + qc * 512 : H + (qc + 1) * 512],
                        start=(c == 0),
                        stop=(c == NEC - 1),
                    )
            for qc in range(NQC):
                pq = pqs[qc]
                qs = slice(qc * 512, (qc + 1) * 512)
                nc.vector.tensor_scalar_add(qv_s[:, qs], pq[:], vb_s[:])
                for h in range(HEADS_PER_CORE):
                    hp = slice(h * D, (h + 1) * D)
                    nc.vector.tensor_scalar_add(
                        M[h][hp, 2, qc, :], pq[hp, :], ub_s[hp]
                    )
                    nc.vector.scalar_tensor_tensor(
                        M[h][hp, 4, qc, :], pq[hp, :], ub_s[hp],
                        M[h][hp, 2, qc, :], ALU.add, ALU.subtract,
                    )

            def emit_uw_g(h, qc, sfd, ssd):
                hp = slice(h * D, (h + 1) * D)
                qs = slice(qc * 512, (qc + 1) * 512)
                # G: e 0:128 sin-fast + 256:384 cos-fast (sf);
                #    e 128:256 sin-slow + 384:512 cos-slow (ss)
                # four 1-bank psums; sfd/ssd halves get separate copies
                for half, dst in ((0, sfd), (1, ssd)):
                    for j in range(2):
                        g = pr.tile([P, 512], F32, tag="sp", name="g")
                        nc.tensor.matmul(
                            g[:], wkr_s[hp, 2 * j + half, :], qv_s[hp, qs],
                            start=True, stop=True,
                        )
                        nc.scalar.copy(dst[:, j * 512 : (j + 1) * 512], g[:])

            def emit_uw_rot(h, qc, sf, ss, usw):
                qs = slice(qc * 512, (qc + 1) * 512)
                # muls split DVE/Pool: h0 runs in phase A (Pool idle, DVE has
                # the psum copies) -> 6 DVE / 2 Pool; h1 runs during h0's
                # attention (DVE carries half the exps) -> 4/4
                V, G = nc.vector, nc.gpsimd
                me = (V, G, V, V, V, G, V, V) if h == 0 else (V, G, V, G, V, G, V, G)
                # fast half: U = G*cos + Gc*sin ; W = Gc*cos - G*sin
                m1 = mst.tile([P, 512], BF16, tag="m1")
                m2 = mst.tile([P, 512], BF16, tag="m2")
                m3 = mst.tile([P, 512], BF16, tag="m3")
                m4 = mst.tile([P, 512], BF16, tag="m4")
                me[0].tensor_mul(m1[:], sf[:, 0:512], rot_s[:, 0, qs])
                me[1].tensor_mul(m2[:], sf[:, 512:1024], rot_s[:, 2, qs])
                me[2].tensor_mul(m3[:], sf[:, 512:1024], rot_s[:, 0, qs])
                me[3].tensor_mul(m4[:], sf[:, 0:512], rot_s[:, 2, qs])
                ubf = mst.tile([P, 512], BF16, tag="ubf")
                wbf = mst.tile([P, 512], BF16, tag="wbf")
                nc.gpsimd.tensor_add(ubf[:], m1[:], m2[:])
                nc.gpsimd.tensor_sub(wbf[:], m3[:], m4[:])
                nc.vector.tensor_copy(M[h][:, 0, qc, :], ubf[:])
                nc.gpsimd.tensor_copy(M[h][:, 1, qc, :], wbf[:])
                # half lo-comp into the pad slot (partition-aligned halves)
                nc.vector.tensor_sub(
                    M[h][0:D, 5, qc, :], wbf[0:D, :], M[h][0:D, 1, qc, :]
                )
                nc.vector.tensor_sub(
                    M[h][D:P, 5, qc, :], ubf[D:P, :], M[h][D:P, 0, qc, :]
                )
                # slow half: rotate; compression happens in emit_uw_cheb
                m5 = mst.tile([P, 512], BF16, tag="m1", name="m5")
                m6 = mst.tile([P, 512], BF16, tag="m2", name="m6")
                m7 = mst.tile([P, 512], BF16, tag="m3", name="m7")
                m8 = mst.tile([P, 512], BF16, tag="m4", name="m8")
                me[4].tensor_mul(m5[:], ss[:, 0:512], rot_s[:, 1, qs])
                me[5].tensor_mul(m6[:], ss[:, 512:1024], rot_s[:, 3, qs])
                me[6].tensor_mul(m7[:], ss[:, 512:1024], rot_s[:, 1, qs])
                me[7].tensor_mul(m8[:], ss[:, 0:512], rot_s[:, 3, qs])
                nc.gpsimd.tensor_add(usw[:, 0, :], m5[:], m6[:])
                nc.gpsimd.tensor_sub(usw[:, 1, :], m7[:], m8[:])

            def emit_uw_cheb(h, qc, usw, pc=None):
                # cheb coefs land on the head's opposite partition half
                po = (1 - h) * D
                cs = slice(po, po + NT)
                if pc is None:
                    pc = small_psum([P, 512], "pc")
                for k in range(2):
                    nc.tensor.matmul(
                        pc[cs, :], sc_s[:, k, :], usw[:, k, :],
                        start=(k == 0), stop=(k == 1),
                    )
                nc.scalar.copy(M[h][cs, 2, qc, :], pc[cs, :])
                nc.vector.tensor_sub(
                    M[h][cs, 3, qc, :], pc[cs, :], M[h][cs, 2, qc, :]
                )

            def emit_k(kc):
                pk = small_psum([P, 512], "pk")
                for c in range(NEC):
                    nc.tensor.matmul(
                        pk[:],
                        wk2_s[:, c, :],
                        axT[c][:, kc * 512 : (kc + 1) * 512],
                        start=(c == 0),
                        stop=(c == NEC - 1),
                    )
                ks = slice(4 * kc, 4 * kc + 4)
                for h in range(HEADS_PER_CORE):
                    hp = slice(h * D, (h + 1) * D)
                    pkv = pk[hp, :].rearrange("p (t j) -> p t j", j=P)
                    nc.scalar.copy(SgA[h][hp, 0, ks, :], pkv)
                    nc.vector.tensor_sub(
                        SgA[h][hp, 1, ks, :], pkv, SgA[h][hp, 0, ks, :]
                    )

            def emit_v(h, g, tag=None):
                hs = slice(h * D, (h + 1) * D)
                pv = small_psum([P, 512], "pv", tag=tag)
                for k8 in range(8):
                    kt = g * 8 + k8
                    for c in range(NEC):
                        nc.tensor.matmul(
                            pv[:, k8 * D : (k8 + 1) * D],
                            axT[c][:, kt * P : (kt + 1) * P],
                            wv2_s[:, c, hs],
                            start=(c == 0),
                            stop=(c == NEC - 1),
                        )
                nc.vector.tensor_copy(
                    vo[h][:, g * 8 : (g + 1) * 8, 0:D],
                    pv[:].rearrange("p (t d) -> p t d", d=D),
                )

            # h0 UW fully in phase A (streaming); h1's G copies land in a
            # persistent tile recycled from axT's tag so h1's rotation
            # (engine-only) can run during h0's attention.
            h1b = persist.tile(
                [P, NQC, 6, 512], BF16, tag="axT", name="h1buf"
            )
            h1buf = [h1b[:, u, :, :] for u in range(NQC)]

            # x keys (kc 4..7, v groups 2..3) first: their axT DMA lands well
            # before the history half.
            for u in range(NQC):
                sf = gst.tile([P, 1024], BF16, tag="sf")
                ss = gst.tile([P, 1024], BF16, tag="ss")
                usw = mst.tile([P, 2, 512], BF16, tag="usw")
                emit_uw_g(0, u, sf[:], ss[:])
                emit_uw_rot(0, u, sf, ss, usw)
                emit_uw_cheb(0, u, usw)
                emit_k(4 + u)
            for g in (2, 3):
                emit_v(0, g)
                emit_v(1, g)
            for u in range(NQC):
                emit_k(u)
            for g in (0, 1):
                emit_v(0, g)
                emit_v(1, g)

            # dups via DMA (off-engine): M chunk 3 q-half <- chunk 2 q-half;
            # M chunk 4 cheb-half <- chunk 2 cheb-half (h0 now, h1 after its
            # cheb block); SgA chunk 2 <- chunk 0
            for h in range(HEADS_PER_CORE):
                hp = slice(h * D, (h + 1) * D)
                nc.sync.dma_start(M[h][hp, 3, :, :], M[h][hp, 2, :, :])
                nc.sync.dma_start(SgA[h][hp, 2, :, :], SgA[h][hp, 0, :, :])
            cs0 = slice(D, D + NT)
            nc.sync.dma_start(M[0][cs0, 4, :, :], M[0][cs0, 2, :, :])

            # ---------------- phase B: attention ----------------
            # Unit = one (key tile, query chunk): score psum is a 1-bank
            # [P, 512] tile from the 5-deep pr pool, so the
            # ps -> exp -> frees-slot chain never stalls the PE. exp
            # alternates Act (exact) / DVE (Schraudolph) per unit; during
            # h0's attention DVE also carries h1's rotation, so it only
            # takes 3 of 8 exps there.
            _expctr = [0]
            _dve_exp = {0: (1, 3, 5), 1: (1, 3, 5, 7)}

            def emit_av(h, kt, qc, pE, avv):
                for qt in range(4):
                    qg = qc * 4 + qt
                    bk, sl = divmod(qg, 6)
                    nc.tensor.matmul(
                        avv[bk][:, sl, :],
                        pE[:, qt * P : (qt + 1) * P],
                        vo[h][:, kt, 0:65],
                        start=(kt == 0 and qg in (0, 6, 12)),
                        stop=(kt == NKT - 1 and qg in (5, 11, 15)),
                        skip_group_check=True,
                    )

            def emit_unit(h, kt, qc, avv, pend):
                ps = pr.tile([P, 512], F32, tag="sp", name="ps")
                nc.tensor.matmul(
                    ps[:], SgF[:, kt, :, :], M[h][:, 0:2, qc, :],
                    start=True, stop=False, perf_mode=DR,
                )
                nc.tensor.matmul(
                    ps[:], SgA[h][:, 0:2, kt, :], M[h][:, 2:4, qc, :],
                    start=False, stop=False, perf_mode=DR,
                )
                nc.tensor.matmul(
                    ps[:], SgA[h][:, 2:4, kt, :], M[h][:, 4:6, qc, :],
                    start=False, stop=True, perf_mode=DR,
                )
                if qc in pend:
                    pkt, pE = pend.pop(qc)
                    emit_av(h, pkt, qc, pE, avv)
                et = est.tile([P, 512], BF16, tag="E")
                if _expctr[0] % 8 not in _dve_exp[h]:
                    nc.scalar.activation(
                        et[:], ps[:], AF.Exp, scale=0.125, bias=nbias[:]
                    )
                else:
                    # Schraudolph: int16 bits = 128*(log2e*(s/8 - c) + 127)
                    nc.vector.tensor_scalar(
                        et[:].bitcast(I16), ps[:],
                        0.125 * P * LOG2E,
                        P * 127.0 - SCORE_SHIFT * P * LOG2E - 8.5,
                        ALU.mult, ALU.add,
                    )
                _expctr[0] += 1
                pend[qc] = (kt, et)

            def emit_av_flush(h, avv, pend):
                for qc, (pkt, pE) in sorted(pend.items()):
                    emit_av(h, pkt, qc, pE, avv)
                pend.clear()

            def emit_z(h, avv):
                # av is query-major with the ones-column z in slot 64: copy
                # the z columns, take the reciprocal, then write numTT
                # PRE-SCALED by 1/z (per-partition scalar per s-tile) so the
                # out-projection result needs no further scaling.
                zc = zcs[h]
                nc.vector.tensor_copy(zc[:, 0:6], avv[0][:, :, 64])
                nc.vector.tensor_copy(zc[:, 6:12], avv[1][:, :, 64])
                nc.vector.tensor_copy(zc[:, 12:16], avv[2][:, :, 64])
                zrec = persist.tile([P, NS], F32, tag=f"zrec{h}", name=f"zrec{h}")
                nc.vector.reciprocal(zrec[:], zc[:])
                ntt = numTT[h]
                for s in range(NS):
                    bk, sl = divmod(s, 6)
                    nc.vector.tensor_scalar_mul(
                        ntt[:, s, :], avv[bk][:, sl, 0:D], zrec[:, s : s + 1]
                    )

            def emit_z_tr(h, s2):
                # transpose one PAIR of numerator s-tiles ([128,128] block)
                # back to d-major via the DMA xbar (off-engine)
                nc.sync.dma_start_transpose(
                    numT[h][:, s2, :],
                    numTT[h][:, 2 * s2 : 2 * s2 + 2, :],
                )

            def emit_z_tr_pe(h, s2, copy_eng):
                pz = pr.tile([P, P], BF16, tag="sp", name="pz")
                nc.tensor.transpose(
                    pz[:], numTT[h][:, 2 * s2 : 2 * s2 + 2, :], identb[:]
                )
                copy_eng(numT[h][:, s2, :], pz[:])

            def emit_out_s(h, s):
                # numT is pre-scaled by 1/z, so the psum->sbuf conversion is
                # a plain copy (alternating Act/DVE to spread the load)
                po = pr.tile([P, 512], F32, tag="sp", name="po")
                hp = (s % 2) * D
                nc.tensor.matmul(
                    po[:], numT[h][hp : hp + D, s // 2, :],
                    wo_s[hp : hp + D, h, :],
                    start=True, stop=True,
                )
                if s % 2 == 0:
                    nc.scalar.copy(out_acc[:, s, :], po[:])
                else:
                    nc.vector.tensor_copy(out_acc[:, s, :], po[:])
                od = oA_d if h == 0 else oB_d
                nc.sync.dma_start(
                    od[:].rearrange("(s p) e -> p s e", p=P)[:, s, :],
                    out_acc[:, s, :],
                )

            # h0 attention with h1's G/rotation/cheb interleaved (their
            # elementwise runs on Pool/Act; DVE carries the exp stream)
            av0 = [
                ph.tile([P, 6 if j < 2 else 4, 65], F32, tag=f"bank{j}",
                        name=f"av0{j}")
                for j in range(3)
            ]
            pend0 = {}
            for kt in range(NKT):
                for qc in range(NQC):
                    emit_unit(0, kt, qc, av0, pend0)
                if kt in (1, 8, 15, 22):
                    u = (kt - 1) // 7
                    emit_uw_g(
                        1, u,
                        h1buf[u][:, 0:2, :].rearrange("p a b -> p (a b)"),
                        h1buf[u][:, 2:4, :].rearrange("p a b -> p (a b)"),
                    )
                if kt in (3, 10, 17, 24):
                    u = (kt - 3) // 7
                    emit_uw_rot(
                        1, u, h1buf[u][:, 0:2, :].rearrange("p a b -> p (a b)"),
                        h1buf[u][:, 2:4, :].rearrange("p a b -> p (a b)"),
                        h1buf[u][:, 4:6, :],
                    )
                if kt in (5, 12, 19, 26):
                    # pr slot: the ph banks are held by av0 here (a ph
                    # allocation would deadlock the in-order PE queue)
                    u = (kt - 5) // 7
                    emit_uw_cheb(
                        1, u, h1buf[u][:, 4:6, :],
                        pc=pr.tile([P, 512], F32, tag="sp", name="pc1"),
                    )

            emit_av_flush(0, av0, pend0)
            emit_z(0, av0)
            cs1 = slice(0, NT)
            nc.sync.dma_start(M[1][cs1, 4, :, :], M[1][cs1, 2, :, :])

            # h1 attention with h0's transpose + output projection streamed
            # (out tile s at kt = 6 + 3s//2, i.e. 2 tiles per 3 key tiles)
            _out_sched = {6 + (3 * s) // 2: s for s in range(NS)}
            av1 = [
                ph.tile([P, 6 if j < 2 else 4, 65], F32, tag=f"bank{j}",
                        name=f"av1{j}")
                for j in range(3)
            ]
            pend1 = {}
            for kt in range(NKT):
                for qc in range(NQC):
                    emit_unit(1, kt, qc, av1, pend1)
                if 1 <= kt <= 8:
                    emit_z_tr(0, kt - 1)
                if kt in _out_sched:
                    emit_out_s(0, _out_sched[kt])
            emit_av_flush(1, av1, pend1)
            emit_z(1, av1)
            for s2 in range(NS // 2):
                emit_z_tr_pe(
                    1, s2, nc.vector.tensor_copy if s2 % 2 else nc.scalar.copy
                )
                emit_out_s(1, 2 * s2)
                emit_out_s(1, 2 * s2 + 1)

    nc.compile()
    return nc


_NC_CACHE = None


def _get_program():
    global _NC_CACHE
    if _NC_CACHE is None:
        _NC_CACHE = build_program()
    return _NC_CACHE


def _fp8_hl(x):
    hi = np.clip(np.asarray(x, np.float32), -240, 240).astype(ml_dtypes.float8_e4m3)
    lo = np.clip(
        np.asarray(x, np.float32) - hi.astype(np.float32), -240, 240
    ).astype(ml_dtypes.float8_e4m3)
    return hi, lo


def make_in_maps(x, history, w_q, w_k, w_v, w_kr, w_o, u_bias, v_bias):
    bf = ml_dtypes.bfloat16
    all_x = np.concatenate([history, x], axis=1)  # [B, HpN, E]

    inv_freq = 1.0 / (10000.0 ** (np.arange(0, E, 2, dtype=np.float64) / E))  # [256]
    ang_f = np.outer(inv_freq[:128], np.arange(HpN, dtype=np.float64) - H)
    xn = (np.arange(HpN, dtype=np.float64) - H) / 2048.0
    T = np.polynomial.chebyshev.chebvander(xn, NT - 1)  # [HpN, NT]
    ang_s = np.outer(xn * 2048.0, inv_freq[128:256])  # [HpN, 128]
    tgt = np.concatenate([np.sin(ang_s), np.cos(ang_s)], axis=1)  # [HpN, 256]
    coef, *_ = np.linalg.lstsq(T, tgt, rcond=None)  # [NT, 256]
    sc = np.ascontiguousarray(coef.T)  # [256, NT]: rows 0-127 sin, 128-255 cos

    sin_hi, _ = _fp8_hl(np.sin(ang_f))
    cos_hi, _ = _fp8_hl(np.cos(ang_f))
    T_hi, T_lo = _fp8_hl(T.T)  # [NT, HpN]
    sin_f = sin_hi.astype(np.float32)
    cos_f = cos_hi.astype(np.float32)
    # SgF partition-major: [p][t][c][j], chunks c = [sin_hi, cos_hi]
    psiF = np.ascontiguousarray(
        np.stack(
            [sin_f.reshape(P, NKT, P), cos_f.reshape(P, NKT, P)], axis=2
        ).reshape(P, NKT * 2 * P)
    )
    # shared cheb T basis [p(64)][hi/lo][t][j] (device places it per head)
    psiT = np.ascontiguousarray(
        np.stack(
            [
                T_hi.astype(np.float32).reshape(NT, NKT, P),
                T_lo.astype(np.float32).reshape(NT, NKT, P),
            ],
            axis=1,
        ).reshape(NT, 2 * NKT * P)
    )
    # fast-psi half-compensation stationary [cos_hi(f0:64)|sin_hi(f64:128)]:
    # pairs with M chunk 5 = [Wlo(f0:64)|Ulo(f64:128)]
    psiC = np.ascontiguousarray(
        np.concatenate([cos_f[0:D], sin_f[D:P]], axis=0).reshape(P, NKT * P)
    )

    ang_b = np.outer(inv_freq, np.arange(N, dtype=np.float64))  # [256, N]
    rot = np.ascontiguousarray(
        np.concatenate([np.cos(ang_b), np.sin(ang_b)]).astype(bf)
    )  # [512, N]: rows 0:128 cos-fast, 128:256 cos-slow, 256:384 sin-fast, ...

    clip8 = lambda a: np.clip(a, -240, 240).astype(ml_dtypes.float8_e4m3)

    in_maps = []
    for c in range(N_CORES):
        b = c // 4
        h0 = HEADS_PER_CORE * (c % 4)
        axT = np.ascontiguousarray(all_x[b].T).astype(bf)
        wq2 = np.concatenate([w_q[h0], w_q[h0 + 1]], axis=1).astype(bf)  # [E, 128]
        wk2 = np.concatenate([w_k[h0], w_k[h0 + 1]], axis=1).astype(bf)
        wv2 = np.concatenate([w_v[h0], w_v[h0 + 1]], axis=1).astype(bf)
        wkrT = np.concatenate(
            [w_kr[h0].T, w_kr[h0 + 1].T], axis=0
        ).astype(bf)  # [128, E]: rows 0:64 = head0 (d), 64:128 = head1
        wo1h = np.stack([w_o[h0], w_o[h0 + 1]], axis=1).reshape(D, 2 * E)
        wo2 = np.concatenate([wo1h, wo1h], axis=0).astype(bf)  # [P, 2E]
        in_maps.append(
            {
                "axT": axT,
                "rot": rot,
                "psiF": clip8(psiF),
                "psiT": clip8(psiT),
                "psiC": clip8(psiC),
                "sc": np.ascontiguousarray(sc).astype(bf),
                "wq2": np.ascontiguousarray(wq2),
                "wk2": np.ascontiguousarray(wk2),
                "wv2": np.ascontiguousarray(wv2),
                "wkrT": np.ascontiguousarray(wkrT),
                "wo2": np.ascontiguousarray(wo2),
                "ub2": np.ascontiguousarray(
                    np.concatenate([u_bias[h0], u_bias[h0 + 1]]).reshape(P, 1)
                ).astype(np.float32),
                "vb2": np.ascontiguousarray(
                    np.concatenate([v_bias[h0], v_bias[h0 + 1]]).reshape(P, 1)
                ).astype(np.float32),
            }
        )
    return in_maps


def run(inputs, trace=False, **kw):
    from concourse.bass_utils import run_bass_kernel_spmd

    nc = _get_program()
    in_maps = make_in_maps(
        np.asarray(inputs["x"], np.float32),
        np.asarray(inputs["history"], np.float32),
        np.asarray(inputs["w_q"], np.float32),
        np.asarray(inputs["w_k"], np.float32),
        np.asarray(inputs["w_v"], np.float32),
        np.asarray(inputs["w_kr"], np.float32),
        np.asarray(inputs["w_o"], np.float32),
        np.asarray(inputs["u_bias"], np.float32),
        np.asarray(inputs["v_bias"], np.float32),
    )
    res = run_bass_kernel_spmd(nc, in_maps, list(range(N_CORES)), trace=trace, **kw)
    out = np.zeros((B, N, E), np.float32)
    for c in range(N_CORES):
        out[c // 4] += res.results[c]["oA"].astype(np.float32).reshape(N, E)
        out[c // 4] += res.results[c]["oB"].astype(np.float32).reshape(N, E)
    return out, res


def kernel(**inputs):
    # mask is all ones (per the problem spec), so score masking is a no-op
    # and the tensor is ignored.
    out, _ = run(inputs, trace=False)
    return out



# revision 82
# speedup vs baseline: 1.1268x; 1.0441x over previous
"""Transformer-XL multi-head self-attention on 8 Trainium2 NeuronCores.

Sharding: core c handles batch b = c//4 and heads {2*(c%4), 2*(c%4)+1}
(data-parallel over B x tensor-parallel over heads). Each core produces a
partial [N, E] output (its heads' w_o contributions); the host sums the 4
partials per batch element.

The XL relative-position term BD[i,j] = (q_i+v)·BDk[j-i+N-1] is computed
without the rel_shift gather via per-query rotation (angle-difference
identities): BD^T = Psi @ UW with Psi a shape-derived constant basis
(128 exact sin rows + 128 exact cos rows + 64 Chebyshev rows for the slow
frequencies) and UW per-query rotated coefficients.

Scores run on the PE in fp8e4 DoubleRow mode (0.5 cycles/row in the cost
model) with hi/lo error compensation: a bf16-accurate operand x is split
as x = hi + lo with hi = fp8(x), lo = fp8(x - hi), keeping selected cross
terms. Per 128-key tile the contraction is 6 chunks of 128 rows consumed
by 3 DoubleRow calls:
  [sin|cos]x[Uhi|Whi],
  [khi|Thi]x[qhi|chi], [klo|Thi]x[qhi|clo], [khi|Tlo]x[qlo|chi], pad
where T/c are the Chebyshev basis/coefficients and k/q carry the content
term (q+u)·k. The U/W (fast psi coefficient) lo-compensation is dropped
(one-sided both psi and U/W): host-side simulation puts the end-to-end
max-rel error at ~1.4% vs the 2% gate (vs ~1.2% with the compensation).
The value path (exp, V, attn@V, output projection) stays in bf16: fp8
noise there does not average out. exp is spread over Act/DVE/Pool
(Schraudolph on DVE/Pool; the extra Schraudolph noise is ~free: ~1.47%
even if every tile uses it).
"""

import sys

sys.path.insert(0, "/opt/trn_rl_repo")

import ml_dtypes
import numpy as np

import concourse.bass as bass
import concourse.mybir as mybir
from concourse import bacc
from concourse.masks import make_identity
from concourse.tile import TileContext

F32 = mybir.dt.float32
BF16 = mybir.dt.bfloat16
FP8 = mybir.dt.float8e4
I16 = mybir.dt.int16
AF = mybir.ActivationFunctionType
ALU = mybir.AluOpType
DR = mybir.MatmulPerfMode.DoubleRow

B, N, H, E, NH, D = 2, 2048, 2048, 512, 8, 64
HpN = H + N  # 4096
P = 128
NKT = HpN // P  # 32 key tiles
NPAIR = NKT // 2  # 16 key-tile pairs
NQC = N // 512  # 4 query chunks of 512
NEC = E // P  # 4 contraction chunks over E
NS = N // P  # 16 output row tiles
NT = 64  # chebyshev terms
HEADS_PER_CORE = 2
N_CORES = 8

LOG2E = 1.4426950408889634
SCORE_SHIFT = 1.5  # exp(s - c): cancels in softmax, bounds exp values
# exp tile engine rotation: (ctr % MOD) -> r < EXP_ACT on Act (exact),
# rest on DVE (Schraudolph). GPSIMD cannot read PSUM so Pool is out.
# Strict alternation: consecutive units' exps overlap across the two
# engines (each engine sees one ~1.1us exp per two 858ns PE units).
EXP_MOD, EXP_ACT = 2, 1


def build_program():
    nc = bacc.Bacc("TRN2", target_bir_lowering=False, debug=False)

    axT_d = nc.declare_dram_parameter("axT", [E, HpN], BF16, isOutput=False)
    rot_d = nc.declare_dram_parameter("rot", [E, N], BF16, isOutput=False)
    # SgF: shared fast-psi chunks, partition-major [p][t][c][j] so the DMA is
    # an identity layout with 8KB per-partition runs
    psiF_d = nc.declare_dram_parameter("psiF", [P, NKT * 2 * P], FP8, isOutput=False)
    # shared cheb T basis rows [p(64)][hi/lo][t][j]; identical for both heads
    # (placed at opposite partition halves on device)
    psiT_d = nc.declare_dram_parameter("psiT", [NT, 2 * NKT * P], FP8, isOutput=False)
    # fast-psi half-compensation stationary [cos_hi(f0:64)|sin_hi(f64:128)],
    # shared by both heads: [p][t][j]
    psiC_d = nc.declare_dram_parameter("psiC", [P, NKT * P], FP8, isOutput=False)
    sc_d = nc.declare_dram_parameter("sc", [2 * P, NT], BF16, isOutput=False)
    wq2_d = nc.declare_dram_parameter("wq2", [E, P], BF16, isOutput=False)
    wk2_d = nc.declare_dram_parameter("wk2", [E, P], BF16, isOutput=False)
    wv2_d = nc.declare_dram_parameter("wv2", [E, P], BF16, isOutput=False)
    wkrT_d = nc.declare_dram_parameter("wkrT", [P, E], BF16, isOutput=False)
    # wo duplicated on both partition halves (odd numT s-tiles live at 64:128)
    wo2_d = nc.declare_dram_parameter("wo2", [P, 2 * E], BF16, isOutput=False)
    ub2_d = nc.declare_dram_parameter("ub2", [P, 1], F32, isOutput=False)
    vb2_d = nc.declare_dram_parameter("vb2", [P, 1], F32, isOutput=False)
    # two per-head partial outputs (host sums): h0 streams during h1's
    # attention; h1 drains at the tail
    oA_d = nc.declare_dram_parameter("oA", [N, E], BF16, isOutput=True)
    oB_d = nc.declare_dram_parameter("oB", [N, E], BF16, isOutput=True)

    with TileContext(nc) as tc:
        with (
            tc.tile_pool(name="persist", bufs=1) as persist,
            tc.tile_pool(name="gst", bufs=4) as gst,       # G copies stream
            tc.tile_pool(name="mst", bufs=2) as mst,       # rotation temps
            tc.tile_pool(name="est", bufs=6) as est,       # exp tiles
            tc.tile_pool(name="dram", bufs=1, space="DRAM") as dram_pool,
            tc.tile_pool(name="pr", bufs=5, space="PSUM") as pr,   # 5x [P,512]
            tc.tile_pool(name="ph", bufs=1, space="PSUM") as ph,   # 3x [P,512]
        ):
            _sm = [0]

            def small_psum(shape, name, dtype=F32, tag=None):
                if tag is None:
                    i = _sm[0] % 3
                    _sm[0] += 1
                    tag = f"bank{i}"
                return ph.tile(shape, dtype, tag=tag, name=name)

            # ---------------- DMAs ----------------
            # One prioritized stream on the sync queue: the DMA engines are a
            # serialized resource, so emission order here IS the priority.
            # q proj needs {wq2, x-half}; the uw chain adds {wkr, rot, sc};
            # emit_k(4..7)/emit_v(x) add {wk2, wv2}; history keys come next,
            # then the attention-only psi tables and wo.
            wq2_s = persist.tile([P, NEC, P], BF16, tag="wq2")
            nc.sync.dma_start(wq2_s[:], wq2_d[:].rearrange("(c p) d -> p c d", p=P))
            # wkr stacked on partitions: rows 0:64 = head0 d, 64:128 = head1 d
            wkr_s = persist.tile([P, NEC, P], BF16, tag="wkr")
            nc.sync.dma_start(
                wkr_s[:], wkrT_d[:].rearrange("p (c e) -> p c e", c=NEC)
            )
            ub_s = persist.tile([P, 1], F32, tag="ub")
            nc.sync.dma_start(ub_s[:], ub2_d[:])
            vb_s = persist.tile([P, 1], F32, tag="vb")
            nc.sync.dma_start(vb_s[:], vb2_d[:])
            axT_s = persist.tile([P, NEC, HpN], BF16, tag="axT", name="axT")
            axT = [axT_s[:, c, :] for c in range(NEC)]
            nc.sync.dma_start(
                axT_s[:, :, H : H + 1024],
                axT_d[:, H : H + 1024].rearrange("(c p) k -> p c k", p=P),
            )
            nc.sync.dma_start(
                axT_s[:, :, H + 1024 :],
                axT_d[:, H + 1024 :].rearrange("(c p) k -> p c k", p=P),
            )
            rot_s = persist.tile([P, 4, N], BF16, tag="rot")
            nc.sync.dma_start(rot_s[:, 0, :], rot_d[0:P, :])
            nc.sync.dma_start(rot_s[:, 2, :], rot_d[2 * P : 3 * P, :])
            nc.sync.dma_start(rot_s[:, 1, :], rot_d[P : 2 * P, :])
            nc.sync.dma_start(rot_s[:, 3, :], rot_d[3 * P : 4 * P, :])
            sc_s = persist.tile([P, 2, NT], BF16, tag="sc")
            nc.sync.dma_start(sc_s[:], sc_d[:].rearrange("(k p) r -> p k r", p=P))
            wk2_s = persist.tile([P, NEC, P], BF16, tag="wk2")
            nc.sync.dma_start(wk2_s[:], wk2_d[:].rearrange("(c p) d -> p c d", p=P))
            wv2_s = persist.tile([P, NEC, P], BF16, tag="wv2")
            nc.sync.dma_start(wv2_s[:], wv2_d[:].rearrange("(c p) d -> p c d", p=P))
            nc.sync.dma_start(
                axT_s[:, :, 0:1024],
                axT_d[:, 0:1024].rearrange("(c p) k -> p c k", p=P),
            )
            nc.sync.dma_start(
                axT_s[:, :, 1024:H],
                axT_d[:, 1024:H].rearrange("(c p) k -> p c k", p=P),
            )
            SgF = persist.tile([P, NKT, 2, P], FP8, tag="SgF")
            nc.sync.dma_start(
                SgF[:], psiF_d[:].rearrange("p (t c j) -> p t c j", c=2, j=P)
            )
            # SgA free layout is chunk-major [c][t][j] so partition-sliced
            # chunk DMAs have 4KB contiguous runs.
            SgA = []
            for h in range(HEADS_PER_CORE):
                t = persist.tile([P, 4, NKT, P], FP8, tag=f"SgA{h}", name=f"SgA{h}")
                SgA.append(t)
            for h in range(HEADS_PER_CORE):
                tp = (1 - h) * D
                tps = slice(tp, tp + NT)
                nc.sync.dma_start(
                    SgA[h][tps, 0, :, :],
                    psiT_d[:, 0 : NKT * P].rearrange("p (t j) -> p t j", j=P),
                )
                nc.sync.dma_start(
                    SgA[h][tps, 2, :, :],
                    psiT_d[:, NKT * P :].rearrange("p (t j) -> p t j", j=P),
                )
            for h in range(HEADS_PER_CORE):
                nc.sync.dma_start(
                    SgA[h][:, 3, :, :],
                    psiC_d[:].rearrange("p (t j) -> p t j", j=P),
                )
                # chunk1's T-half duplicates chunk0's (device-side dup)
                tps = slice((1 - h) * D, (1 - h) * D + NT)
                nc.sync.dma_start(SgA[h][tps, 1, :, :], SgA[h][tps, 0, :, :])
            wo_s = persist.tile([P, 2, E], BF16, tag="wo")
            nc.sync.dma_start(wo_s[:], wo2_d[:].rearrange("p (h e) -> p h e", h=2))

            identb = persist.tile([P, P], BF16, tag="identb")
            make_identity(nc, identb[:])

            # ---------------- persistent compute tiles ----------------
            # M chunks per head: 0=Uhi 1=Whi 2=[qhi|chi]
            # 3=[qhi-dup|clo] 4=[qlo|chi-dup] 5=[Wlo(f 0:64)|Ulo(f 64:128)]
            # (chunk 5 pairs with the psiA half-compensation stationary
            # [cos_hi(0:64)|sin_hi(64:128)] in the otherwise-wasted pad slot)
            M = []
            for h in range(HEADS_PER_CORE):
                m = persist.tile([P, 6, NQC, 512], FP8, tag=f"M{h}", name=f"M{h}")
                M.append(m)
            qv_s = persist.tile([P, N], BF16, tag="qv_s")
            vo = []
            for h in range(HEADS_PER_CORE):
                v = persist.tile([P, NKT, 66], BF16, tag=f"vo{h}", name=f"vo{h}")
                nc.gpsimd.memset(v[:, :, 64:66], 0.0)
                nc.gpsimd.memset(v[:, :, 64:65], 1.0)
                vo.append(v)
            # numTT: query-major pre-scaled numerators [q, s, d] (z separate);
            # numT: d-major via 128x128 transposes of s-tile PAIRS -- even
            # s-tile's d on partitions 0:64, odd on 64:128
            numT = []
            numTT = []
            zcs = []
            for h in range(HEADS_PER_CORE):
                t = persist.tile(
                    [P, NS // 2, P], BF16, tag=f"numT{h}", name=f"numT{h}"
                )
                numT.append(t)
                tt = persist.tile(
                    [P, NS, D], BF16, tag=f"numTT{h}", name=f"numTT{h}"
                )
                numTT.append(tt)
                zcs.append(
                    persist.tile([P, NS], F32, tag=f"zc{h}", name=f"zc{h}")
                )
            out_acc = persist.tile([P, NS, E], BF16, tag="out_acc")
            nbias = persist.tile([P, 1], F32, tag="nbias")
            nc.vector.memset(nbias[:], -SCORE_SHIFT)

            # ---------------- phase A: projections ----------------
            # q projection, both heads packed, emitted chunk-outer so the PE
            # starts as soon as each axT chunk lands. pq psums use the ph
            # banks (free until the av accumulators take them).
            pqs = [small_psum([P, 512], f"pq{qc}") for qc in range(NQC)]
            for c in range(NEC):
                for qc in range(NQC):
                    nc.tensor.matmul(
                        pqs[qc][:],
                        wq2_s[:, c, :],
                        axT[c][:, H + qc * 512 : H + (qc + 1) * 512],
                        start=(c == 0),
                        stop=(c == NEC - 1),
                    )
            for qc in range(NQC):
                pq = pqs[qc]
                qs = slice(qc * 512, (qc + 1) * 512)
                nc.vector.tensor_scalar_add(qv_s[:, qs], pq[:], vb_s[:])
                for h in range(HEADS_PER_CORE):
                    hp = slice(h * D, (h + 1) * D)
                    nc.vector.tensor_scalar_add(
                        M[h][hp, 2, qc, :], pq[hp, :], ub_s[hp]
                    )
                    nc.vector.scalar_tensor_tensor(
                        M[h][hp, 4, qc, :], pq[hp, :], ub_s[hp],
                        M[h][hp, 2, qc, :], ALU.add, ALU.subtract,
                    )

            def emit_uw_g_chunk(h, qc, j, sfd, ssd):
                # G: e 0:128 sin-fast + 256:384 cos-fast (sf);
                #    e 128:256 sin-slow + 384:512 cos-slow (ss)
                # one 1-bank psum + copy per chunk so at most one score-stream
                # slot is borrowed at a time
                hp = slice(h * D, (h + 1) * D)
                qs = slice(qc * 512, (qc + 1) * 512)
                half, jj = j // 2, j % 2
                dst = sfd if half == 0 else ssd
                g = pr.tile([P, 512], F32, tag="sp", name="g")
                nc.tensor.matmul(
                    g[:], wkr_s[hp, 2 * jj + half, :], qv_s[hp, qs],
                    start=True, stop=True,
                )
                nc.scalar.copy(dst[:, jj * 512 : (jj + 1) * 512], g[:])

            def emit_uw_g(h, qc, sfd, ssd):
                for j in range(4):
                    emit_uw_g_chunk(h, qc, j, sfd, ssd)

            def emit_uw_rot(h, qc, sf, ss, usw):
                qs = slice(qc * 512, (qc + 1) * 512)
                # muls split DVE/Pool: h0 runs in phase A (Pool idle, DVE has
                # the psum copies) -> 6 DVE / 2 Pool; h1 runs during h0's
                # attention (DVE carries half the exps) -> 4/4
                V, G = nc.vector, nc.gpsimd
                me = (V, G, V, V, V, G, V, V) if h == 0 else (V, G, V, G, V, G, V, G)
                # fast half: U = G*cos + Gc*sin ; W = Gc*cos - G*sin
                m1 = mst.tile([P, 512], BF16, tag="m1")
                m2 = mst.tile([P, 512], BF16, tag="m2")
                m3 = mst.tile([P, 512], BF16, tag="m3")
                m4 = mst.tile([P, 512], BF16, tag="m4")
                me[0].tensor_mul(m1[:], sf[:, 0:512], rot_s[:, 0, qs])
                me[1].tensor_mul(m2[:], sf[:, 512:1024], rot_s[:, 2, qs])
                me[2].tensor_mul(m3[:], sf[:, 512:1024], rot_s[:, 0, qs])
                me[3].tensor_mul(m4[:], sf[:, 0:512], rot_s[:, 2, qs])
                ubf = mst.tile([P, 512], BF16, tag="ubf")
                wbf = mst.tile([P, 512], BF16, tag="wbf")
                nc.gpsimd.tensor_add(ubf[:], m1[:], m2[:])
                nc.gpsimd.tensor_sub(wbf[:], m3[:], m4[:])
                nc.vector.tensor_copy(M[h][:, 0, qc, :], ubf[:])
                nc.gpsimd.tensor_copy(M[h][:, 1, qc, :], wbf[:])
                # half lo-comp into the pad slot (partition-aligned halves)
                nc.vector.tensor_sub(
                    M[h][0:D, 5, qc, :], wbf[0:D, :], M[h][0:D, 1, qc, :]
                )
                nc.vector.tensor_sub(
                    M[h][D:P, 5, qc, :], ubf[D:P, :], M[h][D:P, 0, qc, :]
                )
                # slow half: rotate; compression happens in emit_uw_cheb
                m5 = mst.tile([P, 512], BF16, tag="m1", name="m5")
                m6 = mst.tile([P, 512], BF16, tag="m2", name="m6")
                m7 = mst.tile([P, 512], BF16, tag="m3", name="m7")
                m8 = mst.tile([P, 512], BF16, tag="m4", name="m8")
                me[4].tensor_mul(m5[:], ss[:, 0:512], rot_s[:, 1, qs])
                me[5].tensor_mul(m6[:], ss[:, 512:1024], rot_s[:, 3, qs])
                me[6].tensor_mul(m7[:], ss[:, 512:1024], rot_s[:, 1, qs])
                me[7].tensor_mul(m8[:], ss[:, 0:512], rot_s[:, 3, qs])
                nc.gpsimd.tensor_add(usw[:, 0, :], m5[:], m6[:])
                nc.gpsimd.tensor_sub(usw[:, 1, :], m7[:], m8[:])

            def emit_uw_cheb(h, qc, usw, pc=None):
                # cheb coefs land on the head's opposite partition half
                po = (1 - h) * D
                cs = slice(po, po + NT)
                if pc is None:
                    pc = small_psum([P, 512], "pc")
                for k in range(2):
                    nc.tensor.matmul(
                        pc[cs, :], sc_s[:, k, :], usw[:, k, :],
                        start=(k == 0), stop=(k == 1),
                    )
                nc.scalar.copy(M[h][cs, 2, qc, :], pc[cs, :])
                nc.vector.tensor_sub(
                    M[h][cs, 3, qc, :], pc[cs, :], M[h][cs, 2, qc, :]
                )

            def emit_k(kc):
                pk = small_psum([P, 512], "pk")
                for c in range(NEC):
                    nc.tensor.matmul(
                        pk[:],
                        wk2_s[:, c, :],
                        axT[c][:, kc * 512 : (kc + 1) * 512],
                        start=(c == 0),
                        stop=(c == NEC - 1),
                    )
                ks = slice(4 * kc, 4 * kc + 4)
                for h in range(HEADS_PER_CORE):
                    hp = slice(h * D, (h + 1) * D)
                    pkv = pk[hp, :].rearrange("p (t j) -> p t j", j=P)
                    nc.scalar.copy(SgA[h][hp, 0, ks, :], pkv)
                    nc.vector.tensor_sub(
                        SgA[h][hp, 1, ks, :], pkv, SgA[h][hp, 0, ks, :]
                    )

            def emit_v(h, g, tag=None):
                hs = slice(h * D, (h + 1) * D)
                pv = small_psum([P, 512], "pv", tag=tag)
                for k8 in range(8):
                    kt = g * 8 + k8
                    for c in range(NEC):
                        nc.tensor.matmul(
                            pv[:, k8 * D : (k8 + 1) * D],
                            axT[c][:, kt * P : (kt + 1) * P],
                            wv2_s[:, c, hs],
                            start=(c == 0),
                            stop=(c == NEC - 1),
                        )
                nc.vector.tensor_copy(
                    vo[h][:, g * 8 : (g + 1) * 8, 0:D],
                    pv[:].rearrange("p (t d) -> p t d", d=D),
                )

            # h0 UW fully in phase A (streaming); h1's G copies land in a
            # persistent tile recycled from axT's tag so h1's rotation
            # (engine-only) can run during h0's attention.
            h1b = persist.tile(
                [P, NQC, 6, 512], BF16, tag="axT", name="h1buf"
            )
            h1buf = [h1b[:, u, :, :] for u in range(NQC)]

            # x keys (kc 4..7, v groups 2..3) first: their axT DMA lands well
            # before the history half. All G emissions go early (their
            # rotations then stream on DVE/Pool); chebs are deferred so the
            # PE never waits on a rotation.
            uwt = []
            for u in range(NQC):
                sf = gst.tile([P, 1024], BF16, tag="sf")
                ss = gst.tile([P, 1024], BF16, tag="ss")
                usw = gst.tile([P, 2, 512], BF16, tag="usw")
                uwt.append((sf, ss, usw))
                emit_uw_g(0, u, sf[:], ss[:])
                emit_uw_rot(0, u, sf, ss, usw)
                emit_k(4 + u)
            for g in (2, 3):
                emit_v(0, g)
                emit_v(1, g)
            emit_uw_cheb(0, 0, uwt[0][2])
            emit_uw_cheb(0, 1, uwt[1][2])
            for u in range(NQC):
                emit_k(u)
            emit_uw_cheb(0, 2, uwt[2][2])
            for g in (0, 1):
                emit_v(0, g)
                emit_v(1, g)
            emit_uw_cheb(0, 3, uwt[3][2])

            # dups via DMA (off-engine): M chunk 3 q-half <- chunk 2 q-half;
            # M chunk 4 cheb-half <- chunk 2 cheb-half (h0 now, h1 after its
            # cheb block); SgA chunk 2 <- chunk 0
            for h in range(HEADS_PER_CORE):
                hp = slice(h * D, (h + 1) * D)
                nc.sync.dma_start(M[h][hp, 3, :, :], M[h][hp, 2, :, :])
                nc.sync.dma_start(SgA[h][hp, 2, :, :], SgA[h][hp, 0, :, :])
            cs0 = slice(D, D + NT)
            nc.sync.dma_start(M[0][cs0, 4, :, :], M[0][cs0, 2, :, :])

            # ---------------- phase B: attention ----------------
            # Unit = one (key tile, query chunk): score psum is a 1-bank
            # [P, 512] tile from the 5-deep pr pool, so the
            # ps -> exp -> frees-slot chain never stalls the PE. exp
            # alternates Act (exact) / DVE (Schraudolph) per unit; during
            # h0's attention DVE also carries h1's rotation, so it only
            # takes 3 of 8 exps there.
            _expctr = [0]
            _dve_exp = {0: (1, 3, 5), 1: (1, 3, 5, 7)}

            def emit_av(h, kt, qc, pE, avv):
                for qt in range(4):
                    qg = qc * 4 + qt
                    bk, sl = divmod(qg, 6)
                    nc.tensor.matmul(
                        avv[bk][:, sl, :],
                        pE[:, qt * P : (qt + 1) * P],
                        vo[h][:, kt, 0:65],
                        start=(kt == 0 and qg in (0, 6, 12)),
                        stop=(kt == NKT - 1 and qg in (5, 11, 15)),
                        skip_group_check=True,
                    )

            def emit_unit(h, kt, qc, avv, pend):
                ps = pr.tile([P, 512], F32, tag="sp", name="ps")
                nc.tensor.matmul(
                    ps[:], SgF[:, kt, :, :], M[h][:, 0:2, qc, :],
                    start=True, stop=False, perf_mode=DR,
                )
                nc.tensor.matmul(
                    ps[:], SgA[h][:, 0:2, kt, :], M[h][:, 2:4, qc, :],
                    start=False, stop=False, perf_mode=DR,
                )
                nc.tensor.matmul(
                    ps[:], SgA[h][:, 2:4, kt, :], M[h][:, 4:6, qc, :],
                    start=False, stop=True, perf_mode=DR,
                )
                if qc in pend:
                    pkt, pE = pend.pop(qc)
                    emit_av(h, pkt, qc, pE, avv)
                et = est.tile([P, 512], BF16, tag="E")
                if _expctr[0] % 8 not in _dve_exp[h]:
                    nc.scalar.activation(
                        et[:], ps[:], AF.Exp, scale=0.125, bias=nbias[:]
                    )
                else:
                    # Schraudolph: int16 bits = 128*(log2e*(s/8 - c) + 127)
                    nc.vector.tensor_scalar(
                        et[:].bitcast(I16), ps[:],
                        0.125 * P * LOG2E,
                        P * 127.0 - SCORE_SHIFT * P * LOG2E - 8.5,
                        ALU.mult, ALU.add,
                    )
                _expctr[0] += 1
                pend[qc] = (kt, et)

            def emit_av_flush(h, avv, pend):
                for qc, (pkt, pE) in sorted(pend.items()):
                    emit_av(h, pkt, qc, pE, avv)
                pend.clear()

            def emit_z(h, avv):
                # av is query-major with the ones-column z in slot 64: copy
                # the z columns, take the reciprocal, then write numTT
                # PRE-SCALED by 1/z (per-partition scalar per s-tile) so the
                # out-projection result needs no further scaling.
                zc = zcs[h]
                nc.vector.tensor_copy(zc[:, 0:6], avv[0][:, :, 64])
                nc.vector.tensor_copy(zc[:, 6:12], avv[1][:, :, 64])
                nc.vector.tensor_copy(zc[:, 12:16], avv[2][:, :, 64])
                zrec = persist.tile([P, NS], F32, tag=f"zrec{h}", name=f"zrec{h}")
                nc.vector.reciprocal(zrec[:], zc[:])
                ntt = numTT[h]
                for s in range(NS):
                    bk, sl = divmod(s, 6)
                    nc.vector.tensor_scalar_mul(
                        ntt[:, s, :], avv[bk][:, sl, 0:D], zrec[:, s : s + 1]
                    )

            def emit_z_tr(h, s2):
                # transpose one PAIR of numerator s-tiles ([128,128] block)
                # back to d-major via the DMA xbar (off-engine)
                nc.sync.dma_start_transpose(
                    numT[h][:, s2, :],
                    numTT[h][:, 2 * s2 : 2 * s2 + 2, :],
                )

            def emit_z_tr_pe(h, s2, copy_eng):
                pz = pr.tile([P, P], BF16, tag="sp", name="pz")
                nc.tensor.transpose(
                    pz[:], numTT[h][:, 2 * s2 : 2 * s2 + 2, :], identb[:]
                )
                copy_eng(numT[h][:, s2, :], pz[:])

            def emit_out_s(h, s):
                # numT is pre-scaled by 1/z, so the psum->sbuf conversion is
                # a plain copy (alternating Act/DVE to spread the load)
                po = pr.tile([P, 512], F32, tag="sp", name="po")
                hp = (s % 2) * D
                nc.tensor.matmul(
                    po[:], numT[h][hp : hp + D, s // 2, :],
                    wo_s[hp : hp + D, h, :],
                    start=True, stop=True,
                )
                if s % 2 == 0:
                    nc.scalar.copy(out_acc[:, s, :], po[:])
                else:
                    nc.vector.tensor_copy(out_acc[:, s, :], po[:])
                od = oA_d if h == 0 else oB_d
                nc.sync.dma_start(
                    od[:].rearrange("(s p) e -> p s e", p=P)[:, s, :],
                    out_acc[:, s, :],
                )

            # h0 attention with h1's G/rotation/cheb interleaved (their
            # elementwise runs on Pool/Act; DVE carries the exp stream)
            av0 = [
                ph.tile([P, 6 if j < 2 else 4, 65], F32, tag=f"bank{j}",
                        name=f"av0{j}")
                for j in range(3)
            ]
            pend0 = {}
            for kt in range(NKT):
                for qc in range(NQC):
                    emit_unit(0, kt, qc, av0, pend0)
                # h1 prep spread: per u-block of 7 kts, one G chunk per kt
                # (each borrows one score-stream slot briefly), then the
                # rotation (engine-only), then cheb (one pr slot).
                if 1 <= kt <= 28:
                    u, ph7 = divmod(kt - 1, 7)
                    if ph7 < 4:
                        emit_uw_g_chunk(
                            1, u, ph7,
                            h1buf[u][:, 0:2, :].rearrange("p a b -> p (a b)"),
                            h1buf[u][:, 2:4, :].rearrange("p a b -> p (a b)"),
                        )
                    elif ph7 == 4:
                        emit_uw_rot(
                            1, u,
                            h1buf[u][:, 0:2, :].rearrange("p a b -> p (a b)"),
                            h1buf[u][:, 2:4, :].rearrange("p a b -> p (a b)"),
                            h1buf[u][:, 4:6, :],
                        )
                    elif ph7 == 6:
                        # pr slot: the ph banks are held by av0 here (a ph
                        # allocation would deadlock the in-order PE queue)
                        emit_uw_cheb(
                            1, u, h1buf[u][:, 4:6, :],
                            pc=pr.tile([P, 512], F32, tag="sp", name="pc1"),
                        )

            emit_av_flush(0, av0, pend0)
            emit_z(0, av0)
            cs1 = slice(0, NT)
            nc.sync.dma_start(M[1][cs1, 4, :, :], M[1][cs1, 2, :, :])

            # h1 attention with h0's transpose + output projection streamed
            # (out tile s at kt = 6 + 3s//2, i.e. 2 tiles per 3 key tiles)
            _out_sched = {6 + (3 * s) // 2: s for s in range(NS)}
            av1 = [
                ph.tile([P, 6 if j < 2 else 4, 65], F32, tag=f"bank{j}",
                        name=f"av1{j}")
                for j in range(3)
            ]
            pend1 = {}
            for kt in range(NKT):
                for qc in range(NQC):
                    emit_unit(1, kt, qc, av1, pend1)
                if 1 <= kt <= 8:
                    emit_z_tr(0, kt - 1)
                if kt in _out_sched:
                    emit_out_s(0, _out_sched[kt])
            emit_av_flush(1, av1, pend1)
            emit_z(1, av1)
            for s2 in range(NS // 2):
                emit_z_tr_pe(
                    1, s2, nc.vector.tensor_copy if s2 % 2 else nc.scalar.copy
                )
                emit_out_s(1, 2 * s2)
                emit_out_s(1, 2 * s2 + 1)

    nc.compile()
    return nc


_NC_CACHE = None


def _get_program():
    global _NC_CACHE
    if _NC_CACHE is None:
        _NC_CACHE = build_program()
    return _NC_CACHE


def _fp8_hl(x):
    hi = np.clip(np.asarray(x, np.float32), -240, 240).astype(ml_dtypes.float8_e4m3)
    lo = np.clip(
        np.asarray(x, np.float32) - hi.astype(np.float32), -240, 240
    ).astype(ml_dtypes.float8_e4m3)
    return hi, lo


def make_in_maps(x, history, w_q, w_k, w_v, w_kr, w_o, u_bias, v_bias):
    bf = ml_dtypes.bfloat16
    all_x = np.concatenate([history, x], axis=1)  # [B, HpN, E]

    inv_freq = 1.0 / (10000.0 ** (np.arange(0, E, 2, dtype=np.float64) / E))  # [256]
    ang_f = np.outer(inv_freq[:128], np.arange(HpN, dtype=np.float64) - H)
    xn = (np.arange(HpN, dtype=np.float64) - H) / 2048.0
    T = np.polynomial.chebyshev.chebvander(xn, NT - 1)  # [HpN, NT]
    ang_s = np.outer(xn * 2048.0, inv_freq[128:256])  # [HpN, 128]
    tgt = np.concatenate([np.sin(ang_s), np.cos(ang_s)], axis=1)  # [HpN, 256]
    coef, *_ = np.linalg.lstsq(T, tgt, rcond=None)  # [NT, 256]
    sc = np.ascontiguousarray(coef.T)  # [256, NT]: rows 0-127 sin, 128-255 cos

    sin_hi, _ = _fp8_hl(np.sin(ang_f))
    cos_hi, _ = _fp8_hl(np.cos(ang_f))
    T_hi, T_lo = _fp8_hl(T.T)  # [NT, HpN]
    sin_f = sin_hi.astype(np.float32)
    cos_f = cos_hi.astype(np.float32)
    # SgF partition-major: [p][t][c][j], chunks c = [sin_hi, cos_hi]
    psiF = np.ascontiguousarray(
        np.stack(
            [sin_f.reshape(P, NKT, P), cos_f.reshape(P, NKT, P)], axis=2
        ).reshape(P, NKT * 2 * P)
    )
    # shared cheb T basis [p(64)][hi/lo][t][j] (device places it per head)
    psiT = np.ascontiguousarray(
        np.stack(
            [
                T_hi.astype(np.float32).reshape(NT, NKT, P),
                T_lo.astype(np.float32).reshape(NT, NKT, P),
            ],
            axis=1,
        ).reshape(NT, 2 * NKT * P)
    )
    # fast-psi half-compensation stationary [cos_hi(f0:64)|sin_hi(f64:128)]:
    # pairs with M chunk 5 = [Wlo(f0:64)|Ulo(f64:128)]
    psiC = np.ascontiguousarray(
        np.concatenate([cos_f[0:D], sin_f[D:P]], axis=0).reshape(P, NKT * P)
    )

    ang_b = np.outer(inv_freq, np.arange(N, dtype=np.float64))  # [256, N]
    rot = np.ascontiguousarray(
        np.concatenate([np.cos(ang_b), np.sin(ang_b)]).astype(bf)
    )  # [512, N]: rows 0:128 cos-fast, 128:256 cos-slow, 256:384 sin-fast, ...

    clip8 = lambda a: np.clip(a, -240, 240).astype(ml_dtypes.float8_e4m3)

    in_maps = []
    for c in range(N_CORES):
        b = c // 4
        h0 = HEADS_PER_CORE * (c % 4)
        axT = np.ascontiguousarray(all_x[b].T).astype(bf)
        wq2 = np.concatenate([w_q[h0], w_q[h0 + 1]], axis=1).astype(bf)  # [E, 128]
        wk2 = np.concatenate([w_k[h0], w_k[h0 + 1]], axis=1).astype(bf)
        wv2 = np.concatenate([w_v[h0], w_v[h0 + 1]], axis=1).astype(bf)
        wkrT = np.concatenate(
            [w_kr[h0].T, w_kr[h0 + 1].T], axis=0
        ).astype(bf)  # [128, E]: rows 0:64 = head0 (d), 64:128 = head1
        wo1h = np.stack([w_o[h0], w_o[h0 + 1]], axis=1).reshape(D, 2 * E)
        wo2 = np.concatenate([wo1h, wo1h], axis=0).astype(bf)  # [P, 2E]
        in_maps.append(
            {
                "axT": axT,
                "rot": rot,
                "psiF": clip8(psiF),
                "psiT": clip8(psiT),
                "psiC": clip8(psiC),
                "sc": np.ascontiguousarray(sc).astype(bf),
                "wq2": np.ascontiguousarray(wq2),
                "wk2": np.ascontiguousarray(wk2),
                "wv2": np.ascontiguousarray(wv2),
                "wkrT": np.ascontiguousarray(wkrT),
                "wo2": np.ascontiguousarray(wo2),
                "ub2": np.ascontiguousarray(
                    np.concatenate([u_bias[h0], u_bias[h0 + 1]]).reshape(P, 1)
                ).astype(np.float32),
                "vb2": np.ascontiguousarray(
                    np.concatenate([v_bias[h0], v_bias[h0 + 1]]).reshape(P, 1)
                ).astype(np.float32),
            }
        )
    return in_maps


def run(inputs, trace=False, **kw):
    from concourse.bass_utils import run_bass_kernel_spmd

    nc = _get_program()
    in_maps = make_in_maps(
        np.asarray(inputs["x"], np.float32),
        np.asarray(inputs["history"], np.float32),
        np.asarray(inputs["w_q"], np.float32),
        np.asarray(inputs["w_k"], np.float32),
        np.asarray(inputs["w_v"], np.float32),
        np.asarray(inputs["w_kr"], np.float32),
        np.asarray(inputs["w_o"], np.float32),
        np.asarray(inputs["u_bias"], np.float32),
        np.asarray(inputs["v_bias"], np.float32),
    )
    res = run_bass_kernel_spmd(nc, in_maps, list(range(N_CORES)), trace=trace, **kw)
    out = np.zeros((B, N, E), np.float32)
    for c in range(N_CORES):
        out[c // 4] += res.results[c]["oA"].astype(np.float32).reshape(N, E)
        out[c // 4] += res.results[c]["oB"].astype(np.float32).reshape(N, E)
    return out, res


def kernel(**inputs):
    # mask is all ones (per the problem spec), so score masking is a no-op
    # and the tensor is ignored.
    out, _ = run(inputs, trace=False)
    return out



# revision 87
# speedup vs baseline: 1.1963x; 1.0617x over previous
"""Transformer-XL multi-head self-attention on 8 Trainium2 NeuronCores.

Sharding: core c handles batch b = c//4 and heads {2*(c%4), 2*(c%4)+1}
(data-parallel over B x tensor-parallel over heads). Each core produces a
partial [N, E] output (its heads' w_o contributions); the host sums the 4
partials per batch element.

The XL relative-position term BD[i,j] = (q_i+v)·BDk[j-i+N-1] is computed
without the rel_shift gather via per-query rotation (angle-difference
identities): BD^T = Psi @ UW with Psi a shape-derived constant basis
(128 exact sin rows + 128 exact cos rows + 64 Chebyshev rows for the slow
frequencies) and UW per-query rotated coefficients.

Scores run on the PE in fp8e4 DoubleRow mode (0.5 cycles/row in the cost
model) with hi/lo error compensation: a bf16-accurate operand x is split
as x = hi + lo with hi = fp8(x), lo = fp8(x - hi), keeping selected cross
terms. Per 128-key tile the contraction is 6 chunks of 128 rows consumed
by 3 DoubleRow calls:
  [sin|cos]x[Uhi|Whi],
  [khi|Thi]x[qhi|chi], [klo|Thi]x[qhi|clo], [khi|Tlo]x[qlo|chi], pad
where T/c are the Chebyshev basis/coefficients and k/q carry the content
term (q+u)·k. The U/W (fast psi coefficient) lo-compensation is dropped
(one-sided both psi and U/W): host-side simulation puts the end-to-end
max-rel error at ~1.4% vs the 2% gate (vs ~1.2% with the compensation).
The value path (exp, V, attn@V, output projection) stays in bf16: fp8
noise there does not average out. exp is spread over Act/DVE/Pool
(Schraudolph on DVE/Pool; the extra Schraudolph noise is ~free: ~1.47%
even if every tile uses it).
"""

import sys

sys.path.insert(0, "/opt/trn_rl_repo")

import ml_dtypes
import numpy as np

import concourse.bass as bass
import concourse.mybir as mybir
from concourse import bacc
from concourse.masks import make_identity
from concourse.tile import TileContext

F32 = mybir.dt.float32
BF16 = mybir.dt.bfloat16
FP8 = mybir.dt.float8e4
I16 = mybir.dt.int16
AF = mybir.ActivationFunctionType
ALU = mybir.AluOpType
DR = mybir.MatmulPerfMode.DoubleRow

B, N, H, E, NH, D = 2, 2048, 2048, 512, 8, 64
HpN = H + N  # 4096
P = 128
NKT = HpN // P  # 32 key tiles
NPAIR = NKT // 2  # 16 key-tile pairs
NQC = N // 512  # 4 query chunks of 512
NEC = E // P  # 4 contraction chunks over E
NS = N // P  # 16 output row tiles
NT = 64  # chebyshev terms
HEADS_PER_CORE = 2
N_CORES = 8

LOG2E = 1.4426950408889634
SCORE_SHIFT = 1.5  # exp(s - c): cancels in softmax, bounds exp values
# exp tile engine rotation: (ctr % MOD) -> r < EXP_ACT on Act (exact),
# rest on DVE (Schraudolph). GPSIMD cannot read PSUM so Pool is out.
# Strict alternation: consecutive units' exps overlap across the two
# engines (each engine sees one ~1.1us exp per two 858ns PE units).
EXP_MOD, EXP_ACT = 2, 1


def build_program():
    nc = bacc.Bacc("TRN2", target_bir_lowering=False, debug=False)

    axT_d = nc.declare_dram_parameter("axT", [E, HpN], BF16, isOutput=False)
    rot_d = nc.declare_dram_parameter("rot", [E, N], BF16, isOutput=False)
    # SgF: shared fast-psi chunks, partition-major [p][t][c][j] so the DMA is
    # an identity layout with 8KB per-partition runs
    psiF_d = nc.declare_dram_parameter("psiF", [P, NKT * 2 * P], FP8, isOutput=False)
    # shared cheb T basis rows [p(64)][hi/lo][t][j]; identical for both heads
    # (placed at opposite partition halves on device)
    psiT_d = nc.declare_dram_parameter("psiT", [NT, 2 * NKT * P], FP8, isOutput=False)
    # fast-psi half-compensation stationary [cos_hi(f0:64)|sin_hi(f64:128)],
    # shared by both heads: [p][t][j]
    psiC_d = nc.declare_dram_parameter("psiC", [P, NKT * P], FP8, isOutput=False)
    sc_d = nc.declare_dram_parameter("sc", [2 * P, NT], BF16, isOutput=False)
    wq2_d = nc.declare_dram_parameter("wq2", [E, P], BF16, isOutput=False)
    wk2_d = nc.declare_dram_parameter("wk2", [E, P], BF16, isOutput=False)
    wv2_d = nc.declare_dram_parameter("wv2", [E, P], BF16, isOutput=False)
    wkrT_d = nc.declare_dram_parameter("wkrT", [P, E], BF16, isOutput=False)
    # wo duplicated on both partition halves (odd numT s-tiles live at 64:128)
    wo2_d = nc.declare_dram_parameter("wo2", [P, 2 * E], BF16, isOutput=False)
    ub2_d = nc.declare_dram_parameter("ub2", [P, 1], F32, isOutput=False)
    vb2_d = nc.declare_dram_parameter("vb2", [P, 1], F32, isOutput=False)
    # two per-head partial outputs (host sums): h0 streams during h1's
    # attention; h1 drains at the tail
    oA_d = nc.declare_dram_parameter("oA", [N, E], BF16, isOutput=True)
    oB_d = nc.declare_dram_parameter("oB", [N, E], BF16, isOutput=True)

    with TileContext(nc) as tc:
        with (
            tc.tile_pool(name="persist", bufs=1) as persist,
            tc.tile_pool(name="gst", bufs=4) as gst,       # G copies stream
            tc.tile_pool(name="mst", bufs=2) as mst,       # rotation temps
            tc.tile_pool(name="est", bufs=6) as est,       # exp tiles
            tc.tile_pool(name="dram", bufs=1, space="DRAM") as dram_pool,
            tc.tile_pool(name="pr", bufs=5, space="PSUM") as pr,   # 5x [P,512]
            tc.tile_pool(name="ph", bufs=1, space="PSUM") as ph,   # 3x [P,512]
        ):
            _sm = [0]

            def small_psum(shape, name, dtype=F32, tag=None):
                if tag is None:
                    i = _sm[0] % 3
                    _sm[0] += 1
                    tag = f"bank{i}"
                return ph.tile(shape, dtype, tag=tag, name=name)

            # ---------------- DMAs ----------------
            # One prioritized stream on the sync queue: the DMA engines are a
            # serialized resource, so emission order here IS the priority.
            # q proj needs {wq2, x-half}; the uw chain adds {wkr, rot, sc};
            # emit_k(4..7)/emit_v(x) add {wk2, wv2}; history keys come next,
            # then the attention-only psi tables and wo.
            wq2_s = persist.tile([P, NEC, P], BF16, tag="wq2")
            nc.sync.dma_start(wq2_s[:], wq2_d[:].rearrange("(c p) d -> p c d", p=P))
            # wkr stacked on partitions: rows 0:64 = head0 d, 64:128 = head1 d
            wkr_s = persist.tile([P, NEC, P], BF16, tag="wkr")
            nc.sync.dma_start(
                wkr_s[:], wkrT_d[:].rearrange("p (c e) -> p c e", c=NEC)
            )
            ub_s = persist.tile([P, 1], F32, tag="ub")
            nc.sync.dma_start(ub_s[:], ub2_d[:])
            vb_s = persist.tile([P, 1], F32, tag="vb")
            nc.sync.dma_start(vb_s[:], vb2_d[:])
            axT_s = persist.tile([P, NEC, HpN], BF16, tag="axT", name="axT")
            axT = [axT_s[:, c, :] for c in range(NEC)]
            nc.sync.dma_start(
                axT_s[:, :, H : H + 1024],
                axT_d[:, H : H + 1024].rearrange("(c p) k -> p c k", p=P),
            )
            nc.sync.dma_start(
                axT_s[:, :, H + 1024 :],
                axT_d[:, H + 1024 :].rearrange("(c p) k -> p c k", p=P),
            )
            rot_s = persist.tile([P, 4, N], BF16, tag="rot")
            nc.sync.dma_start(rot_s[:, 0, :], rot_d[0:P, :])
            nc.sync.dma_start(rot_s[:, 2, :], rot_d[2 * P : 3 * P, :])
            nc.sync.dma_start(rot_s[:, 1, :], rot_d[P : 2 * P, :])
            nc.sync.dma_start(rot_s[:, 3, :], rot_d[3 * P : 4 * P, :])
            sc_s = persist.tile([P, 2, NT], BF16, tag="sc")
            nc.sync.dma_start(sc_s[:], sc_d[:].rearrange("(k p) r -> p k r", p=P))
            wk2_s = persist.tile([P, NEC, P], BF16, tag="wk2")
            nc.sync.dma_start(wk2_s[:], wk2_d[:].rearrange("(c p) d -> p c d", p=P))
            wv2_s = persist.tile([P, NEC, P], BF16, tag="wv2")
            nc.sync.dma_start(wv2_s[:], wv2_d[:].rearrange("(c p) d -> p c d", p=P))
            nc.sync.dma_start(
                axT_s[:, :, 0:1024],
                axT_d[:, 0:1024].rearrange("(c p) k -> p c k", p=P),
            )
            nc.sync.dma_start(
                axT_s[:, :, 1024:H],
                axT_d[:, 1024:H].rearrange("(c p) k -> p c k", p=P),
            )
            SgF = persist.tile([P, NKT, 2, P], FP8, tag="SgF")
            nc.sync.dma_start(
                SgF[:], psiF_d[:].rearrange("p (t c j) -> p t c j", c=2, j=P)
            )
            # SgA free layout is chunk-major [c][t][j] so partition-sliced
            # chunk DMAs have 4KB contiguous runs.
            SgA = []
            for h in range(HEADS_PER_CORE):
                t = persist.tile([P, 4, NKT, P], FP8, tag=f"SgA{h}", name=f"SgA{h}")
                SgA.append(t)
            for h in range(HEADS_PER_CORE):
                tp = (1 - h) * D
                tps = slice(tp, tp + NT)
                nc.sync.dma_start(
                    SgA[h][tps, 0, :, :],
                    psiT_d[:, 0 : NKT * P].rearrange("p (t j) -> p t j", j=P),
                )
                nc.sync.dma_start(
                    SgA[h][tps, 2, :, :],
                    psiT_d[:, NKT * P :].rearrange("p (t j) -> p t j", j=P),
                )
            for h in range(HEADS_PER_CORE):
                nc.sync.dma_start(
                    SgA[h][:, 3, :, :],
                    psiC_d[:].rearrange("p (t j) -> p t j", j=P),
                )
                # chunk1's T-half duplicates chunk0's (device-side dup)
                tps = slice((1 - h) * D, (1 - h) * D + NT)
                nc.sync.dma_start(SgA[h][tps, 1, :, :], SgA[h][tps, 0, :, :])
            wo_s = persist.tile([P, 2, E], BF16, tag="wo")
            nc.sync.dma_start(wo_s[:], wo2_d[:].rearrange("p (h e) -> p h e", h=2))

            identb = persist.tile([P, P], BF16, tag="identb")
            make_identity(nc, identb[:])

            # ---------------- persistent compute tiles ----------------
            # M chunks per head: 0=Uhi 1=Whi 2=[qhi|chi]
            # 3=[qhi-dup|clo] 4=[qlo|chi-dup] 5=[Wlo(f 0:64)|Ulo(f 64:128)]
            # (chunk 5 pairs with the psiA half-compensation stationary
            # [cos_hi(0:64)|sin_hi(64:128)] in the otherwise-wasted pad slot)
            M = []
            for h in range(HEADS_PER_CORE):
                m = persist.tile([P, 6, NQC, 512], FP8, tag=f"M{h}", name=f"M{h}")
                M.append(m)
            qv_s = persist.tile([P, N], BF16, tag="qv_s")
            vo = []
            for h in range(HEADS_PER_CORE):
                v = persist.tile([P, NKT, 66], BF16, tag=f"vo{h}", name=f"vo{h}")
                nc.gpsimd.memset(v[:, :, 64:66], 0.0)
                nc.gpsimd.memset(v[:, :, 64:65], 1.0)
                vo.append(v)
            # numTT: query-major pre-scaled numerators [q, s, d] (z separate);
            # numT: d-major via 128x128 transposes of s-tile PAIRS -- even
            # s-tile's d on partitions 0:64, odd on 64:128
            numT = []
            numTT = []
            zcs = []
            for h in range(HEADS_PER_CORE):
                t = persist.tile(
                    [P, NS // 2, P], BF16, tag=f"numT{h}", name=f"numT{h}"
                )
                numT.append(t)
                tt = persist.tile(
                    [P, NS, D], BF16, tag=f"numTT{h}", name=f"numTT{h}"
                )
                numTT.append(tt)
                zcs.append(
                    persist.tile([P, NS], F32, tag=f"zc{h}", name=f"zc{h}")
                )
            out_acc = persist.tile([P, NS, E], BF16, tag="out_acc")
            nbias = persist.tile([P, 1], F32, tag="nbias")
            nc.vector.memset(nbias[:], -SCORE_SHIFT)

            # ---------------- phase A: projections ----------------
            # q projection, both heads packed, emitted chunk-outer so the PE
            # starts as soon as each axT chunk lands. pq psums use the ph
            # banks (free until the av accumulators take them).
            pqs = [small_psum([P, 512], f"pq{qc}") for qc in range(NQC)]
            for c in range(NEC):
                for qc in range(NQC):
                    nc.tensor.matmul(
                        pqs[qc][:],
                        wq2_s[:, c, :],
                        axT[c][:, H + qc * 512 : H + (qc + 1) * 512],
                        start=(c == 0),
                        stop=(c == NEC - 1),
                    )
            for qc in range(NQC):
                pq = pqs[qc]
                qs = slice(qc * 512, (qc + 1) * 512)
                nc.vector.tensor_scalar_add(qv_s[:, qs], pq[:], vb_s[:])
                for h in range(HEADS_PER_CORE):
                    hp = slice(h * D, (h + 1) * D)
                    nc.vector.tensor_scalar_add(
                        M[h][hp, 2, qc, :], pq[hp, :], ub_s[hp]
                    )
                    nc.vector.scalar_tensor_tensor(
                        M[h][hp, 4, qc, :], pq[hp, :], ub_s[hp],
                        M[h][hp, 2, qc, :], ALU.add, ALU.subtract,
                    )

            def emit_uw_g_chunk(h, qc, j, sfd, ssd):
                # G: e 0:128 sin-fast + 256:384 cos-fast (sf);
                #    e 128:256 sin-slow + 384:512 cos-slow (ss)
                # one 1-bank psum + copy per chunk so at most one score-stream
                # slot is borrowed at a time
                hp = slice(h * D, (h + 1) * D)
                qs = slice(qc * 512, (qc + 1) * 512)
                half, jj = j // 2, j % 2
                dst = sfd if half == 0 else ssd
                g = pr.tile([P, 512], F32, tag="sp", name="g")
                nc.tensor.matmul(
                    g[:], wkr_s[hp, 2 * jj + half, :], qv_s[hp, qs],
                    start=True, stop=True,
                )
                nc.scalar.copy(dst[:, jj * 512 : (jj + 1) * 512], g[:])

            def emit_uw_g(h, qc, sfd, ssd):
                for j in range(4):
                    emit_uw_g_chunk(h, qc, j, sfd, ssd)

            def emit_uw_rot(h, qc, sf, ss, usw):
                qs = slice(qc * 512, (qc + 1) * 512)
                # h0 runs in phase A: muls 6 DVE / 2 Pool, M copies on the
                # idle Act. h1 runs during h0's attention where DVE only has
                # 3/8 exps: muls all DVE (fast), every op downstream of a
                # mul on Pool so the DVE queue never waits cross-engine.
                V, G, A = nc.vector, nc.gpsimd, nc.scalar
                if h == 0:
                    me = (V, G, V, V, V, G, V, V)
                    c0 = c1 = A.copy
                    s5 = V.tensor_sub
                else:
                    me = (V, V, V, V, V, V, V, V)
                    c0 = c1 = G.tensor_copy
                    s5 = G.tensor_sub
                # fast half: U = G*cos + Gc*sin ; W = Gc*cos - G*sin
                m1 = mst.tile([P, 512], BF16, tag="m1")
                m2 = mst.tile([P, 512], BF16, tag="m2")
                m3 = mst.tile([P, 512], BF16, tag="m3")
                m4 = mst.tile([P, 512], BF16, tag="m4")
                me[0].tensor_mul(m1[:], sf[:, 0:512], rot_s[:, 0, qs])
                me[1].tensor_mul(m2[:], sf[:, 512:1024], rot_s[:, 2, qs])
                me[2].tensor_mul(m3[:], sf[:, 512:1024], rot_s[:, 0, qs])
                me[3].tensor_mul(m4[:], sf[:, 0:512], rot_s[:, 2, qs])
                ubf = mst.tile([P, 512], BF16, tag="ubf")
                wbf = mst.tile([P, 512], BF16, tag="wbf")
                nc.gpsimd.tensor_add(ubf[:], m1[:], m2[:])
                nc.gpsimd.tensor_sub(wbf[:], m3[:], m4[:])
                c0(M[h][:, 0, qc, :], ubf[:])
                c1(M[h][:, 1, qc, :], wbf[:])
                # half lo-comp into the pad slot (partition-aligned halves)
                s5(M[h][0:D, 5, qc, :], wbf[0:D, :], M[h][0:D, 1, qc, :])
                s5(M[h][D:P, 5, qc, :], ubf[D:P, :], M[h][D:P, 0, qc, :])
                # slow half: rotate; compression happens in emit_uw_cheb
                m5 = mst.tile([P, 512], BF16, tag="m1", name="m5")
                m6 = mst.tile([P, 512], BF16, tag="m2", name="m6")
                m7 = mst.tile([P, 512], BF16, tag="m3", name="m7")
                m8 = mst.tile([P, 512], BF16, tag="m4", name="m8")
                me[4].tensor_mul(m5[:], ss[:, 0:512], rot_s[:, 1, qs])
                me[5].tensor_mul(m6[:], ss[:, 512:1024], rot_s[:, 3, qs])
                me[6].tensor_mul(m7[:], ss[:, 512:1024], rot_s[:, 1, qs])
                me[7].tensor_mul(m8[:], ss[:, 0:512], rot_s[:, 3, qs])
                nc.gpsimd.tensor_add(usw[:, 0, :], m5[:], m6[:])
                nc.gpsimd.tensor_sub(usw[:, 1, :], m7[:], m8[:])

            def emit_uw_cheb(h, qc, usw, pc=None):
                # cheb coefs land on the head's opposite partition half
                po = (1 - h) * D
                cs = slice(po, po + NT)
                if pc is None:
                    pc = small_psum([P, 512], "pc")
                for k in range(2):
                    nc.tensor.matmul(
                        pc[cs, :], sc_s[:, k, :], usw[:, k, :],
                        start=(k == 0), stop=(k == 1),
                    )
                nc.scalar.copy(M[h][cs, 2, qc, :], pc[cs, :])
                nc.vector.tensor_sub(
                    M[h][cs, 3, qc, :], pc[cs, :], M[h][cs, 2, qc, :]
                )

            def emit_k(kc):
                pk = small_psum([P, 512], "pk")
                for c in range(NEC):
                    nc.tensor.matmul(
                        pk[:],
                        wk2_s[:, c, :],
                        axT[c][:, kc * 512 : (kc + 1) * 512],
                        start=(c == 0),
                        stop=(c == NEC - 1),
                    )
                ks = slice(4 * kc, 4 * kc + 4)
                for h in range(HEADS_PER_CORE):
                    hp = slice(h * D, (h + 1) * D)
                    pkv = pk[hp, :].rearrange("p (t j) -> p t j", j=P)
                    nc.scalar.copy(SgA[h][hp, 0, ks, :], pkv)
                    nc.vector.tensor_sub(
                        SgA[h][hp, 1, ks, :], pkv, SgA[h][hp, 0, ks, :]
                    )

            def emit_v(h, g, tag=None):
                hs = slice(h * D, (h + 1) * D)
                pv = small_psum([P, 512], "pv", tag=tag)
                for k8 in range(8):
                    kt = g * 8 + k8
                    for c in range(NEC):
                        nc.tensor.matmul(
                            pv[:, k8 * D : (k8 + 1) * D],
                            axT[c][:, kt * P : (kt + 1) * P],
                            wv2_s[:, c, hs],
                            start=(c == 0),
                            stop=(c == NEC - 1),
                        )
                nc.scalar.copy(
                    vo[h][:, g * 8 : (g + 1) * 8, 0:D],
                    pv[:].rearrange("p (t d) -> p t d", d=D),
                )

            # h0 UW fully in phase A (streaming); h1's G copies land in a
            # persistent tile recycled from axT's tag so h1's rotation
            # (engine-only) can run during h0's attention.
            h1b = persist.tile(
                [P, NQC, 6, 512], BF16, tag="axT", name="h1buf"
            )
            h1buf = [h1b[:, u, :, :] for u in range(NQC)]

            # x keys (kc 4..7, v groups 2..3) first: their axT DMA lands well
            # before the history half. All G emissions go early (their
            # rotations then stream on DVE/Pool); chebs are deferred so the
            # PE never waits on a rotation.
            uwt = []
            for u in range(NQC):
                sf = gst.tile([P, 1024], BF16, tag="sf")
                ss = gst.tile([P, 1024], BF16, tag="ss")
                usw = gst.tile([P, 2, 512], BF16, tag="usw")
                uwt.append((sf, ss, usw))
                emit_uw_g(0, u, sf[:], ss[:])
                emit_uw_rot(0, u, sf, ss, usw)
                emit_k(4 + u)
            for g in (2, 3):
                emit_v(0, g)
                emit_v(1, g)
            emit_uw_cheb(0, 0, uwt[0][2])
            emit_uw_cheb(0, 1, uwt[1][2])
            for u in range(NQC):
                emit_k(u)
            emit_uw_cheb(0, 2, uwt[2][2])
            for g in (0, 1):
                emit_v(0, g)
                emit_v(1, g)
            emit_uw_cheb(0, 3, uwt[3][2])

            # dups via DMA (off-engine): M chunk 3 q-half <- chunk 2 q-half;
            # M chunk 4 cheb-half <- chunk 2 cheb-half (h0 now, h1 after its
            # cheb block); SgA chunk 2 <- chunk 0
            for h in range(HEADS_PER_CORE):
                hp = slice(h * D, (h + 1) * D)
                nc.sync.dma_start(M[h][hp, 3, :, :], M[h][hp, 2, :, :])
                nc.sync.dma_start(SgA[h][hp, 2, :, :], SgA[h][hp, 0, :, :])
            cs0 = slice(D, D + NT)
            nc.sync.dma_start(M[0][cs0, 4, :, :], M[0][cs0, 2, :, :])

            # ---------------- phase B: attention ----------------
            # Unit = one (key tile, query chunk): score psum is a 1-bank
            # [P, 512] tile from the 5-deep pr pool, so the
            # ps -> exp -> frees-slot chain never stalls the PE. exp
            # alternates Act (exact) / DVE (Schraudolph) per unit; during
            # h0's attention DVE also carries h1's rotation, so it only
            # takes 3 of 8 exps there.
            _expctr = [0]
            _dve_exp = {0: (1, 3, 5), 1: (1, 3, 5, 7)}

            def emit_av(h, kt, qc, pE, avv):
                for qt in range(4):
                    qg = qc * 4 + qt
                    bk, sl = divmod(qg, 6)
                    nc.tensor.matmul(
                        avv[bk][:, sl, :],
                        pE[:, qt * P : (qt + 1) * P],
                        vo[h][:, kt, 0:65],
                        start=(kt == 0 and qg in (0, 6, 12)),
                        stop=(kt == NKT - 1 and qg in (5, 11, 15)),
                        skip_group_check=True,
                    )

            def emit_unit(h, kt, qc, avv, pend):
                ps = pr.tile([P, 512], F32, tag="sp", name="ps")
                nc.tensor.matmul(
                    ps[:], SgF[:, kt, :, :], M[h][:, 0:2, qc, :],
                    start=True, stop=False, perf_mode=DR,
                )
                nc.tensor.matmul(
                    ps[:], SgA[h][:, 0:2, kt, :], M[h][:, 2:4, qc, :],
                    start=False, stop=False, perf_mode=DR,
                )
                nc.tensor.matmul(
                    ps[:], SgA[h][:, 2:4, kt, :], M[h][:, 4:6, qc, :],
                    start=False, stop=True, perf_mode=DR,
                )
                if qc in pend:
                    pkt, pE = pend.pop(qc)
                    emit_av(h, pkt, qc, pE, avv)
                et = est.tile([P, 512], BF16, tag="E")
                if _expctr[0] % 8 not in _dve_exp[h]:
                    nc.scalar.activation(
                        et[:], ps[:], AF.Exp, scale=0.125, bias=nbias[:]
                    )
                else:
                    # Schraudolph: int16 bits = 128*(log2e*(s/8 - c) + 127)
                    nc.vector.tensor_scalar(
                        et[:].bitcast(I16), ps[:],
                        0.125 * P * LOG2E,
                        P * 127.0 - SCORE_SHIFT * P * LOG2E - 8.5,
                        ALU.mult, ALU.add,
                    )
                _expctr[0] += 1
                pend[qc] = (kt, et)

            def emit_av_flush(h, avv, pend):
                for qc, (pkt, pE) in sorted(pend.items()):
                    emit_av(h, pkt, qc, pE, avv)
                pend.clear()

            def emit_z(h, avv):
                # av is query-major with the ones-column z in slot 64: copy
                # the z columns, take the reciprocal, then write numTT
                # PRE-SCALED by 1/z (per-partition scalar per s-tile) so the
                # out-projection result needs no further scaling.
                zc = zcs[h]
                nc.vector.tensor_copy(zc[:, 0:6], avv[0][:, :, 64])
                nc.vector.tensor_copy(zc[:, 6:12], avv[1][:, :, 64])
                nc.vector.tensor_copy(zc[:, 12:16], avv[2][:, :, 64])
                zrec = persist.tile([P, NS], F32, tag=f"zrec{h}", name=f"zrec{h}")
                nc.vector.reciprocal(zrec[:], zc[:])
                ntt = numTT[h]
                for s in range(NS):
                    bk, sl = divmod(s, 6)
                    if s % 2 == 0:
                        nc.scalar.activation(
                            ntt[:, s, :], avv[bk][:, sl, 0:D], AF.Copy,
                            scale=zrec[:, s : s + 1],
                        )
                    else:
                        nc.vector.tensor_scalar_mul(
                            ntt[:, s, :], avv[bk][:, sl, 0:D],
                            zrec[:, s : s + 1],
                        )

            def emit_z_tr(h, s2):
                # transpose one PAIR of numerator s-tiles ([128,128] block)
                # back to d-major via the DMA xbar (off-engine)
                nc.sync.dma_start_transpose(
                    numT[h][:, s2, :],
                    numTT[h][:, 2 * s2 : 2 * s2 + 2, :],
                )

            def emit_z_tr_pe(h, s2, copy_eng):
                pz = pr.tile([P, P], BF16, tag="sp", name="pz")
                nc.tensor.transpose(
                    pz[:], numTT[h][:, 2 * s2 : 2 * s2 + 2, :], identb[:]
                )
                copy_eng(numT[h][:, s2, :], pz[:])

            def emit_out_s(h, s):
                # numT is pre-scaled by 1/z, so the psum->sbuf conversion is
                # a plain copy (alternating Act/DVE to spread the load)
                po = pr.tile([P, 512], F32, tag="sp", name="po")
                hp = (s % 2) * D
                nc.tensor.matmul(
                    po[:], numT[h][hp : hp + D, s // 2, :],
                    wo_s[hp : hp + D, h, :],
                    start=True, stop=True,
                )
                if s % 2 == 0:
                    nc.scalar.copy(out_acc[:, s, :], po[:])
                else:
                    nc.vector.tensor_copy(out_acc[:, s, :], po[:])
                if h == 0:
                    nc.sync.dma_start(
                        oA_d[:].rearrange("(s p) e -> p s e", p=P)[:, s, :],
                        out_acc[:, s, :],
                    )
                elif s % 4 == 3:
                    # h1 drains at the tail: batch 4 s-tiles per DMA
                    nc.sync.dma_start(
                        oB_d[:].rearrange("(s p) e -> p s e", p=P)[:, s - 3 : s + 1, :],
                        out_acc[:, s - 3 : s + 1, :],
                    )

            # h0 attention with h1's G/rotation/cheb interleaved (their
            # elementwise runs on Pool/Act; DVE carries the exp stream)
            av0 = [
                ph.tile([P, 6 if j < 2 else 4, 65], F32, tag=f"bank{j}",
                        name=f"av0{j}")
                for j in range(3)
            ]
            pend0 = {}
            for kt in range(NKT):
                for qc in range(NQC):
                    emit_unit(0, kt, qc, av0, pend0)
                # h1 prep spread: per u-block of 7 kts, one G chunk per kt
                # (each borrows one score-stream slot briefly), then the
                # rotation (engine-only); cheb trails the rotation by a full
                # block so the Pool finishing ops have drained.
                if 1 <= kt <= 28:
                    u, ph7 = divmod(kt - 1, 7)
                    if ph7 < 4:
                        emit_uw_g_chunk(
                            1, u, ph7,
                            h1buf[u][:, 0:2, :].rearrange("p a b -> p (a b)"),
                            h1buf[u][:, 2:4, :].rearrange("p a b -> p (a b)"),
                        )
                    elif ph7 == 4:
                        emit_uw_rot(
                            1, u,
                            h1buf[u][:, 0:2, :].rearrange("p a b -> p (a b)"),
                            h1buf[u][:, 2:4, :].rearrange("p a b -> p (a b)"),
                            h1buf[u][:, 4:6, :],
                        )
                if kt in (13, 20, 27, 30):
                    # pr slot: the ph banks are held by av0 here (a ph
                    # allocation would deadlock the in-order PE queue)
                    u = (13, 20, 27, 30).index(kt)
                    emit_uw_cheb(
                        1, u, h1buf[u][:, 4:6, :],
                        pc=pr.tile([P, 512], F32, tag="sp", name="pc1"),
                    )

            emit_av_flush(0, av0, pend0)
            emit_z(0, av0)
            cs1 = slice(0, NT)
            nc.sync.dma_start(M[1][cs1, 4, :, :], M[1][cs1, 2, :, :])

            # h1 attention with h0's transpose + output projection streamed
            # (out tile s at kt = 6 + 3s//2, i.e. 2 tiles per 3 key tiles)
            _out_sched = {6 + (3 * s) // 2: s for s in range(NS)}
            av1 = [
                ph.tile([P, 6 if j < 2 else 4, 65], F32, tag=f"bank{j}",
                        name=f"av1{j}")
                for j in range(3)
            ]
            pend1 = {}
            for kt in range(NKT):
                for qc in range(NQC):
                    emit_unit(1, kt, qc, av1, pend1)
                if 1 <= kt <= 8:
                    emit_z_tr(0, kt - 1)
                if kt in _out_sched:
                    emit_out_s(0, _out_sched[kt])
            emit_av_flush(1, av1, pend1)
            emit_z(1, av1)
            for s2 in range(NS // 2):
                emit_z_tr_pe(
                    1, s2, nc.vector.tensor_copy if s2 % 2 else nc.scalar.copy
                )
                emit_out_s(1, 2 * s2)
                emit_out_s(1, 2 * s2 + 1)

    nc.compile()
    return nc


_NC_CACHE = None


def _get_program():
    global _NC_CACHE
    if _NC_CACHE is None:
        _NC_CACHE = build_program()
    return _NC_CACHE


def _fp8_hl(x):
    hi = np.clip(np.asarray(x, np.float32), -240, 240).astype(ml_dtypes.float8_e4m3)
    lo = np.clip(
        np.asarray(x, np.float32) - hi.astype(np.float32), -240, 240
    ).astype(ml_dtypes.float8_e4m3)
    return hi, lo


def make_in_maps(x, history, w_q, w_k, w_v, w_kr, w_o, u_bias, v_bias):
    bf = ml_dtypes.bfloat16
    all_x = np.concatenate([history, x], axis=1)  # [B, HpN, E]

    inv_freq = 1.0 / (10000.0 ** (np.arange(0, E, 2, dtype=np.float64) / E))  # [256]
    ang_f = np.outer(inv_freq[:128], np.arange(HpN, dtype=np.float64) - H)
    xn = (np.arange(HpN, dtype=np.float64) - H) / 2048.0
    T = np.polynomial.chebyshev.chebvander(xn, NT - 1)  # [HpN, NT]
    ang_s = np.outer(xn * 2048.0, inv_freq[128:256])  # [HpN, 128]
    tgt = np.concatenate([np.sin(ang_s), np.cos(ang_s)], axis=1)  # [HpN, 256]
    coef, *_ = np.linalg.lstsq(T, tgt, rcond=None)  # [NT, 256]
    sc = np.ascontiguousarray(coef.T)  # [256, NT]: rows 0-127 sin, 128-255 cos

    sin_hi, _ = _fp8_hl(np.sin(ang_f))
    cos_hi, _ = _fp8_hl(np.cos(ang_f))
    T_hi, T_lo = _fp8_hl(T.T)  # [NT, HpN]
    sin_f = sin_hi.astype(np.float32)
    cos_f = cos_hi.astype(np.float32)
    # SgF partition-major: [p][t][c][j], chunks c = [sin_hi, cos_hi]
    psiF = np.ascontiguousarray(
        np.stack(
            [sin_f.reshape(P, NKT, P), cos_f.reshape(P, NKT, P)], axis=2
        ).reshape(P, NKT * 2 * P)
    )
    # shared cheb T basis [p(64)][hi/lo][t][j] (device places it per head)
    psiT = np.ascontiguousarray(
        np.stack(
            [
                T_hi.astype(np.float32).reshape(NT, NKT, P),
                T_lo.astype(np.float32).reshape(NT, NKT, P),
            ],
            axis=1,
        ).reshape(NT, 2 * NKT * P)
    )
    # fast-psi half-compensation stationary [cos_hi(f0:64)|sin_hi(f64:128)]:
    # pairs with M chunk 5 = [Wlo(f0:64)|Ulo(f64:128)]
    psiC = np.ascontiguousarray(
        np.concatenate([cos_f[0:D], sin_f[D:P]], axis=0).reshape(P, NKT * P)
    )

    ang_b = np.outer(inv_freq, np.arange(N, dtype=np.float64))  # [256, N]
    rot = np.ascontiguousarray(
        np.concatenate([np.cos(ang_b), np.sin(ang_b)]).astype(bf)
    )  # [512, N]: rows 0:128 cos-fast, 128:256 cos-slow, 256:384 sin-fast, ...

    clip8 = lambda a: np.clip(a, -240, 240).astype(ml_dtypes.float8_e4m3)

    in_maps = []
    for c in range(N_CORES):
        b = c // 4
        h0 = HEADS_PER_CORE * (c % 4)
        axT = np.ascontiguousarray(all_x[b].T).astype(bf)
        wq2 = np.concatenate([w_q[h0], w_q[h0 + 1]], axis=1).astype(bf)  # [E, 128]
        wk2 = np.concatenate([w_k[h0], w_k[h0 + 1]], axis=1).astype(bf)
        wv2 = np.concatenate([w_v[h0], w_v[h0 + 1]], axis=1).astype(bf)
        wkrT = np.concatenate(
            [w_kr[h0].T, w_kr[h0 + 1].T], axis=0
        ).astype(bf)  # [128, E]: rows 0:64 = head0 (d), 64:128 = head1
        wo1h = np.stack([w_o[h0], w_o[h0 + 1]], axis=1).reshape(D, 2 * E)
        wo2 = np.concatenate([wo1h, wo1h], axis=0).astype(bf)  # [P, 2E]
        in_maps.append(
            {
                "axT": axT,
                "rot": rot,
                "psiF": clip8(psiF),
                "psiT": clip8(psiT),
                "psiC": clip8(psiC),
                "sc": np.ascontiguousarray(sc).astype(bf),
                "wq2": np.ascontiguousarray(wq2),
                "wk2": np.ascontiguousarray(wk2),
                "wv2": np.ascontiguousarray(wv2),
                "wkrT": np.ascontiguousarray(wkrT),
                "wo2": np.ascontiguousarray(wo2),
                "ub2": np.ascontiguousarray(
                    np.concatenate([u_bias[h0], u_bias[h0 + 1]]).reshape(P, 1)
                ).astype(np.float32),
                "vb2": np.ascontiguousarray(
                    np.concatenate([v_bias[h0], v_bias[h0 + 1]]).reshape(P, 1)
                ).astype(np.float32),
            }
        )
    return in_maps


def run(inputs, trace=False, **kw):
    from concourse.bass_utils import run_bass_kernel_spmd

    nc = _get_program()
    in_maps = make_in_maps(
        np.asarray(inputs["x"], np.float32),
        np.asarray(inputs["history"], np.float32),
        np.asarray(inputs["w_q"], np.float32),
        np.asarray(inputs["w_k"], np.float32),
        np.asarray(inputs["w_v"], np.float32),
        np.asarray(inputs["w_kr"], np.float32),
        np.asarray(inputs["w_o"], np.float32),
        np.asarray(inputs["u_bias"], np.float32),
        np.asarray(inputs["v_bias"], np.float32),
    )
    res = run_bass_kernel_spmd(nc, in_maps, list(range(N_CORES)), trace=trace, **kw)
    out = np.zeros((B, N, E), np.float32)
    for c in range(N_CORES):
        out[c // 4] += res.results[c]["oA"].astype(np.float32).reshape(N, E)
        out[c // 4] += res.results[c]["oB"].astype(np.float32).reshape(N, E)
    return out, res


def kernel(**inputs):
    # mask is all ones (per the problem spec), so score masking is a no-op
    # and the tensor is ignored.
    out, _ = run(inputs, trace=False)
    return out



# revision 93
# speedup vs baseline: 1.2013x; 1.0041x over previous
"""Transformer-XL multi-head self-attention on 8 Trainium2 NeuronCores.

Sharding: core c handles batch b = c//4 and heads {2*(c%4), 2*(c%4)+1}
(data-parallel over B x tensor-parallel over heads). Each core produces a
partial [N, E] output (its heads' w_o contributions); the host sums the 4
partials per batch element.

The XL relative-position term BD[i,j] = (q_i+v)·BDk[j-i+N-1] is computed
without the rel_shift gather via per-query rotation (angle-difference
identities): BD^T = Psi @ UW with Psi a shape-derived constant basis
(128 exact sin rows + 128 exact cos rows + 64 Chebyshev rows for the slow
frequencies) and UW per-query rotated coefficients.

Scores run on the PE in fp8e4 DoubleRow mode (0.5 cycles/row in the cost
model) with hi/lo error compensation: a bf16-accurate operand x is split
as x = hi + lo with hi = fp8(x), lo = fp8(x - hi), keeping selected cross
terms. Per 128-key tile the contraction is 6 chunks of 128 rows consumed
by 3 DoubleRow calls:
  [sin|cos]x[Uhi|Whi],
  [khi|Thi]x[qhi|chi], [klo|Thi]x[qhi|clo], [khi|Tlo]x[qlo|chi], pad
where T/c are the Chebyshev basis/coefficients and k/q carry the content
term (q+u)·k. The U/W (fast psi coefficient) lo-compensation is dropped
(one-sided both psi and U/W): host-side simulation puts the end-to-end
max-rel error at ~1.4% vs the 2% gate (vs ~1.2% with the compensation).
The value path (exp, V, attn@V, output projection) stays in bf16: fp8
noise there does not average out. exp is spread over Act/DVE/Pool
(Schraudolph on DVE/Pool; the extra Schraudolph noise is ~free: ~1.47%
even if every tile uses it).
"""

import sys

sys.path.insert(0, "/opt/trn_rl_repo")

import ml_dtypes
import numpy as np

import concourse.bass as bass
import concourse.mybir as mybir
from concourse import bacc
from concourse.masks import make_identity
from concourse.tile import TileContext

F32 = mybir.dt.float32
BF16 = mybir.dt.bfloat16
FP8 = mybir.dt.float8e4
I16 = mybir.dt.int16
AF = mybir.ActivationFunctionType
ALU = mybir.AluOpType
DR = mybir.MatmulPerfMode.DoubleRow

B, N, H, E, NH, D = 2, 2048, 2048, 512, 8, 64
HpN = H + N  # 4096
P = 128
NKT = HpN // P  # 32 key tiles
NPAIR = NKT // 2  # 16 key-tile pairs
NQC = N // 512  # 4 query chunks of 512
NEC = E // P  # 4 contraction chunks over E
NS = N // P  # 16 output row tiles
NT = 64  # chebyshev terms
HEADS_PER_CORE = 2
N_CORES = 8

LOG2E = 1.4426950408889634
SCORE_SHIFT = 1.5  # exp(s - c): cancels in softmax, bounds exp values
# exp tile engine rotation: (ctr % MOD) -> r < EXP_ACT on Act (exact),
# rest on DVE (Schraudolph). GPSIMD cannot read PSUM so Pool is out.
# Strict alternation: consecutive units' exps overlap across the two
# engines (each engine sees one ~1.1us exp per two 858ns PE units).
EXP_MOD, EXP_ACT = 2, 1


def build_program():
    nc = bacc.Bacc("TRN2", target_bir_lowering=False, debug=False)

    axT_d = nc.declare_dram_parameter("axT", [E, HpN], BF16, isOutput=False)
    rot_d = nc.declare_dram_parameter("rot", [E, N], BF16, isOutput=False)
    # SgF: shared fast-psi chunks, partition-major [p][t][c][j] so the DMA is
    # an identity layout with 8KB per-partition runs
    psiF_d = nc.declare_dram_parameter("psiF", [P, NKT * 2 * P], FP8, isOutput=False)
    # shared cheb T basis rows [p(64)][hi/lo][t][j]; identical for both heads
    # (placed at opposite partition halves on device)
    psiT_d = nc.declare_dram_parameter("psiT", [NT, 2 * NKT * P], FP8, isOutput=False)
    # fast-psi half-compensation stationary [cos_hi(f0:64)|sin_hi(f64:128)],
    # shared by both heads: [p][t][j]
    psiC_d = nc.declare_dram_parameter("psiC", [P, NKT * P], FP8, isOutput=False)
    sc_d = nc.declare_dram_parameter("sc", [2 * P, NT], BF16, isOutput=False)
    wq2_d = nc.declare_dram_parameter("wq2", [E, P], BF16, isOutput=False)
    wk2_d = nc.declare_dram_parameter("wk2", [E, P], BF16, isOutput=False)
    wv2_d = nc.declare_dram_parameter("wv2", [E, P], BF16, isOutput=False)
    wkrT_d = nc.declare_dram_parameter("wkrT", [P, E], BF16, isOutput=False)
    # wo duplicated on both partition halves (odd numT s-tiles live at 64:128)
    wo2_d = nc.declare_dram_parameter("wo2", [P, 2 * E], BF16, isOutput=False)
    ub2_d = nc.declare_dram_parameter("ub2", [P, 1], F32, isOutput=False)
    vb2_d = nc.declare_dram_parameter("vb2", [P, 1], F32, isOutput=False)
    # two per-head partial outputs (host sums): h0 streams during h1's
    # attention; h1 drains at the tail
    oA_d = nc.declare_dram_parameter("oA", [N, E], BF16, isOutput=True)
    oB_d = nc.declare_dram_parameter("oB", [N, E], BF16, isOutput=True)

    with TileContext(nc) as tc:
        with (
            tc.tile_pool(name="persist", bufs=1) as persist,
            tc.tile_pool(name="gst", bufs=4) as gst,       # G copies stream
            tc.tile_pool(name="mst", bufs=2) as mst,       # rotation temps
            tc.tile_pool(name="est", bufs=6) as est,       # exp tiles
            tc.tile_pool(name="dram", bufs=1, space="DRAM") as dram_pool,
            tc.tile_pool(name="pr", bufs=5, space="PSUM") as pr,   # 5x [P,512]
            tc.tile_pool(name="ph", bufs=1, space="PSUM") as ph,   # 3x [P,512]
        ):
            _sm = [0]

            def small_psum(shape, name, dtype=F32, tag=None):
                if tag is None:
                    i = _sm[0] % 3
                    _sm[0] += 1
                    tag = f"bank{i}"
                return ph.tile(shape, dtype, tag=tag, name=name)

            # ---------------- DMAs ----------------
            # One prioritized stream on the sync queue: the DMA engines are a
            # serialized resource, so emission order here IS the priority.
            # q proj needs {wq2, x-half}; the uw chain adds {wkr, rot, sc};
            # emit_k(4..7)/emit_v(x) add {wk2, wv2}; history keys come next,
            # then the attention-only psi tables and wo.
            wq2_s = persist.tile([P, NEC, P], BF16, tag="wq2")
            nc.sync.dma_start(wq2_s[:], wq2_d[:].rearrange("(c p) d -> p c d", p=P))
            # wkr stacked on partitions: rows 0:64 = head0 d, 64:128 = head1 d
            wkr_s = persist.tile([P, NEC, P], BF16, tag="wkr")
            nc.sync.dma_start(
                wkr_s[:], wkrT_d[:].rearrange("p (c e) -> p c e", c=NEC)
            )
            ub_s = persist.tile([P, 1], F32, tag="ub")
            nc.sync.dma_start(ub_s[:], ub2_d[:])
            vb_s = persist.tile([P, 1], F32, tag="vb")
            nc.sync.dma_start(vb_s[:], vb2_d[:])
            axT_s = persist.tile([P, NEC, HpN], BF16, tag="axT", name="axT")
            axT = [axT_s[:, c, :] for c in range(NEC)]
            for r in range(4):
                ks = slice(H + r * 512, H + (r + 1) * 512)
                nc.sync.dma_start(
                    axT_s[:, :, ks],
                    axT_d[:, ks].rearrange("(c p) k -> p c k", p=P),
                )
            rot_s = persist.tile([P, 4, N], BF16, tag="rot")
            nc.sync.dma_start(rot_s[:, 0, :], rot_d[0:P, :])
            nc.sync.dma_start(rot_s[:, 2, :], rot_d[2 * P : 3 * P, :])
            nc.sync.dma_start(rot_s[:, 1, :], rot_d[P : 2 * P, :])
            nc.sync.dma_start(rot_s[:, 3, :], rot_d[3 * P : 4 * P, :])
            sc_s = persist.tile([P, 2, NT], BF16, tag="sc")
            nc.sync.dma_start(sc_s[:], sc_d[:].rearrange("(k p) r -> p k r", p=P))
            wk2_s = persist.tile([P, NEC, P], BF16, tag="wk2")
            nc.sync.dma_start(wk2_s[:], wk2_d[:].rearrange("(c p) d -> p c d", p=P))
            wv2_s = persist.tile([P, NEC, P], BF16, tag="wv2")
            nc.sync.dma_start(wv2_s[:], wv2_d[:].rearrange("(c p) d -> p c d", p=P))
            nc.sync.dma_start(
                axT_s[:, :, 0:1024],
                axT_d[:, 0:1024].rearrange("(c p) k -> p c k", p=P),
            )
            nc.sync.dma_start(
                axT_s[:, :, 1024:H],
                axT_d[:, 1024:H].rearrange("(c p) k -> p c k", p=P),
            )
            SgF = persist.tile([P, NKT, 2, P], FP8, tag="SgF")
            nc.sync.dma_start(
                SgF[:], psiF_d[:].rearrange("p (t c j) -> p t c j", c=2, j=P)
            )
            # SgA free layout is chunk-major [c][t][j] so partition-sliced
            # chunk DMAs have 4KB contiguous runs.
            SgA = []
            for h in range(HEADS_PER_CORE):
                t = persist.tile([P, 4, NKT, P], FP8, tag=f"SgA{h}", name=f"SgA{h}")
                SgA.append(t)
            for h in range(HEADS_PER_CORE):
                tp = (1 - h) * D
                tps = slice(tp, tp + NT)
                nc.sync.dma_start(
                    SgA[h][tps, 0, :, :],
                    psiT_d[:, 0 : NKT * P].rearrange("p (t j) -> p t j", j=P),
                )
                nc.sync.dma_start(
                    SgA[h][tps, 2, :, :],
                    psiT_d[:, NKT * P :].rearrange("p (t j) -> p t j", j=P),
                )
            for h in range(HEADS_PER_CORE):
                nc.sync.dma_start(
                    SgA[h][:, 3, :, :],
                    psiC_d[:].rearrange("p (t j) -> p t j", j=P),
                )
                # chunk1's T-half duplicates chunk0's (device-side dup)
                tps = slice((1 - h) * D, (1 - h) * D + NT)
                nc.sync.dma_start(SgA[h][tps, 1, :, :], SgA[h][tps, 0, :, :])
            wo_s = persist.tile([P, 2, E], BF16, tag="wo")
            nc.sync.dma_start(wo_s[:], wo2_d[:].rearrange("p (h e) -> p h e", h=2))

            identb = persist.tile([P, P], BF16, tag="identb")
            make_identity(nc, identb[:])

            # ---------------- persistent compute tiles ----------------
            # M chunks per head: 0=Uhi 1=Whi 2=[qhi|chi]
            # 3=[qhi-dup|clo] 4=[qlo|chi-dup] 5=[Wlo(f 0:64)|Ulo(f 64:128)]
            # (chunk 5 pairs with the psiA half-compensation stationary
            # [cos_hi(0:64)|sin_hi(64:128)] in the otherwise-wasted pad slot)
            M = []
            for h in range(HEADS_PER_CORE):
                m = persist.tile([P, 6, NQC, 512], FP8, tag=f"M{h}", name=f"M{h}")
                M.append(m)
            qv_s = persist.tile([P, N], BF16, tag="qv_s")
            vo = []
            for h in range(HEADS_PER_CORE):
                v = persist.tile([P, NKT, 66], BF16, tag=f"vo{h}", name=f"vo{h}")
                nc.gpsimd.memset(v[:, :, 64:66], 0.0)
                nc.gpsimd.memset(v[:, :, 64:65], 1.0)
                vo.append(v)
            # numTT: query-major pre-scaled numerators [q, s, d] (z separate);
            # numT: d-major via 128x128 transposes of s-tile PAIRS -- even
            # s-tile's d on partitions 0:64, odd on 64:128
            numT = []
            numTT = []
            zcs = []
            for h in range(HEADS_PER_CORE):
                t = persist.tile(
                    [P, NS // 2, P], BF16, tag=f"numT{h}", name=f"numT{h}"
                )
                numT.append(t)
                tt = persist.tile(
                    [P, NS, D], BF16, tag=f"numTT{h}", name=f"numTT{h}"
                )
                numTT.append(tt)
                zcs.append(
                    persist.tile([P, NS], F32, tag=f"zc{h}", name=f"zc{h}")
                )
            out_acc = persist.tile([P, NS, E], BF16, tag="out_acc")
            nbias = persist.tile([P, 1], F32, tag="nbias")
            nc.vector.memset(nbias[:], -SCORE_SHIFT)

            # ---------------- phase A: projections ----------------
            # q projection, both heads packed, emitted chunk-outer so the PE
            # starts as soon as each axT chunk lands. pq psums use the ph
            # banks (free until the av accumulators take them).
            pqs = [small_psum([P, 512], f"pq{qc}") for qc in range(NQC)]
            for c in range(NEC):
                for qc in range(NQC):
                    nc.tensor.matmul(
                        pqs[qc][:],
                        wq2_s[:, c, :],
                        axT[c][:, H + qc * 512 : H + (qc + 1) * 512],
                        start=(c == 0),
                        stop=(c == NEC - 1),
                    )
            for qc in range(NQC):
                pq = pqs[qc]
                qs = slice(qc * 512, (qc + 1) * 512)
                nc.vector.tensor_scalar_add(qv_s[:, qs], pq[:], vb_s[:])
                for h in range(HEADS_PER_CORE):
                    hp = slice(h * D, (h + 1) * D)
                    nc.vector.tensor_scalar_add(
                        M[h][hp, 2, qc, :], pq[hp, :], ub_s[hp]
                    )
                    nc.vector.scalar_tensor_tensor(
                        M[h][hp, 4, qc, :], pq[hp, :], ub_s[hp],
                        M[h][hp, 2, qc, :], ALU.add, ALU.subtract,
                    )

            def emit_uw_g_chunk(h, qc, j, sfd, ssd):
                # G: e 0:128 sin-fast + 256:384 cos-fast (sf);
                #    e 128:256 sin-slow + 384:512 cos-slow (ss)
                # one 1-bank psum + copy per chunk so at most one score-stream
                # slot is borrowed at a time
                hp = slice(h * D, (h + 1) * D)
                qs = slice(qc * 512, (qc + 1) * 512)
                half, jj = j // 2, j % 2
                dst = sfd if half == 0 else ssd
                g = pr.tile([P, 512], F32, tag="sp", name="g")
                nc.tensor.matmul(
                    g[:], wkr_s[hp, 2 * jj + half, :], qv_s[hp, qs],
                    start=True, stop=True,
                )
                nc.scalar.copy(dst[:, jj * 512 : (jj + 1) * 512], g[:])

            def emit_uw_g(h, qc, sfd, ssd):
                for j in range(4):
                    emit_uw_g_chunk(h, qc, j, sfd, ssd)

            def emit_uw_rot(h, qc, sf, ss, usw):
                qs = slice(qc * 512, (qc + 1) * 512)
                # h0 runs in phase A: muls 6 DVE / 2 Pool, M copies on the
                # idle Act. h1 runs during h0's attention where DVE only has
                # 3/8 exps: muls all DVE (fast), every op downstream of a
                # mul on Pool so the DVE queue never waits cross-engine.
                V, G, A = nc.vector, nc.gpsimd, nc.scalar
                if h == 0:
                    me = (V, V, V, G, V, V, V, G)
                    c0 = c1 = A.copy
                    s5 = G.tensor_sub
                else:
                    me = (V, V, V, V, V, V, V, V)
                    c0 = c1 = G.tensor_copy
                    s5 = G.tensor_sub
                # fast half: U = G*cos + Gc*sin ; W = Gc*cos - G*sin
                m1 = mst.tile([P, 512], BF16, tag="m1")
                m2 = mst.tile([P, 512], BF16, tag="m2")
                m3 = mst.tile([P, 512], BF16, tag="m3")
                m4 = mst.tile([P, 512], BF16, tag="m4")
                me[0].tensor_mul(m1[:], sf[:, 0:512], rot_s[:, 0, qs])
                me[1].tensor_mul(m2[:], sf[:, 512:1024], rot_s[:, 2, qs])
                me[2].tensor_mul(m3[:], sf[:, 512:1024], rot_s[:, 0, qs])
                me[3].tensor_mul(m4[:], sf[:, 0:512], rot_s[:, 2, qs])
                ubf = mst.tile([P, 512], BF16, tag="ubf")
                wbf = mst.tile([P, 512], BF16, tag="wbf")
                nc.gpsimd.tensor_add(ubf[:], m1[:], m2[:])
                nc.gpsimd.tensor_sub(wbf[:], m3[:], m4[:])
                c0(M[h][:, 0, qc, :], ubf[:])
                c1(M[h][:, 1, qc, :], wbf[:])
                # half lo-comp into the pad slot (partition-aligned halves)
                s5(M[h][0:D, 5, qc, :], wbf[0:D, :], M[h][0:D, 1, qc, :])
                s5(M[h][D:P, 5, qc, :], ubf[D:P, :], M[h][D:P, 0, qc, :])
                # slow half: rotate; compression happens in emit_uw_cheb
                m5 = mst.tile([P, 512], BF16, tag="m1", name="m5")
                m6 = mst.tile([P, 512], BF16, tag="m2", name="m6")
                m7 = mst.tile([P, 512], BF16, tag="m3", name="m7")
                m8 = mst.tile([P, 512], BF16, tag="m4", name="m8")
                me[4].tensor_mul(m5[:], ss[:, 0:512], rot_s[:, 1, qs])
                me[5].tensor_mul(m6[:], ss[:, 512:1024], rot_s[:, 3, qs])
                me[6].tensor_mul(m7[:], ss[:, 512:1024], rot_s[:, 1, qs])
                me[7].tensor_mul(m8[:], ss[:, 0:512], rot_s[:, 3, qs])
                nc.gpsimd.tensor_add(usw[:, 0, :], m5[:], m6[:])
                nc.gpsimd.tensor_sub(usw[:, 1, :], m7[:], m8[:])

            def emit_uw_cheb(h, qc, usw, pc=None):
                # cheb coefs land on the head's opposite partition half
                po = (1 - h) * D
                cs = slice(po, po + NT)
                if pc is None:
                    pc = small_psum([P, 512], "pc")
                for k in range(2):
                    nc.tensor.matmul(
                        pc[cs, :], sc_s[:, k, :], usw[:, k, :],
                        start=(k == 0), stop=(k == 1),
                    )
                nc.scalar.copy(M[h][cs, 2, qc, :], pc[cs, :])
                nc.vector.tensor_sub(
                    M[h][cs, 3, qc, :], pc[cs, :], M[h][cs, 2, qc, :]
                )

            def emit_k(kc):
                pk = small_psum([P, 512], "pk")
                for c in range(NEC):
                    nc.tensor.matmul(
                        pk[:],
                        wk2_s[:, c, :],
                        axT[c][:, kc * 512 : (kc + 1) * 512],
                        start=(c == 0),
                        stop=(c == NEC - 1),
                    )
                ks = slice(4 * kc, 4 * kc + 4)
                for h in range(HEADS_PER_CORE):
                    hp = slice(h * D, (h + 1) * D)
                    pkv = pk[hp, :].rearrange("p (t j) -> p t j", j=P)
                    nc.scalar.copy(SgA[h][hp, 0, ks, :], pkv)
                    nc.vector.tensor_sub(
                        SgA[h][hp, 1, ks, :], pkv, SgA[h][hp, 0, ks, :]
                    )

            def emit_v(h, g, tag=None):
                hs = slice(h * D, (h + 1) * D)
                pv = small_psum([P, 512], "pv", tag=tag)
                for k8 in range(8):
                    kt = g * 8 + k8
                    for c in range(NEC):
                        nc.tensor.matmul(
                            pv[:, k8 * D : (k8 + 1) * D],
                            axT[c][:, kt * P : (kt + 1) * P],
                            wv2_s[:, c, hs],
                            start=(c == 0),
                            stop=(c == NEC - 1),
                        )
                cp = nc.scalar.copy if (h + g) % 2 else nc.vector.tensor_copy
                cp(
                    vo[h][:, g * 8 : (g + 1) * 8, 0:D],
                    pv[:].rearrange("p (t d) -> p t d", d=D),
                )

            # h0 UW fully in phase A (streaming); h1's G copies land in a
            # persistent tile recycled from axT's tag so h1's rotation
            # (engine-only) can run during h0's attention.
            h1b = persist.tile(
                [P, NQC, 6, 512], BF16, tag="axT", name="h1buf"
            )
            h1buf = [h1b[:, u, :, :] for u in range(NQC)]

            # x keys (kc 4..7, v groups 2..3) first: their axT DMA lands well
            # before the history half. All G emissions go early (their
            # rotations then stream on DVE/Pool); chebs are deferred so the
            # PE never waits on a rotation.
            uwt = []
            for u in range(NQC):
                sf = gst.tile([P, 1024], BF16, tag="sf")
                ss = gst.tile([P, 1024], BF16, tag="ss")
                usw = gst.tile([P, 2, 512], BF16, tag="usw")
                uwt.append((sf, ss, usw))
                emit_uw_g(0, u, sf[:], ss[:])
                emit_uw_rot(0, u, sf, ss, usw)
                emit_k(4 + u)
            for g in (2, 3):
                emit_v(0, g)
                emit_v(1, g)
            emit_uw_cheb(0, 0, uwt[0][2])
            emit_uw_cheb(0, 1, uwt[1][2])
            for u in range(NQC):
                emit_k(u)
            emit_uw_cheb(0, 2, uwt[2][2])
            for g in (0, 1):
                emit_v(0, g)
                emit_v(1, g)
            emit_uw_cheb(0, 3, uwt[3][2])

            # dups via DMA (off-engine): M chunk 3 q-half <- chunk 2 q-half;
            # M chunk 4 cheb-half <- chunk 2 cheb-half (h0 now, h1 after its
            # cheb block); SgA chunk 2 <- chunk 0
            for h in range(HEADS_PER_CORE):
                hp = slice(h * D, (h + 1) * D)
                nc.sync.dma_start(M[h][hp, 3, :, :], M[h][hp, 2, :, :])
                nc.sync.dma_start(SgA[h][hp, 2, :, :], SgA[h][hp, 0, :, :])
            cs0 = slice(D, D + NT)
            nc.sync.dma_start(M[0][cs0, 4, :, :], M[0][cs0, 2, :, :])

            # ---------------- phase B: attention ----------------
            # Unit = one (key tile, query chunk): score psum is a 1-bank
            # [P, 512] tile from the 5-deep pr pool, so the
            # ps -> exp -> frees-slot chain never stalls the PE. exp
            # alternates Act (exact) / DVE (Schraudolph) per unit; during
            # h0's attention DVE also carries h1's rotation, so it only
            # takes 3 of 8 exps there.
            _expctr = [0]
            _dve_exp = {0: (1, 3, 5, 7), 1: (1, 3, 5, 7)}

            def emit_av(h, kt, qc, pE, avv):
                for qt in range(4):
                    qg = qc * 4 + qt
                    bk, sl = divmod(qg, 6)
                    nc.tensor.matmul(
                        avv[bk][:, sl, :],
                        pE[:, qt * P : (qt + 1) * P],
                        vo[h][:, kt, 0:65],
                        start=(kt == 0 and qg in (0, 6, 12)),
                        stop=(kt == NKT - 1 and qg in (5, 11, 15)),
                        skip_group_check=True,
                    )

            def emit_unit(h, kt, qc, avv, pend):
                ps = pr.tile([P, 512], F32, tag="sp", name="ps")
                nc.tensor.matmul(
                    ps[:], SgF[:, kt, :, :], M[h][:, 0:2, qc, :],
                    start=True, stop=False, perf_mode=DR,
                )
                nc.tensor.matmul(
                    ps[:], SgA[h][:, 0:2, kt, :], M[h][:, 2:4, qc, :],
                    start=False, stop=False, perf_mode=DR,
                )
                nc.tensor.matmul(
                    ps[:], SgA[h][:, 2:4, kt, :], M[h][:, 4:6, qc, :],
                    start=False, stop=True, perf_mode=DR,
                )
                if qc in pend:
                    pkt, pE = pend.pop(qc)
                    emit_av(h, pkt, qc, pE, avv)
                et = est.tile([P, 512], BF16, tag="E")
                if _expctr[0] % 8 not in _dve_exp[h]:
                    nc.scalar.activation(
                        et[:], ps[:], AF.Exp, scale=0.125, bias=nbias[:]
                    )
                else:
                    # Schraudolph: int16 bits = 128*(log2e*(s/8 - c) + 127)
                    nc.vector.tensor_scalar(
                        et[:].bitcast(I16), ps[:],
                        0.125 * P * LOG2E,
                        P * 127.0 - SCORE_SHIFT * P * LOG2E - 8.5,
                        ALU.mult, ALU.add,
                    )
                _expctr[0] += 1
                pend[qc] = (kt, et)

            def emit_av_flush(h, avv, pend):
                for qc, (pkt, pE) in sorted(pend.items()):
                    emit_av(h, pkt, qc, pE, avv)
                pend.clear()

            def emit_z_head(h, avv):
                # av is query-major with the ones-column z in slot 64: copy
                # the z columns and take the per-partition reciprocal
                zc = zcs[h]
                nc.vector.tensor_copy(zc[:, 0:6], avv[0][:, :, 64])
                nc.vector.tensor_copy(zc[:, 6:12], avv[1][:, :, 64])
                nc.vector.tensor_copy(zc[:, 12:16], avv[2][:, :, 64])
                zrec = persist.tile([P, NS], F32, tag=f"zrec{h}", name=f"zrec{h}")
                nc.vector.reciprocal(zrec[:], zc[:])
                return zrec

            def emit_z_scale(h, avv, zrec, s):
                # write numTT PRE-SCALED by 1/z (per-partition scalar per
                # s-tile) so the out-projection result needs no scaling
                bk, sl = divmod(s, 6)
                if s % 2 == 0:
                    nc.scalar.activation(
                        numTT[h][:, s, :], avv[bk][:, sl, 0:D], AF.Copy,
                        scale=zrec[:, s : s + 1],
                    )
                else:
                    nc.vector.tensor_scalar_mul(
                        numTT[h][:, s, :], avv[bk][:, sl, 0:D],
                        zrec[:, s : s + 1],
                    )

            def emit_z(h, avv):
                zrec = emit_z_head(h, avv)
                for s in range(NS):
                    emit_z_scale(h, avv, zrec, s)
                return zrec

            def emit_z_tr(h, s2):
                # transpose one PAIR of numerator s-tiles ([128,128] block)
                # back to d-major via the DMA xbar (off-engine)
                nc.sync.dma_start_transpose(
                    numT[h][:, s2, :],
                    numTT[h][:, 2 * s2 : 2 * s2 + 2, :],
                )

            def emit_z_tr_pe(h, s2, copy_eng):
                pz = pr.tile([P, P], BF16, tag="sp", name="pz")
                nc.tensor.transpose(
                    pz[:], numTT[h][:, 2 * s2 : 2 * s2 + 2, :], identb[:]
                )
                copy_eng(numT[h][:, s2, :], pz[:])

            def emit_out_s(h, s):
                # numT is pre-scaled by 1/z, so the psum->sbuf conversion is
                # a plain copy (alternating Act/DVE to spread the load)
                po = pr.tile([P, 512], F32, tag="sp", name="po")
                hp = (s % 2) * D
                nc.tensor.matmul(
                    po[:], numT[h][hp : hp + D, s // 2, :],
                    wo_s[hp : hp + D, h, :],
                    start=True, stop=True,
                )
                if s % 2 == 0:
                    nc.scalar.copy(out_acc[:, s, :], po[:])
                else:
                    nc.vector.tensor_copy(out_acc[:, s, :], po[:])
                if h == 0:
                    nc.sync.dma_start(
                        oA_d[:].rearrange("(s p) e -> p s e", p=P)[:, s, :],
                        out_acc[:, s, :],
                    )
                elif s % 4 == 3:
                    # h1 drains at the tail: batch 4 s-tiles per DMA
                    nc.sync.dma_start(
                        oB_d[:].rearrange("(s p) e -> p s e", p=P)[:, s - 3 : s + 1, :],
                        out_acc[:, s - 3 : s + 1, :],
                    )

            # h0 attention with h1's G/rotation/cheb interleaved (their
            # elementwise runs on Pool/Act; DVE carries the exp stream)
            av0 = [
                ph.tile([P, 6 if j < 2 else 4, 65], F32, tag=f"bank{j}",
                        name=f"av0{j}")
                for j in range(3)
            ]
            pend0 = {}
            for kt in range(NKT):
                for qc in range(NQC):
                    emit_unit(0, kt, qc, av0, pend0)
                # h1 prep spread: per u-block of 7 kts, one G chunk per kt
                # (each borrows one score-stream slot briefly), then the
                # rotation (engine-only); cheb trails the rotation by a full
                # block so the Pool finishing ops have drained.
                if 1 <= kt <= 28:
                    u, ph7 = divmod(kt - 1, 7)
                    if ph7 < 4:
                        emit_uw_g_chunk(
                            1, u, ph7,
                            h1buf[u][:, 0:2, :].rearrange("p a b -> p (a b)"),
                            h1buf[u][:, 2:4, :].rearrange("p a b -> p (a b)"),
                        )
                    elif ph7 == 4:
                        emit_uw_rot(
                            1, u,
                            h1buf[u][:, 0:2, :].rearrange("p a b -> p (a b)"),
                            h1buf[u][:, 2:4, :].rearrange("p a b -> p (a b)"),
                            h1buf[u][:, 4:6, :],
                        )
                if kt in (13, 20, 27, 30):
                    # pr slot: the ph banks are held by av0 here (a ph
                    # allocation would deadlock the in-order PE queue)
                    u = (13, 20, 27, 30).index(kt)
                    emit_uw_cheb(
                        1, u, h1buf[u][:, 4:6, :],
                        pc=pr.tile([P, 512], F32, tag="sp", name="pc1"),
                    )

            emit_av_flush(0, av0, pend0)
            emit_z(0, av0)
            cs1 = slice(0, NT)
            nc.sync.dma_start(M[1][cs1, 4, :, :], M[1][cs1, 2, :, :])

            # h1 attention with h0's transpose + output projection streamed
            # (out tile s at kt = 6 + 3s//2, i.e. 2 tiles per 3 key tiles)
            _out_sched = {6 + (3 * s) // 2: s for s in range(NS)}
            av1 = [
                ph.tile([P, 6 if j < 2 else 4, 65], F32, tag=f"bank{j}",
                        name=f"av1{j}")
                for j in range(3)
            ]
            pend1 = {}
            for kt in range(NKT):
                for qc in range(NQC):
                    emit_unit(1, kt, qc, av1, pend1)
                if 1 <= kt <= 8:
                    emit_z_tr(0, kt - 1)
                if kt in _out_sched:
                    emit_out_s(0, _out_sched[kt])
            # h1 tail: fused per s-pair pipeline (scale -> transpose ->
            # out matmul -> copy -> batched DMA)
            emit_av_flush(1, av1, pend1)
            zrec1 = emit_z_head(1, av1)
            for s2 in range(NS // 2):
                emit_z_scale(1, av1, zrec1, 2 * s2)
                emit_z_scale(1, av1, zrec1, 2 * s2 + 1)
                emit_z_tr_pe(
                    1, s2, nc.vector.tensor_copy if s2 % 2 else nc.scalar.copy
                )
                emit_out_s(1, 2 * s2)
                emit_out_s(1, 2 * s2 + 1)

    nc.compile()
    return nc


_NC_CACHE = None


def _get_program():
    global _NC_CACHE
    if _NC_CACHE is None:
        _NC_CACHE = build_program()
    return _NC_CACHE


def _fp8_hl(x):
    hi = np.clip(np.asarray(x, np.float32), -240, 240).astype(ml_dtypes.float8_e4m3)
    lo = np.clip(
        np.asarray(x, np.float32) - hi.astype(np.float32), -240, 240
    ).astype(ml_dtypes.float8_e4m3)
    return hi, lo


def make_in_maps(x, history, w_q, w_k, w_v, w_kr, w_o, u_bias, v_bias):
    bf = ml_dtypes.bfloat16
    all_x = np.concatenate([history, x], axis=1)  # [B, HpN, E]

    inv_freq = 1.0 / (10000.0 ** (np.arange(0, E, 2, dtype=np.float64) / E))  # [256]
    ang_f = np.outer(inv_freq[:128], np.arange(HpN, dtype=np.float64) - H)
    xn = (np.arange(HpN, dtype=np.float64) - H) / 2048.0
    T = np.polynomial.chebyshev.chebvander(xn, NT - 1)  # [HpN, NT]
    ang_s = np.outer(xn * 2048.0, inv_freq[128:256])  # [HpN, 128]
    tgt = np.concatenate([np.sin(ang_s), np.cos(ang_s)], axis=1)  # [HpN, 256]
    coef, *_ = np.linalg.lstsq(T, tgt, rcond=None)  # [NT, 256]
    sc = np.ascontiguousarray(coef.T)  # [256, NT]: rows 0-127 sin, 128-255 cos

    sin_hi, _ = _fp8_hl(np.sin(ang_f))
    cos_hi, _ = _fp8_hl(np.cos(ang_f))
    T_hi, T_lo = _fp8_hl(T.T)  # [NT, HpN]
    sin_f = sin_hi.astype(np.float32)
    cos_f = cos_hi.astype(np.float32)
    # SgF partition-major: [p][t][c][j], chunks c = [sin_hi, cos_hi]
    psiF = np.ascontiguousarray(
        np.stack(
            [sin_f.reshape(P, NKT, P), cos_f.reshape(P, NKT, P)], axis=2
        ).reshape(P, NKT * 2 * P)
    )
    # shared cheb T basis [p(64)][hi/lo][t][j] (device places it per head)
    psiT = np.ascontiguousarray(
        np.stack(
            [
                T_hi.astype(np.float32).reshape(NT, NKT, P),
                T_lo.astype(np.float32).reshape(NT, NKT, P),
            ],
            axis=1,
        ).reshape(NT, 2 * NKT * P)
    )
    # fast-psi half-compensation stationary [cos_hi(f0:64)|sin_hi(f64:128)]:
    # pairs with M chunk 5 = [Wlo(f0:64)|Ulo(f64:128)]
    psiC = np.ascontiguousarray(
        np.concatenate([cos_f[0:D], sin_f[D:P]], axis=0).reshape(P, NKT * P)
    )

    ang_b = np.outer(inv_freq, np.arange(N, dtype=np.float64))  # [256, N]
    rot = np.ascontiguousarray(
        np.concatenate([np.cos(ang_b), np.sin(ang_b)]).astype(bf)
    )  # [512, N]: rows 0:128 cos-fast, 128:256 cos-slow, 256:384 sin-fast, ...

    clip8 = lambda a: np.clip(a, -240, 240).astype(ml_dtypes.float8_e4m3)

    in_maps = []
    for c in range(N_CORES):
        b = c // 4
        h0 = HEADS_PER_CORE * (c % 4)
        axT = np.ascontiguousarray(all_x[b].T).astype(bf)
        wq2 = np.concatenate([w_q[h0], w_q[h0 + 1]], axis=1).astype(bf)  # [E, 128]
        wk2 = np.concatenate([w_k[h0], w_k[h0 + 1]], axis=1).astype(bf)
        wv2 = np.concatenate([w_v[h0], w_v[h0 + 1]], axis=1).astype(bf)
        wkrT = np.concatenate(
            [w_kr[h0].T, w_kr[h0 + 1].T], axis=0
        ).astype(bf)  # [128, E]: rows 0:64 = head0 (d), 64:128 = head1
        wo1h = np.stack([w_o[h0], w_o[h0 + 1]], axis=1).reshape(D, 2 * E)
        wo2 = np.concatenate([wo1h, wo1h], axis=0).astype(bf)  # [P, 2E]
        in_maps.append(
            {
                "axT": axT,
                "rot": rot,
                "psiF": clip8(psiF),
                "psiT": clip8(psiT),
                "psiC": clip8(psiC),
                "sc": np.ascontiguousarray(sc).astype(bf),
                "wq2": np.ascontiguousarray(wq2),
                "wk2": np.ascontiguousarray(wk2),
                "wv2": np.ascontiguousarray(wv2),
                "wkrT": np.ascontiguousarray(wkrT),
                "wo2": np.ascontiguousarray(wo2),
                "ub2": np.ascontiguousarray(
                    np.concatenate([u_bias[h0], u_bias[h0 + 1]]).reshape(P, 1)
                ).astype(np.float32),
                "vb2": np.ascontiguousarray(
                    np.concatenate([v_bias[h0], v_bias[h0 + 1]]).reshape(P, 1)
                ).astype(np.float32),
            }
        )
    return in_maps


def run(inputs, trace=False, **kw):
    from concourse.bass_utils import run_bass_kernel_spmd

    nc = _get_program()
    in_maps = make_in_maps(
        np.asarray(inputs["x"], np.float32),
        np.asarray(inputs["history"], np.float32),
        np.asarray(inputs["w_q"], np.float32),
        np.asarray(inputs["w_k"], np.float32),
        np.asarray(inputs["w_v"], np.float32),
        np.asarray(inputs["w_kr"], np.float32),
        np.asarray(inputs["w_o"], np.float32),
        np.asarray(inputs["u_bias"], np.float32),
        np.asarray(inputs["v_bias"], np.float32),
    )
    res = run_bass_kernel_spmd(nc, in_maps, list(range(N_CORES)), trace=trace, **kw)
    out = np.zeros((B, N, E), np.float32)
    for c in range(N_CORES):
        out[c // 4] += res.results[c]["oA"].astype(np.float32).reshape(N, E)
        out[c // 4] += res.results[c]["oB"].astype(np.float32).reshape(N, E)
    return out, res


def kernel(**inputs):
    # mask is all ones (per the problem spec), so score masking is a no-op
    # and the tensor is ignored.
    out, _ = run(inputs, trace=False)
    return out



# revision 98
# speedup vs baseline: 1.2150x; 1.0115x over previous
"""Transformer-XL multi-head self-attention on 8 Trainium2 NeuronCores.

Sharding: core c handles batch b = c//4 and heads {2*(c%4), 2*(c%4)+1}
(data-parallel over B x tensor-parallel over heads). Each core produces a
partial [N, E] output (its heads' w_o contributions); the host sums the 4
partials per batch element.

The XL relative-position term BD[i,j] = (q_i+v)·BDk[j-i+N-1] is computed
without the rel_shift gather via per-query rotation (angle-difference
identities): BD^T = Psi @ UW with Psi a shape-derived constant basis
(128 exact sin rows + 128 exact cos rows + 64 Chebyshev rows for the slow
frequencies) and UW per-query rotated coefficients.

Scores run on the PE in fp8e4 DoubleRow mode (0.5 cycles/row in the cost
model) with hi/lo error compensation: a bf16-accurate operand x is split
as x = hi + lo with hi = fp8(x), lo = fp8(x - hi), keeping selected cross
terms. Per 128-key tile the contraction is 6 chunks of 128 rows consumed
by 3 DoubleRow calls:
  [sin|cos]x[Uhi|Whi],
  [khi|Thi]x[qhi|chi], [klo|Thi]x[qhi|clo], [khi|Tlo]x[qlo|chi], pad
where T/c are the Chebyshev basis/coefficients and k/q carry the content
term (q+u)·k. The U/W (fast psi coefficient) lo-compensation is dropped
(one-sided both psi and U/W): host-side simulation puts the end-to-end
max-rel error at ~1.4% vs the 2% gate (vs ~1.2% with the compensation).
The value path (exp, V, attn@V, output projection) stays in bf16: fp8
noise there does not average out. exp is spread over Act/DVE/Pool
(Schraudolph on DVE/Pool; the extra Schraudolph noise is ~free: ~1.47%
even if every tile uses it).
"""

import sys

sys.path.insert(0, "/opt/trn_rl_repo")

import ml_dtypes
import numpy as np

import concourse.bass as bass
import concourse.mybir as mybir
from concourse import bacc
from concourse.masks import make_identity
from concourse.tile import TileContext

F32 = mybir.dt.float32
BF16 = mybir.dt.bfloat16
FP8 = mybir.dt.float8e4
I16 = mybir.dt.int16
AF = mybir.ActivationFunctionType
ALU = mybir.AluOpType
DR = mybir.MatmulPerfMode.DoubleRow

B, N, H, E, NH, D = 2, 2048, 2048, 512, 8, 64
HpN = H + N  # 4096
P = 128
NKT = HpN // P  # 32 key tiles
NPAIR = NKT // 2  # 16 key-tile pairs
NQC = N // 512  # 4 query chunks of 512
NEC = E // P  # 4 contraction chunks over E
NS = N // P  # 16 output row tiles
NT = 64  # chebyshev terms
HEADS_PER_CORE = 2
N_CORES = 8

LOG2E = 1.4426950408889634
SCORE_SHIFT = 1.5  # exp(s - c): cancels in softmax, bounds exp values
# exp tile engine rotation: (ctr % MOD) -> r < EXP_ACT on Act (exact),
# rest on DVE (Schraudolph). GPSIMD cannot read PSUM so Pool is out.
# Strict alternation: consecutive units' exps overlap across the two
# engines (each engine sees one ~1.1us exp per two 858ns PE units).
EXP_MOD, EXP_ACT = 2, 1


def build_program():
    nc = bacc.Bacc("TRN2", target_bir_lowering=False, debug=False)

    axT_d = nc.declare_dram_parameter("axT", [E, HpN], BF16, isOutput=False)
    rot_d = nc.declare_dram_parameter("rot", [E, N], BF16, isOutput=False)
    # SgF: shared fast-psi chunks, partition-major [p][t][c][j] so the DMA is
    # an identity layout with 8KB per-partition runs
    psiF_d = nc.declare_dram_parameter("psiF", [P, NKT * 2 * P], FP8, isOutput=False)
    # shared cheb T basis rows [p(64)][hi/lo][t][j]; identical for both heads
    # (placed at opposite partition halves on device)
    psiT_d = nc.declare_dram_parameter("psiT", [NT, 2 * NKT * P], FP8, isOutput=False)
    # fast-psi half-compensation stationary [cos_hi(f0:64)|sin_hi(f64:128)],
    # shared by both heads: [p][t][j]
    psiC_d = nc.declare_dram_parameter("psiC", [P, NKT * P], FP8, isOutput=False)
    sc_d = nc.declare_dram_parameter("sc", [2 * P, NT], BF16, isOutput=False)
    wq2_d = nc.declare_dram_parameter("wq2", [E, P], BF16, isOutput=False)
    wk2_d = nc.declare_dram_parameter("wk2", [E, P], BF16, isOutput=False)
    wv2_d = nc.declare_dram_parameter("wv2", [E, P], BF16, isOutput=False)
    wkrT_d = nc.declare_dram_parameter("wkrT", [P, E], BF16, isOutput=False)
    # wo duplicated on both partition halves (odd numT s-tiles live at 64:128)
    wo2_d = nc.declare_dram_parameter("wo2", [P, 2 * E], BF16, isOutput=False)
    ub2_d = nc.declare_dram_parameter("ub2", [P, 1], F32, isOutput=False)
    vb2_d = nc.declare_dram_parameter("vb2", [P, 1], F32, isOutput=False)
    # two per-head partial outputs (host sums): h0 streams during h1's
    # attention; h1 drains at the tail
    oA_d = nc.declare_dram_parameter("oA", [N, E], BF16, isOutput=True)
    oB_d = nc.declare_dram_parameter("oB", [N, E], BF16, isOutput=True)

    with TileContext(nc) as tc:
        with (
            tc.tile_pool(name="persist", bufs=1) as persist,
            tc.tile_pool(name="gst", bufs=4) as gst,       # G copies stream
            tc.tile_pool(name="mst", bufs=2) as mst,       # rotation temps
            tc.tile_pool(name="est", bufs=6) as est,       # exp tiles
            tc.tile_pool(name="dram", bufs=1, space="DRAM") as dram_pool,
            tc.tile_pool(name="pr", bufs=5, space="PSUM") as pr,   # 5x [P,512]
            tc.tile_pool(name="ph", bufs=1, space="PSUM") as ph,   # 3x [P,512]
        ):
            _sm = [0]

            def small_psum(shape, name, dtype=F32, tag=None):
                if tag is None:
                    i = _sm[0] % 3
                    _sm[0] += 1
                    tag = f"bank{i}"
                return ph.tile(shape, dtype, tag=tag, name=name)

            # ---------------- DMAs ----------------
            # One prioritized stream on the sync queue: the DMA engines are a
            # serialized resource, so emission order here IS the priority.
            # q proj needs {wq2, x-half}; the uw chain adds {wkr, rot, sc};
            # emit_k(4..7)/emit_v(x) add {wk2, wv2}; history keys come next,
            # then the attention-only psi tables and wo.
            wq2_s = persist.tile([P, NEC, P], BF16, tag="wq2")
            nc.sync.dma_start(wq2_s[:], wq2_d[:].rearrange("(c p) d -> p c d", p=P))
            # wkr stacked on partitions: rows 0:64 = head0 d, 64:128 = head1 d
            wkr_s = persist.tile([P, NEC, P], BF16, tag="wkr")
            nc.sync.dma_start(
                wkr_s[:], wkrT_d[:].rearrange("p (c e) -> p c e", c=NEC)
            )
            ub_s = persist.tile([P, 1], F32, tag="ub")
            nc.sync.dma_start(ub_s[:], ub2_d[:])
            vb_s = persist.tile([P, 1], F32, tag="vb")
            nc.sync.dma_start(vb_s[:], vb2_d[:])
            axT_s = persist.tile([P, NEC, HpN], BF16, tag="axT", name="axT")
            axT = [axT_s[:, c, :] for c in range(NEC)]
            for r in range(4):
                ks = slice(H + r * 512, H + (r + 1) * 512)
                nc.sync.dma_start(
                    axT_s[:, :, ks],
                    axT_d[:, ks].rearrange("(c p) k -> p c k", p=P),
                )
            rot_s = persist.tile([P, 4, N], BF16, tag="rot")
            nc.sync.dma_start(rot_s[:, 0, :], rot_d[0:P, :])
            nc.sync.dma_start(rot_s[:, 2, :], rot_d[2 * P : 3 * P, :])
            nc.sync.dma_start(rot_s[:, 1, :], rot_d[P : 2 * P, :])
            nc.sync.dma_start(rot_s[:, 3, :], rot_d[3 * P : 4 * P, :])
            sc_s = persist.tile([P, 2, NT], BF16, tag="sc")
            nc.sync.dma_start(sc_s[:], sc_d[:].rearrange("(k p) r -> p k r", p=P))
            wk2_s = persist.tile([P, NEC, P], BF16, tag="wk2")
            nc.sync.dma_start(wk2_s[:], wk2_d[:].rearrange("(c p) d -> p c d", p=P))
            wv2_s = persist.tile([P, NEC, P], BF16, tag="wv2")
            nc.sync.dma_start(wv2_s[:], wv2_d[:].rearrange("(c p) d -> p c d", p=P))
            for r in range(4):
                ks = slice(r * 512, (r + 1) * 512)
                nc.sync.dma_start(
                    axT_s[:, :, ks],
                    axT_d[:, ks].rearrange("(c p) k -> p c k", p=P),
                )
            SgF = persist.tile([P, NKT, 2, P], FP8, tag="SgF")
            nc.sync.dma_start(
                SgF[:], psiF_d[:].rearrange("p (t c j) -> p t c j", c=2, j=P)
            )
            # SgA free layout is chunk-major [c][t][j] so partition-sliced
            # chunk DMAs have 4KB contiguous runs. Only h0's tables load in
            # phase A; h1's are deferred past h0's attention start (the DMA
            # engines are a serialized resource on the startup critical path).
            SgA = []
            for h in range(HEADS_PER_CORE):
                t = persist.tile([P, 4, NKT, P], FP8, tag=f"SgA{h}", name=f"SgA{h}")
                SgA.append(t)

            def emit_sga_tables(h):
                tp = (1 - h) * D
                tps = slice(tp, tp + NT)
                nc.sync.dma_start(
                    SgA[h][tps, 0, :, :],
                    psiT_d[:, 0 : NKT * P].rearrange("p (t j) -> p t j", j=P),
                )
                nc.sync.dma_start(
                    SgA[h][tps, 2, :, :],
                    psiT_d[:, NKT * P :].rearrange("p (t j) -> p t j", j=P),
                )
                nc.sync.dma_start(
                    SgA[h][:, 3, :, :],
                    psiC_d[:].rearrange("p (t j) -> p t j", j=P),
                )
                # chunk1's T-half duplicates chunk0's (device-side dup)
                nc.sync.dma_start(SgA[h][tps, 1, :, :], SgA[h][tps, 0, :, :])

            emit_sga_tables(0)
            wo_s = persist.tile([P, 2, E], BF16, tag="wo")
            nc.sync.dma_start(wo_s[:], wo2_d[:].rearrange("p (h e) -> p h e", h=2))

            identb = persist.tile([P, P], BF16, tag="identb")
            make_identity(nc, identb[:])

            # ---------------- persistent compute tiles ----------------
            # M chunks per head: 0=Uhi 1=Whi 2=[qhi|chi]
            # 3=[qhi-dup|clo] 4=[qlo|chi-dup] 5=[Wlo(f 0:64)|Ulo(f 64:128)]
            # (chunk 5 pairs with the psiA half-compensation stationary
            # [cos_hi(0:64)|sin_hi(64:128)] in the otherwise-wasted pad slot)
            M = []
            for h in range(HEADS_PER_CORE):
                m = persist.tile([P, 6, NQC, 512], FP8, tag=f"M{h}", name=f"M{h}")
                M.append(m)
            qv_s = persist.tile([P, N], BF16, tag="qv_s")
            vo = []
            for h in range(HEADS_PER_CORE):
                v = persist.tile([P, NKT, 66], BF16, tag=f"vo{h}", name=f"vo{h}")
                nc.gpsimd.memset(v[:, :, 64:66], 0.0)
                nc.gpsimd.memset(v[:, :, 64:65], 1.0)
                vo.append(v)
            # numTT: query-major pre-scaled numerators [q, s, d] (z separate);
            # numT: d-major via 128x128 transposes of s-tile PAIRS -- even
            # s-tile's d on partitions 0:64, odd on 64:128
            numT = []
            numTT = []
            zcs = []
            for h in range(HEADS_PER_CORE):
                t = persist.tile(
                    [P, NS // 2, P], BF16, tag=f"numT{h}", name=f"numT{h}"
                )
                numT.append(t)
                tt = persist.tile(
                    [P, NS, D], BF16, tag=f"numTT{h}", name=f"numTT{h}"
                )
                numTT.append(tt)
                zcs.append(
                    persist.tile([P, NS], F32, tag=f"zc{h}", name=f"zc{h}")
                )
            out_acc = persist.tile([P, NS, E], BF16, tag="out_acc")
            nbias = persist.tile([P, 1], F32, tag="nbias")
            nc.vector.memset(nbias[:], -SCORE_SHIFT)

            # ---------------- phase A: projections ----------------
            # q projection, both heads packed, emitted chunk-outer so the PE
            # starts as soon as each axT chunk lands. pq psums use the ph
            # banks (free until the av accumulators take them).
            pqs = [small_psum([P, 512], f"pq{qc}") for qc in range(NQC)]
            for qc in range(NQC):
                for c in range(NEC):
                    nc.tensor.matmul(
                        pqs[qc][:],
                        wq2_s[:, c, :],
                        axT[c][:, H + qc * 512 : H + (qc + 1) * 512],
                        start=(c == 0),
                        stop=(c == NEC - 1),
                    )
            for qc in range(NQC):
                pq = pqs[qc]
                qs = slice(qc * 512, (qc + 1) * 512)
                nc.vector.tensor_scalar_add(qv_s[:, qs], pq[:], vb_s[:])
                for h in range(HEADS_PER_CORE):
                    hp = slice(h * D, (h + 1) * D)
                    nc.vector.tensor_scalar_add(
                        M[h][hp, 2, qc, :], pq[hp, :], ub_s[hp]
                    )
                    nc.vector.scalar_tensor_tensor(
                        M[h][hp, 4, qc, :], pq[hp, :], ub_s[hp],
                        M[h][hp, 2, qc, :], ALU.add, ALU.subtract,
                    )

            def emit_uw_g_chunk(h, qc, j, sfd, ssd):
                # G: e 0:128 sin-fast + 256:384 cos-fast (sf);
                #    e 128:256 sin-slow + 384:512 cos-slow (ss)
                # one 1-bank psum + copy per chunk so at most one score-stream
                # slot is borrowed at a time
                hp = slice(h * D, (h + 1) * D)
                qs = slice(qc * 512, (qc + 1) * 512)
                half, jj = j // 2, j % 2
                dst = sfd if half == 0 else ssd
                g = pr.tile([P, 512], F32, tag="sp", name="g")
                nc.tensor.matmul(
                    g[:], wkr_s[hp, 2 * jj + half, :], qv_s[hp, qs],
                    start=True, stop=True,
                )
                nc.scalar.copy(dst[:, jj * 512 : (jj + 1) * 512], g[:])

            def emit_uw_g(h, qc, sfd, ssd):
                for j in range(4):
                    emit_uw_g_chunk(h, qc, j, sfd, ssd)

            def emit_uw_rot(h, qc, sf, ss, usw):
                qs = slice(qc * 512, (qc + 1) * 512)
                # h0 runs in phase A: muls 6 DVE / 2 Pool, M copies on the
                # idle Act. h1 runs during h0's attention where DVE only has
                # 3/8 exps: muls all DVE (fast), every op downstream of a
                # mul on Pool so the DVE queue never waits cross-engine.
                V, G, A = nc.vector, nc.gpsimd, nc.scalar
                if h == 0:
                    me = (V, V, V, G, V, V, V, G)
                    c0, c1 = A.copy, V.tensor_copy
                    s5 = V.tensor_sub
                else:
                    me = (V, V, V, V, V, V, V, V)
                    c0 = c1 = G.tensor_copy
                    s5 = G.tensor_sub
                # fast half: U = G*cos + Gc*sin ; W = Gc*cos - G*sin
                m1 = mst.tile([P, 512], BF16, tag="m1")
                m2 = mst.tile([P, 512], BF16, tag="m2")
                m3 = mst.tile([P, 512], BF16, tag="m3")
                m4 = mst.tile([P, 512], BF16, tag="m4")
                me[0].tensor_mul(m1[:], sf[:, 0:512], rot_s[:, 0, qs])
                me[1].tensor_mul(m2[:], sf[:, 512:1024], rot_s[:, 2, qs])
                me[2].tensor_mul(m3[:], sf[:, 512:1024], rot_s[:, 0, qs])
                me[3].tensor_mul(m4[:], sf[:, 0:512], rot_s[:, 2, qs])
                ubf = mst.tile([P, 512], BF16, tag="ubf")
                wbf = mst.tile([P, 512], BF16, tag="wbf")
                nc.gpsimd.tensor_add(ubf[:], m1[:], m2[:])
                nc.gpsimd.tensor_sub(wbf[:], m3[:], m4[:])
                c0(M[h][:, 0, qc, :], ubf[:])
                c1(M[h][:, 1, qc, :], wbf[:])
                # half lo-comp into the pad slot (partition-aligned halves)
                s5(M[h][0:D, 5, qc, :], wbf[0:D, :], M[h][0:D, 1, qc, :])
                s5(M[h][D:P, 5, qc, :], ubf[D:P, :], M[h][D:P, 0, qc, :])
                # slow half: rotate; compression happens in emit_uw_cheb
                m5 = mst.tile([P, 512], BF16, tag="m1", name="m5")
                m6 = mst.tile([P, 512], BF16, tag="m2", name="m6")
                m7 = mst.tile([P, 512], BF16, tag="m3", name="m7")
                m8 = mst.tile([P, 512], BF16, tag="m4", name="m8")
                me[4].tensor_mul(m5[:], ss[:, 0:512], rot_s[:, 1, qs])
                me[5].tensor_mul(m6[:], ss[:, 512:1024], rot_s[:, 3, qs])
                me[6].tensor_mul(m7[:], ss[:, 512:1024], rot_s[:, 1, qs])
                me[7].tensor_mul(m8[:], ss[:, 0:512], rot_s[:, 3, qs])
                nc.gpsimd.tensor_add(usw[:, 0, :], m5[:], m6[:])
                nc.gpsimd.tensor_sub(usw[:, 1, :], m7[:], m8[:])

            def emit_uw_cheb(h, qc, usw, pc=None):
                # cheb coefs land on the head's opposite partition half
                po = (1 - h) * D
                cs = slice(po, po + NT)
                if pc is None:
                    pc = small_psum([P, 512], "pc")
                for k in range(2):
                    nc.tensor.matmul(
                        pc[cs, :], sc_s[:, k, :], usw[:, k, :],
                        start=(k == 0), stop=(k == 1),
                    )
                nc.scalar.copy(M[h][cs, 2, qc, :], pc[cs, :])
                nc.vector.tensor_sub(
                    M[h][cs, 3, qc, :], pc[cs, :], M[h][cs, 2, qc, :]
                )

            def emit_k(kc):
                pk = small_psum([P, 512], "pk")
                for c in range(NEC):
                    nc.tensor.matmul(
                        pk[:],
                        wk2_s[:, c, :],
                        axT[c][:, kc * 512 : (kc + 1) * 512],
                        start=(c == 0),
                        stop=(c == NEC - 1),
                    )
                ks = slice(4 * kc, 4 * kc + 4)
                for h in range(HEADS_PER_CORE):
                    hp = slice(h * D, (h + 1) * D)
                    pkv = pk[hp, :].rearrange("p (t j) -> p t j", j=P)
                    nc.scalar.copy(SgA[h][hp, 0, ks, :], pkv)
                    nc.vector.tensor_sub(
                        SgA[h][hp, 1, ks, :], pkv, SgA[h][hp, 0, ks, :]
                    )

            def emit_v(h, g, tag=None):
                hs = slice(h * D, (h + 1) * D)
                pv = small_psum([P, 512], "pv", tag=tag)
                for k8 in range(8):
                    kt = g * 8 + k8
                    for c in range(NEC):
                        nc.tensor.matmul(
                            pv[:, k8 * D : (k8 + 1) * D],
                            axT[c][:, kt * P : (kt + 1) * P],
                            wv2_s[:, c, hs],
                            start=(c == 0),
                            stop=(c == NEC - 1),
                        )
                cp = nc.scalar.copy if (h + g) % 2 else nc.vector.tensor_copy
                cp(
                    vo[h][:, g * 8 : (g + 1) * 8, 0:D],
                    pv[:].rearrange("p (t d) -> p t d", d=D),
                )

            # h0 UW fully in phase A (streaming); h1's G copies land in a
            # persistent tile recycled from axT's tag so h1's rotation
            # (engine-only) can run during h0's attention.
            h1b = persist.tile(
                [P, NQC, 6, 512], BF16, tag="axT", name="h1buf"
            )
            h1buf = [h1b[:, u, :, :] for u in range(NQC)]

            # x keys (kc 4..7, v groups 2..3) first: their axT DMA lands well
            # before the history half. All G emissions go early (their
            # rotations then stream on DVE/Pool); chebs are deferred so the
            # PE never waits on a rotation.
            uwt = []
            for u in range(NQC):
                sf = gst.tile([P, 1024], BF16, tag="sf")
                ss = gst.tile([P, 1024], BF16, tag="ss")
                usw = gst.tile([P, 2, 512], BF16, tag="usw")
                uwt.append((sf, ss, usw))
                emit_uw_g(0, u, sf[:], ss[:])
                emit_uw_rot(0, u, sf, ss, usw)
                emit_k(4 + u)
            for g in (2, 3):
                emit_v(0, g)
                emit_v(1, g)
            emit_uw_cheb(0, 0, uwt[0][2])
            emit_uw_cheb(0, 1, uwt[1][2])
            for u in range(NQC):
                emit_k(u)
            emit_uw_cheb(0, 2, uwt[2][2])
            for g in (0, 1):
                emit_v(0, g)
                emit_v(1, g)
            emit_uw_cheb(0, 3, uwt[3][2])

            # dups via DMA (off-engine): M chunk 3 q-half <- chunk 2 q-half;
            # M chunk 4 cheb-half <- chunk 2 cheb-half (h0 now, h1 after its
            # cheb block); SgA chunk 2 k-half <- chunk 0. h1's wait until
            # h0's attention is underway (serial DMA resource).
            nc.sync.dma_start(M[0][0:D, 3, :, :], M[0][0:D, 2, :, :])
            nc.sync.dma_start(SgA[0][0:D, 2, :, :], SgA[0][0:D, 0, :, :])
            cs0 = slice(D, D + NT)
            nc.sync.dma_start(M[0][cs0, 4, :, :], M[0][cs0, 2, :, :])

            # ---------------- phase B: attention ----------------
            # Unit = one (key tile, query chunk): score psum is a 1-bank
            # [P, 512] tile from the 5-deep pr pool, so the
            # ps -> exp -> frees-slot chain never stalls the PE. exp
            # alternates Act (exact) / DVE (Schraudolph) per unit; during
            # h0's attention DVE also carries h1's rotation, so it only
            # takes 3 of 8 exps there.
            _expctr = [0]
            _dve_exp = {0: (1, 3, 5, 7), 1: (1, 3, 5, 7)}

            def emit_av(h, kt, qc, pE, avv):
                for qt in range(4):
                    qg = qc * 4 + qt
                    bk, sl = divmod(qg, 6)
                    nc.tensor.matmul(
                        avv[bk][:, sl, :],
                        pE[:, qt * P : (qt + 1) * P],
                        vo[h][:, kt, 0:65],
                        start=(kt == 0 and qg in (0, 6, 12)),
                        stop=(kt == NKT - 1 and qg in (5, 11, 15)),
                        skip_group_check=True,
                    )

            def emit_unit(h, kt, qc, avv, pend):
                ps = pr.tile([P, 512], F32, tag="sp", name="ps")
                nc.tensor.matmul(
                    ps[:], SgF[:, kt, :, :], M[h][:, 0:2, qc, :],
                    start=True, stop=False, perf_mode=DR,
                )
                nc.tensor.matmul(
                    ps[:], SgA[h][:, 0:2, kt, :], M[h][:, 2:4, qc, :],
                    start=False, stop=False, perf_mode=DR,
                )
                nc.tensor.matmul(
                    ps[:], SgA[h][:, 2:4, kt, :], M[h][:, 4:6, qc, :],
                    start=False, stop=True, perf_mode=DR,
                )
                if qc in pend:
                    pkt, pE = pend.pop(qc)
                    emit_av(h, pkt, qc, pE, avv)
                et = est.tile([P, 512], BF16, tag="E")
                if _expctr[0] % 8 not in _dve_exp[h]:
                    nc.scalar.activation(
                        et[:], ps[:], AF.Exp, scale=0.125, bias=nbias[:]
                    )
                else:
                    # Schraudolph: int16 bits = 128*(log2e*(s/8 - c) + 127)
                    nc.vector.tensor_scalar(
                        et[:].bitcast(I16), ps[:],
                        0.125 * P * LOG2E,
                        P * 127.0 - SCORE_SHIFT * P * LOG2E - 8.5,
                        ALU.mult, ALU.add,
                    )
                _expctr[0] += 1
                pend[qc] = (kt, et)

            def emit_av_flush(h, avv, pend):
                for qc, (pkt, pE) in sorted(pend.items()):
                    emit_av(h, pkt, qc, pE, avv)
                pend.clear()

            def emit_z_head(h, avv):
                # av is query-major with the ones-column z in slot 64: copy
                # the z columns and take the per-partition reciprocal
                zc = zcs[h]
                nc.vector.tensor_copy(zc[:, 0:6], avv[0][:, :, 64])
                nc.vector.tensor_copy(zc[:, 6:12], avv[1][:, :, 64])
                nc.vector.tensor_copy(zc[:, 12:16], avv[2][:, :, 64])
                zrec = persist.tile([P, NS], F32, tag=f"zrec{h}", name=f"zrec{h}")
                nc.vector.reciprocal(zrec[:], zc[:])
                return zrec

            def emit_z_scale(h, avv, zrec, s):
                # write numTT PRE-SCALED by 1/z (per-partition scalar per
                # s-tile) so the out-projection result needs no scaling
                bk, sl = divmod(s, 6)
                if s % 2 == 0:
                    nc.scalar.activation(
                        numTT[h][:, s, :], avv[bk][:, sl, 0:D], AF.Copy,
                        scale=zrec[:, s : s + 1],
                    )
                else:
                    nc.vector.tensor_scalar_mul(
                        numTT[h][:, s, :], avv[bk][:, sl, 0:D],
                        zrec[:, s : s + 1],
                    )

            def emit_z(h, avv):
                zrec = emit_z_head(h, avv)
                for s in range(NS):
                    emit_z_scale(h, avv, zrec, s)
                return zrec

            def emit_z_tr(h, s2):
                # transpose one PAIR of numerator s-tiles ([128,128] block)
                # back to d-major via the DMA xbar (off-engine)
                nc.sync.dma_start_transpose(
                    numT[h][:, s2, :],
                    numTT[h][:, 2 * s2 : 2 * s2 + 2, :],
                )

            def emit_z_tr_pe(h, s2, copy_eng):
                pz = pr.tile([P, P], BF16, tag="sp", name="pz")
                nc.tensor.transpose(
                    pz[:], numTT[h][:, 2 * s2 : 2 * s2 + 2, :], identb[:]
                )
                copy_eng(numT[h][:, s2, :], pz[:])

            def emit_out_s(h, s):
                # numT is pre-scaled by 1/z, so the psum->sbuf conversion is
                # a plain copy (alternating Act/DVE to spread the load)
                po = pr.tile([P, 512], F32, tag="sp", name="po")
                hp = (s % 2) * D
                nc.tensor.matmul(
                    po[:], numT[h][hp : hp + D, s // 2, :],
                    wo_s[hp : hp + D, h, :],
                    start=True, stop=True,
                )
                if s % 2 == 0:
                    nc.scalar.copy(out_acc[:, s, :], po[:])
                else:
                    nc.vector.tensor_copy(out_acc[:, s, :], po[:])
                if h == 0:
                    nc.sync.dma_start(
                        oA_d[:].rearrange("(s p) e -> p s e", p=P)[:, s, :],
                        out_acc[:, s, :],
                    )
                elif s % 4 == 3:
                    # h1 drains at the tail: batch 4 s-tiles per DMA
                    nc.sync.dma_start(
                        oB_d[:].rearrange("(s p) e -> p s e", p=P)[:, s - 3 : s + 1, :],
                        out_acc[:, s - 3 : s + 1, :],
                    )

            # h0 attention with h1's G/rotation/cheb interleaved (their
            # elementwise runs on Pool/Act; DVE carries the exp stream)
            av0 = [
                ph.tile([P, 6 if j < 2 else 4, 65], F32, tag=f"bank{j}",
                        name=f"av0{j}")
                for j in range(3)
            ]
            pend0 = {}
            for kt in range(NKT):
                for qc in range(NQC):
                    emit_unit(0, kt, qc, av0, pend0)
                if kt == 0:
                    # h1's psi tables + dups, now that h0's attention flows
                    emit_sga_tables(1)
                    nc.sync.dma_start(M[1][D:P, 3, :, :], M[1][D:P, 2, :, :])
                    nc.sync.dma_start(SgA[1][D:P, 2, :, :], SgA[1][D:P, 0, :, :])
                # h1 prep spread: per u-block of 7 kts, one G chunk per kt
                # (each borrows one score-stream slot briefly), then the
                # rotation (engine-only); cheb trails the rotation by a full
                # block so the Pool finishing ops have drained.
                if 1 <= kt <= 28:
                    u, ph7 = divmod(kt - 1, 7)
                    if ph7 < 4:
                        emit_uw_g_chunk(
                            1, u, ph7,
                            h1buf[u][:, 0:2, :].rearrange("p a b -> p (a b)"),
                            h1buf[u][:, 2:4, :].rearrange("p a b -> p (a b)"),
                        )
                    elif ph7 == 4:
                        emit_uw_rot(
                            1, u,
                            h1buf[u][:, 0:2, :].rearrange("p a b -> p (a b)"),
                            h1buf[u][:, 2:4, :].rearrange("p a b -> p (a b)"),
                            h1buf[u][:, 4:6, :],
                        )
                if kt in (13, 20, 27, 30):
                    # pr slot: the ph banks are held by av0 here (a ph
                    # allocation would deadlock the in-order PE queue)
                    u = (13, 20, 27, 30).index(kt)
                    emit_uw_cheb(
                        1, u, h1buf[u][:, 4:6, :],
                        pc=pr.tile([P, 512], F32, tag="sp", name="pc1"),
                    )

            emit_av_flush(0, av0, pend0)
            emit_z(0, av0)
            cs1 = slice(0, NT)
            nc.sync.dma_start(M[1][cs1, 4, :, :], M[1][cs1, 2, :, :])

            # h1 attention with h0's transpose + output projection streamed
            # (out tile s at kt = 6 + 3s//2, i.e. 2 tiles per 3 key tiles)
            _out_sched = {6 + (3 * s) // 2: s for s in range(NS)}
            av1 = [
                ph.tile([P, 6 if j < 2 else 4, 65], F32, tag=f"bank{j}",
                        name=f"av1{j}")
                for j in range(3)
            ]
            pend1 = {}
            for kt in range(NKT):
                for qc in range(NQC):
                    emit_unit(1, kt, qc, av1, pend1)
                if 1 <= kt <= 8:
                    emit_z_tr(0, kt - 1)
                if kt in _out_sched:
                    emit_out_s(0, _out_sched[kt])
            # h1 tail: fused per s-pair pipeline (scale -> transpose ->
            # out matmul -> copy -> batched DMA)
            emit_av_flush(1, av1, pend1)
            zrec1 = emit_z_head(1, av1)
            for s2 in range(NS // 2):
                emit_z_scale(1, av1, zrec1, 2 * s2)
                emit_z_scale(1, av1, zrec1, 2 * s2 + 1)
                emit_z_tr_pe(
                    1, s2, nc.vector.tensor_copy if s2 % 2 else nc.scalar.copy
                )
                emit_out_s(1, 2 * s2)
                emit_out_s(1, 2 * s2 + 1)

    nc.compile()
    return nc


_NC_CACHE = None


def _get_program():
    global _NC_CACHE
    if _NC_CACHE is None:
        _NC_CACHE = build_program()
    return _NC_CACHE


def _fp8_hl(x):
    hi = np.clip(np.asarray(x, np.float32), -240, 240).astype(ml_dtypes.float8_e4m3)
    lo = np.clip(
        np.asarray(x, np.float32) - hi.astype(np.float32), -240, 240
    ).astype(ml_dtypes.float8_e4m3)
    return hi, lo


def make_in_maps(x, history, w_q, w_k, w_v, w_kr, w_o, u_bias, v_bias):
    bf = ml_dtypes.bfloat16
    all_x = np.concatenate([history, x], axis=1)  # [B, HpN, E]

    inv_freq = 1.0 / (10000.0 ** (np.arange(0, E, 2, dtype=np.float64) / E))  # [256]
    ang_f = np.outer(inv_freq[:128], np.arange(HpN, dtype=np.float64) - H)
    xn = (np.arange(HpN, dtype=np.float64) - H) / 2048.0
    T = np.polynomial.chebyshev.chebvander(xn, NT - 1)  # [HpN, NT]
    ang_s = np.outer(xn * 2048.0, inv_freq[128:256])  # [HpN, 128]
    tgt = np.concatenate([np.sin(ang_s), np.cos(ang_s)], axis=1)  # [HpN, 256]
    coef, *_ = np.linalg.lstsq(T, tgt, rcond=None)  # [NT, 256]
    sc = np.ascontiguousarray(coef.T)  # [256, NT]: rows 0-127 sin, 128-255 cos

    sin_hi, _ = _fp8_hl(np.sin(ang_f))
    cos_hi, _ = _fp8_hl(np.cos(ang_f))
    T_hi, T_lo = _fp8_hl(T.T)  # [NT, HpN]
    sin_f = sin_hi.astype(np.float32)
    cos_f = cos_hi.astype(np.float32)
    # SgF partition-major: [p][t][c][j], chunks c = [sin_hi, cos_hi]
    psiF = np.ascontiguousarray(
        np.stack(
            [sin_f.reshape(P, NKT, P), cos_f.reshape(P, NKT, P)], axis=2
        ).reshape(P, NKT * 2 * P)
    )
    # shared cheb T basis [p(64)][hi/lo][t][j] (device places it per head)
    psiT = np.ascontiguousarray(
        np.stack(
            [
                T_hi.astype(np.float32).reshape(NT, NKT, P),
                T_lo.astype(np.float32).reshape(NT, NKT, P),
            ],
            axis=1,
        ).reshape(NT, 2 * NKT * P)
    )
    # fast-psi half-compensation stationary [cos_hi(f0:64)|sin_hi(f64:128)]:
    # pairs with M chunk 5 = [Wlo(f0:64)|Ulo(f64:128)]
    psiC = np.ascontiguousarray(
        np.concatenate([cos_f[0:D], sin_f[D:P]], axis=0).reshape(P, NKT * P)
    )

    ang_b = np.outer(inv_freq, np.arange(N, dtype=np.float64))  # [256, N]
    rot = np.ascontiguousarray(
        np.concatenate([np.cos(ang_b), np.sin(ang_b)]).astype(bf)
    )  # [512, N]: rows 0:128 cos-fast, 128:256 cos-slow, 256:384 sin-fast, ...

    clip8 = lambda a: np.clip(a, -240, 240).astype(ml_dtypes.float8_e4m3)

    in_maps = []
    for c in range(N_CORES):
        b = c // 4
        h0 = HEADS_PER_CORE * (c % 4)
        axT = np.ascontiguousarray(all_x[b].T).astype(bf)
        wq2 = np.concatenate([w_q[h0], w_q[h0 + 1]], axis=1).astype(bf)  # [E, 128]
        wk2 = np.concatenate([w_k[h0], w_k[h0 + 1]], axis=1).astype(bf)
        wv2 = np.concatenate([w_v[h0], w_v[h0 + 1]], axis=1).astype(bf)
        wkrT = np.concatenate(
            [w_kr[h0].T, w_kr[h0 + 1].T], axis=0
        ).astype(bf)  # [128, E]: rows 0:64 = head0 (d), 64:128 = head1
        wo1h = np.stack([w_o[h0], w_o[h0 + 1]], axis=1).reshape(D, 2 * E)
        wo2 = np.concatenate([wo1h, wo1h], axis=0).astype(bf)  # [P, 2E]
        in_maps.append(
            {
                "axT": axT,
                "rot": rot,
                "psiF": clip8(psiF),
                "psiT": clip8(psiT),
                "psiC": clip8(psiC),
                "sc": np.ascontiguousarray(sc).astype(bf),
                "wq2": np.ascontiguousarray(wq2),
                "wk2": np.ascontiguousarray(wk2),
                "wv2": np.ascontiguousarray(wv2),
                "wkrT": np.ascontiguousarray(wkrT),
                "wo2": np.ascontiguousarray(wo2),
                "ub2": np.ascontiguousarray(
                    np.concatenate([u_bias[h0], u_bias[h0 + 1]]).reshape(P, 1)
                ).astype(np.float32),
                "vb2": np.ascontiguousarray(
                    np.concatenate([v_bias[h0], v_bias[h0 + 1]]).reshape(P, 1)
                ).astype(np.float32),
            }
        )
    return in_maps


def run(inputs, trace=False, **kw):
    from concourse.bass_utils import run_bass_kernel_spmd

    nc = _get_program()
    in_maps = make_in_maps(
        np.asarray(inputs["x"], np.float32),
        np.asarray(inputs["history"], np.float32),
        np.asarray(inputs["w_q"], np.float32),
        np.asarray(inputs["w_k"], np.float32),
        np.asarray(inputs["w_v"], np.float32),
        np.asarray(inputs["w_kr"], np.float32),
        np.asarray(inputs["w_o"], np.float32),
        np.asarray(inputs["u_bias"], np.float32),
        np.asarray(inputs["v_bias"], np.float32),
    )
    res = run_bass_kernel_spmd(nc, in_maps, list(range(N_CORES)), trace=trace, **kw)
    out = np.zeros((B, N, E), np.float32)
    for c in range(N_CORES):
        out[c // 4] += res.results[c]["oA"].astype(np.float32).reshape(N, E)
        out[c // 4] += res.results[c]["oB"].astype(np.float32).reshape(N, E)
    return out, res


def kernel(**inputs):
    # mask is all ones (per the problem spec), so score masking is a no-op
    # and the tensor is ignored.
    out, _ = run(inputs, trace=False)
    return out

